# revision 1
# baseline (speedup 1.0000x reference)
"""Trainium2 Bass kernel for HIVNet GCN message passing (8-core SPMD).

Strategy:
  - Pad N=10000 nodes to 10240 = 80 blocks x 128; core c owns 10 dst-blocks.
  - Per layer: hs = h*rsqrt(deg) (per-node scale), hws = hs @ W[l] computed on
    the owned shard, AllGather of bf16 hws into a DRAM table on every core.
  - Edge aggregation: edges (with self loops) sorted by dst; per dst-block a
    bulk dma_gather pulls the src rows (bf16, 512B each) into SBUF tiles
    [128 edges, 256]; one-hot "sel" matrices (host-built, bf16) reduce each
    128-edge tile onto the 128 dst rows via TensorE matmuls accumulated in
    PSUM.  t = nrm * segsum(hws[src]) applied via per-partition ACT scale.
  - BatchNorm: partial sums/sumsq per core -> 2KB AllReduce -> scale/shift
    broadcast via rank-1 TensorE matmul; relu + residual on DVE.
  - Readout: graph mean-pool via one-hot pool matrices (transposed layout so
    MLP runs with weights as lhsT), 257-row AllReduce, 3-layer MLP on core 0.
"""

import sys

sys.path.insert(0, "/opt/trn_rl_repo")

from contextlib import ExitStack

import numpy as np
import ml_dtypes

from concourse import bass, mybir, bacc, tile, library_config
from concourse.bass_utils import run_bass_kernel_spmd
from concourse.masks import make_identity

NCORE = 8
P = 128
H = 256
L = 4
NF = 9
G = 256
N = 10000
BPC = 10                # dst blocks per core
NPC = BPC * P           # 1280 nodes per core
NPAD = NCORE * NPC      # 10240
BN_EPS = 1e-5

f32 = mybir.dt.float32
bf16 = mybir.dt.bfloat16
i16 = mybir.dt.int16
bfnp = ml_dtypes.bfloat16

FT = mybir.ActivationFunctionType
OP = mybir.AluOpType

_compiled = {}


# --------------------------------------------------------------------------
# host-side structural preprocessing (sorting / padding / one-hot layout)
# --------------------------------------------------------------------------

def _preprocess(x, edge_index, batch_ids, emb, W, gamma, beta,
                mlp_W1, mlp_b1, mlp_W2, mlp_b2, mlp_W3, mlp_b3):
    src = np.asarray(edge_index[0], np.int64)
    dst = np.asarray(edge_index[1], np.int64)
    # self loops for every real node (weight nrm[d]^2 == nrm[d]*nrm[d] folds in)
    src_all = np.concatenate([src, np.arange(N, dtype=np.int64)])
    dst_all = np.concatenate([dst, np.arange(N, dtype=np.int64)])
    order = np.argsort(dst_all, kind="stable")
    s_sorted = src_all[order].astype(np.int64)
    d_sorted = dst_all[order]

    deg = np.bincount(dst_all, minlength=NPAD).astype(np.float64)  # incl self
    nblk = NCORE * BPC
    cnt_blk = np.bincount(d_sorted // P, minlength=nblk)
    T_blk = int(np.ceil(cnt_blk.max() / P))
    NI = T_blk * P
    NIB = NI // 16

    idx_slots = np.zeros((nblk, NI), np.int16)
    dloc = np.full((nblk, NI), -1, np.int32)
    starts = np.searchsorted(d_sorted, np.arange(nblk) * P)
    ends = np.searchsorted(d_sorted, (np.arange(nblk) + 1) * P)
    for g in range(nblk):
        c = ends[g] - starts[g]
        idx_slots[g, :c] = s_sorted[starts[g]:ends[g]]
        dloc[g, :c] = d_sorted[starts[g]:ends[g]] - g * P

    # one-hot sel: [blk, T_blk, 128 slots, 128 dst_local] bf16
    sel = (dloc.reshape(nblk, T_blk, P)[..., None]
           == np.arange(P, dtype=np.int32)).astype(bfnp)

    # graph pool one-hot [node, graph]
    bids = np.asarray(batch_ids, np.int64)
    psel_full = np.zeros((NPAD, G), np.float32)
    psel_full[np.arange(N), bids] = 1.0

    x_np = np.zeros((NPAD, NF), np.float32)
    x_np[:N] = np.asarray(x, np.float64)

    # shared parameter tensors (layout for device)
    Wf = np.asarray(W, np.float32)                       # [L,H,H]
    W_lhsT = Wf.reshape(L, 2, P, H).transpose(2, 0, 1, 3).reshape(P, L * 2 * H)
    gb = np.concatenate([np.asarray(gamma, np.float32).reshape(-1),
                         np.asarray(beta, np.float32).reshape(-1)])[None, :]
    embf = np.asarray(emb, np.float32)
    emb0 = np.ascontiguousarray(embf[:, 0, :])
    emb1 = np.ascontiguousarray(embf[:, 1, :])
    w1 = np.asarray(mlp_W1, np.float32).reshape(2, P, P).transpose(1, 0, 2).reshape(P, 2 * P)
    w2 = np.asarray(mlp_W2, np.float32)                  # [128,64]
    w3 = np.asarray(mlp_W3, np.float32)                  # [64,1]
    b1 = np.asarray(mlp_b1, np.float32).reshape(P, 1)
    b2 = np.asarray(mlp_b2, np.float32).reshape(64, 1)
    b3 = np.asarray(mlp_b3, np.float32).reshape(1, 1)

    in_maps = []
    for c in range(NCORE):
        lo, hi = c * NPC, (c + 1) * NPC
        gsl = slice(c * BPC, (c + 1) * BPC)

        selc = sel[gsl].reshape(BPC * T_blk, P, P)
        selc = np.ascontiguousarray(selc.transpose(1, 0, 2)).reshape(P, BPC * T_blk * P)

        idxc = idx_slots[gsl].reshape(BPC, NI // 16, 16)
        idxc = idxc.transpose(0, 2, 1)                    # [BPC, 16, NI/16]
        idxc = np.tile(idxc, (1, 8, 1))                   # replicate to 128 parts
        idxc = np.ascontiguousarray(idxc.transpose(1, 0, 2)).reshape(P, BPC * NIB)

        degc = deg[lo:hi].reshape(BPC, P).T               # [P, BPC]
        maskc = (degc > 0).astype(np.float32)
        degc = np.maximum(degc, 1.0).astype(np.float32)

        pselc = psel_full[lo:hi].reshape(BPC, P, G)
        pselc = np.ascontiguousarray(pselc.transpose(1, 0, 2)).reshape(P, BPC * G)

        xTc = np.ascontiguousarray(x_np[lo:hi].T)         # [NF, NPC]

        in_maps.append(dict(
            selw=selc.astype(bfnp), idx=idxc.astype(np.int16),
            xT=xTc, deg=degc, mask=maskc, psel=pselc,
            W=W_lhsT.astype(bfnp), gb=gb, emb0=emb0, emb1=emb1,
            w1=w1, w2=w2, w3=w3, b1=b1, b2=b2, b3=b3,
        ))
    return T_blk, in_maps


# --------------------------------------------------------------------------
# device program
# --------------------------------------------------------------------------

def _build(T_blk, variant='full'):
    NI = T_blk * P
    NIB = NI // 16
    nc = bacc.Bacc(None, target_bir_lowering=False)

    d_sel = nc.dram_tensor("selw", [P, BPC * T_blk * P], bf16, kind="ExternalInput")
    d_idx = nc.dram_tensor("idx", [P, BPC * NIB], i16, kind="ExternalInput")
    d_xT = nc.dram_tensor("xT", [NF, NPC], f32, kind="ExternalInput")
    d_deg = nc.dram_tensor("deg", [P, BPC], f32, kind="ExternalInput")
    d_mask = nc.dram_tensor("mask", [P, BPC], f32, kind="ExternalInput")
    d_psel = nc.dram_tensor("psel", [P, BPC * G], f32, kind="ExternalInput")
    d_W = nc.dram_tensor("W", [P, L * 2 * H], bf16, kind="ExternalInput")
    d_gb = nc.dram_tensor("gb", [1, 2 * L * H], f32, kind="ExternalInput")
    d_emb0 = nc.dram_tensor("emb0", [NF, H], f32, kind="ExternalInput")
    d_emb1 = nc.dram_tensor("emb1", [NF, H], f32, kind="ExternalInput")
    d_w1 = nc.dram_tensor("w1", [P, 2 * P], f32, kind="ExternalInput")
    d_w2 = nc.dram_tensor("w2", [P, 64], f32, kind="ExternalInput")
    d_w3 = nc.dram_tensor("w3", [64, 1], f32, kind="ExternalInput")
    d_b1 = nc.dram_tensor("b1", [P, 1], f32, kind="ExternalInput")
    d_b2 = nc.dram_tensor("b2", [64, 1], f32, kind="ExternalInput")
    d_b3 = nc.dram_tensor("b3", [1, 1], f32, kind="ExternalInput")
    d_out = nc.dram_tensor("out", [1, G], f32, kind="ExternalOutput")

    rg = [list(range(NCORE))]

    with tile.TileContext(nc) as tc, ExitStack() as ctx:
        pers = ctx.enter_context(tc.tile_pool(name="pers", bufs=1))
        psA = ctx.enter_context(tc.tile_pool(name="psA", bufs=2, space="PSUM"))
        psB = ctx.enter_context(tc.tile_pool(name="psB", bufs=2, space="PSUM"))
        psC = ctx.enter_context(tc.tile_pool(name="psC", bufs=1, space="PSUM"))
        gpool = ctx.enter_context(tc.tile_pool(name="gpool", bufs=2))
        work = ctx.enter_context(tc.tile_pool(name="work", bufs=2))
        stream = ctx.enter_context(tc.tile_pool(name="stream", bufs=2))
        dram = ctx.enter_context(tc.tile_pool(name="dram", bufs=2, space="DRAM"))

        # ---- persistent SBUF state -------------------------------------
        sel_sb = pers.tile([P, BPC * T_blk * P], bf16, tag="sel")
        idx_sb = pers.tile([P, BPC * NIB], i16, tag="idx")
        deg_sb = pers.tile([P, BPC], f32, tag="deg")
        mask_sb = pers.tile([P, BPC], f32, tag="mask")
        W_sb = pers.tile([P, L * 2 * H], bf16, tag="W")
        gb_sb = pers.tile([1, 2 * L * H], f32, tag="gb")
        emb0_sb = pers.tile([NF, H], f32, tag="emb0")
        emb1_sb = pers.tile([NF, H], f32, tag="emb1")
        w1_sb = pers.tile([P, 2 * P], f32, tag="w1")
        w2_sb = pers.tile([P, 64], f32, tag="w2")
        w3_sb = pers.tile([64, 1], f32, tag="w3")
        b1_sb = pers.tile([P, 1], f32, tag="b1")
        b2_sb = pers.tile([64, 1], f32, tag="b2")
        b3_sb = pers.tile([1, 1], f32, tag="b3")

        h_sb = pers.tile([P, BPC * H], f32, tag="h")
        hsT_sb = pers.tile([P, BPC * 2 * P], bf16, tag="hsT")
        hws_sb = pers.tile([P, BPC * H], bf16, tag="hws")
        t_all = pers.tile([P, BPC * H], f32, tag="t_all")
        nrm_sb = pers.tile([P, BPC], f32, tag="nrm")
        acc_s = pers.tile([P, H], f32, tag="acc_s")
        acc_q = pers.tile([P, H], f32, tag="acc_q")
        D_sb = pers.tile([NF, H], f32, tag="D")
        base_rep = pers.tile([P, H], f32, tag="base_rep")
        a_rep = pers.tile([P, H], f32, tag="a_rep")
        c_rep = pers.tile([P, H], f32, tag="c_rep")
        ident_bf = pers.tile([P, P], bf16, tag="ident")
        ones9 = pers.tile([NF, 1], f32, tag="ones9")
        ones1 = pers.tile([1, P], f32, tag="ones1")
        ones128 = pers.tile([P, 1], f32, tag="ones128")
        stv = pers.tile([1, 2 * H], f32, tag="stv")
        scal = pers.tile([1, 8 * H], f32, tag="scal")

        # ---- DRAM bounce buffers ---------------------------------------
        ag_in = dram.tile([NPC, H], bf16, tag="ag_in")
        ag_out = dram.tile([NPAD, H], bf16, tag="ag_out")
        ar_in = dram.tile([1, 2 * H], f32, tag="ar_in")
        ar_out = dram.tile([1, 2 * H], f32, tag="ar_out")
        pr_in = dram.tile([2 * P + 1, G], f32, tag="pr_in")
        pr_out = dram.tile([2 * P + 1, G], f32, tag="pr_out")

        # ---- input loads ------------------------------------------------
        for t, d in [(sel_sb, d_sel), (idx_sb, d_idx),
                     (deg_sb, d_deg), (mask_sb, d_mask),
                     (W_sb, d_W), (gb_sb, d_gb), (emb0_sb, d_emb0),
                     (emb1_sb, d_emb1), (w1_sb, d_w1), (w2_sb, d_w2),
                     (w3_sb, d_w3), (b1_sb, d_b1), (b2_sb, d_b2),
                     (b3_sb, d_b3)]:
            nc.sync.dma_start(out=t[:], in_=d[:])

        nc.gpsimd.load_library(library_config.mlp)
        make_identity(nc, ident_bf[:])
        nc.vector.memset(ones9[:], 1.0)
        nc.vector.memset(ones1[:], 1.0)
        nc.vector.memset(ones128[:], 1.0)

        # nrm = rsqrt(deg) * mask
        rdeg = work.tile([P, BPC], f32, tag="rdeg")
        nc.vector.reciprocal(out=rdeg[:], in_=deg_sb[:])
        nc.scalar.activation(out=rdeg[:], in_=rdeg[:], func=FT.Sqrt)
        nc.vector.tensor_tensor(out=nrm_sb[:], in0=rdeg[:], in1=mask_sb[:], op=OP.mult)

        # encoder prep: D = emb1 - emb0 ; base = ones9^T @ emb0, broadcast
        nc.vector.tensor_tensor(out=D_sb[:], in0=emb1_sb[:], in1=emb0_sb[:], op=OP.subtract)
        ps_b = psB.tile([1, H], f32, tag="vec")
        nc.tensor.matmul(out=ps_b[:], lhsT=ones9[:], rhs=emb0_sb[:], start=True, stop=True)
        bvec = scal[:, 0:H]
        nc.vector.tensor_copy(out=bvec, in_=ps_b[:])
        ps_br = psB.tile([P, H], f32, tag="vec")
        nc.tensor.matmul(out=ps_br[:], lhsT=ones1[:], rhs=bvec, start=True, stop=True)
        nc.vector.tensor_copy(out=base_rep[:], in_=ps_br[:])

        def hslice(nb):
            return h_sb[:, nb * H:(nb + 1) * H]

        def emit_hs_transpose(nb):
            """hs = h*nrm (bf16), transpose both 128-halves into hsT_sb."""
            hs_bf = work.tile([P, H], bf16, tag="hs_bf")
            nc.vector.tensor_scalar_mul(hs_bf[:], hslice(nb), nrm_sb[:, nb:nb + 1])
            for k in range(2):
                pst = psB.tile([P, P], bf16, tag="pst")
                nc.tensor.transpose(out=pst[:], in_=hs_bf[:, k * P:(k + 1) * P],
                                    identity=ident_bf[:])
                nc.vector.tensor_copy(out=hsT_sb[:, (nb * 2 + k) * P:(nb * 2 + k + 1) * P],
                                      in_=pst[:])

        # encoder: h0 = base + xT^T @ D  (per block)
        for nb in range(BPC):
            xT_t = stream.tile([NF, P], f32, tag="xT_t")
            nc.sync.dma_start(out=xT_t[:], in_=d_xT[:, nb * P:(nb + 1) * P])
            ps_h = psA.tile([P, H], f32, tag="mm")
            nc.tensor.matmul(out=ps_h[:], lhsT=xT_t[:],
                             rhs=D_sb[:], start=True, stop=True)
            nc.vector.tensor_tensor(out=hslice(nb), in0=ps_h[:], in1=base_rep[:], op=OP.add)
            emit_hs_transpose(nb)

        if variant == "enc":
            nc.sync.dma_start(out=d_out[:], in_=h_sb[0:1, 0:G])
        # ---- layers -----------------------------------------------------
        nlayers = 0 if variant == "enc" else (1 if variant in ("ag", "gat", "agg", "l1") else L)
        for l in range(nlayers):
            # GEMM hws = hs @ W[l]  (lhsT = hsT halves, rhs = W k-halves)
            for nb in range(BPC):
                ps_g = psA.tile([P, H], f32, tag="mm")
                for k in range(2):
                    nc.tensor.matmul(
                        out=ps_g[:],
                        lhsT=hsT_sb[:, (nb * 2 + k) * P:(nb * 2 + k + 1) * P],
                        rhs=W_sb[:, (l * 2 + k) * H:(l * 2 + k + 1) * H],
                        start=(k == 0), stop=(k == 1))
                nc.vector.tensor_copy(out=hws_sb[:, nb * H:(nb + 1) * H], in_=ps_g[:])
                nc.sync.dma_start(out=ag_in[nb * P:(nb + 1) * P, :],
                                  in_=hws_sb[:, nb * H:(nb + 1) * H])
            nc.gpsimd.collective_compute(
                "AllGather", OP.bypass, replica_groups=rg,
                ins=[ag_in[:]], outs=[ag_out[:]])
            if variant == "ag":
                sbtmp = work.tile([1, G], bf16, tag="dbg")
                nc.sync.dma_start(out=sbtmp[:], in_=ag_out[0:1, 0:G])
                sbtmp2 = work.tile([1, G], f32, tag="dbg2")
                nc.vector.tensor_copy(out=sbtmp2[:], in_=sbtmp[:])
                nc.sync.dma_start(out=d_out[:], in_=sbtmp2[:])
                break

            nc.vector.memset(acc_s[:], 0.0)
            nc.vector.memset(acc_q[:], 0.0)

            T0 = (T_blk + 1) // 2
            chunks = [(0, T0), (T0, T_blk)]
            for nb in range(BPC):
                gts = []
                for (j0, j1) in chunks:
                    gath = gpool.tile([P, T0 * H], bf16, tag="gath")
                    nc.gpsimd.dma_gather(
                        out_ap=gath[:, :(j1 - j0) * H].rearrange("p (t h) -> p t h", h=H),
                        in_ap=ag_out[:],
                        idxs_ap=idx_sb[:, nb * NIB + j0 * 8:nb * NIB + j1 * 8],
                        num_idxs=(j1 - j0) * P, num_idxs_reg=(j1 - j0) * P,
                        elem_size=H, single_packet=False)
                    gts.append(gath)
                if variant == "gat":
                    gtmp = work.tile([1, G], bf16, tag="dbg")
                    nc.vector.tensor_copy(out=gtmp[:], in_=gts[0][0:1, 0:G])
                    gtmp2 = work.tile([1, G], f32, tag="dbg2")
                    nc.vector.tensor_copy(out=gtmp2[:], in_=gtmp[:])
                    nc.sync.dma_start(out=d_out[:], in_=gtmp2[:])
                    break
                ps_t = psA.tile([P, H], f32, tag="mm")
                for j in range(T_blk):
                    ti = nb * T_blk + j
                    ci = 0 if j < T0 else 1
                    jj = j if j < T0 else j - T0
                    nc.tensor.matmul(
                        out=ps_t[:],
                        lhsT=sel_sb[:, ti * P:(ti + 1) * P],
                        rhs=gts[ci][:, jj * H:(jj + 1) * H],
                        start=(j == 0), stop=(j == T_blk - 1))
                tsl = t_all[:, nb * H:(nb + 1) * H]
                nc.scalar.activation(out=tsl, in_=ps_t[:], func=FT.Copy,
                                     scale=nrm_sb[:, nb:nb + 1])
                sq = work.tile([P, H], f32, tag="tmp")
                nc.vector.tensor_tensor(out=sq[:], in0=tsl, in1=tsl, op=OP.mult)
                nc.vector.tensor_tensor(out=acc_s[:], in0=acc_s[:], in1=tsl, op=OP.add)
                nc.vector.tensor_tensor(out=acc_q[:], in0=acc_q[:], in1=sq[:], op=OP.add)

            if variant == "gat":
                break
            if variant == "agg":
                nc.sync.dma_start(out=d_out[:], in_=t_all[0:1, 0:G])
                break
            # stats: cross-partition reduce + AllReduce
            ps_s = psB.tile([1, 2 * H], f32, tag="vec")
            nc.tensor.matmul(out=ps_s[:, 0:H], lhsT=ones128[:], rhs=acc_s[:],
                             start=True, stop=True)
            nc.tensor.matmul(out=ps_s[:, H:2 * H], lhsT=ones128[:], rhs=acc_q[:],
                             start=True, stop=True)
            st_sb = scal[:, 6 * H:8 * H]
            nc.vector.tensor_copy(out=st_sb, in_=ps_s[:])
            nc.sync.dma_start(out=ar_in[:], in_=st_sb)
            nc.gpsimd.collective_compute(
                "AllReduce", OP.add, replica_groups=rg,
                ins=[ar_in[:]], outs=[ar_out[:]])
            nc.sync.dma_start(out=stv[:], in_=ar_out[:])

            # a = gamma*istd ; c = beta - mu*a   (all [1,H] lanes)
            mu = scal[:, H:2 * H]
            var = scal[:, 2 * H:3 * H]
            av = scal[:, 3 * H:4 * H]
            cv = scal[:, 4 * H:5 * H]
            msq = scal[:, 5 * H:6 * H]
            nc.vector.tensor_scalar_mul(mu, stv[:, 0:H], 1.0 / N)
            nc.vector.tensor_scalar_mul(var, stv[:, H:2 * H], 1.0 / N)
            nc.vector.tensor_tensor(out=msq, in0=mu, in1=mu, op=OP.mult)
            nc.vector.tensor_tensor(out=var, in0=var, in1=msq, op=OP.subtract)
            nc.vector.tensor_scalar_add(var, var, BN_EPS)
            nc.vector.reciprocal(out=var, in_=var)
            nc.scalar.activation(out=var, in_=var, func=FT.Sqrt)  # istd
            nc.vector.tensor_tensor(out=av, in0=var,
                                    in1=gb_sb[:, l * H:(l + 1) * H], op=OP.mult)
            nc.vector.tensor_tensor(out=msq, in0=mu, in1=av, op=OP.mult)
            nc.vector.tensor_tensor(out=cv, in0=gb_sb[:, (L + l) * H:(L + l + 1) * H],
                                    in1=msq, op=OP.subtract)
            ps_a = psB.tile([P, H], f32, tag="vec")
            nc.tensor.matmul(out=ps_a[:], lhsT=ones1[:], rhs=av, start=True, stop=True)
            nc.vector.tensor_copy(out=a_rep[:], in_=ps_a[:])
            ps_c = psB.tile([P, H], f32, tag="vec")
            nc.tensor.matmul(out=ps_c[:], lhsT=ones1[:], rhs=cv, start=True, stop=True)
            nc.vector.tensor_copy(out=c_rep[:], in_=ps_c[:])

            # h = relu(t*a + c) + h ; prepare hsT for next layer
            for nb in range(BPC):
                tsl = t_all[:, nb * H:(nb + 1) * H]
                u = work.tile([P, H], f32, tag="tmp")
                nc.vector.tensor_tensor(out=u[:], in0=tsl, in1=a_rep[:], op=OP.mult)
                nc.vector.tensor_tensor(out=u[:], in0=u[:], in1=c_rep[:], op=OP.add)
                r = work.tile([P, H], f32, tag="tmp2")
                nc.scalar.activation(out=r[:], in_=u[:], func=FT.Relu)
                nc.vector.tensor_tensor(out=hslice(nb), in0=hslice(nb), in1=r[:], op=OP.add)
                if l < L - 1:
                    emit_hs_transpose(nb)

        if variant == "l1":
            nc.sync.dma_start(out=d_out[:], in_=h_sb[0:1, 0:G])
        skip_pool = variant in ("enc", "ag", "gat", "agg", "l1")
        # ---- pooling ----------------------------------------------------
        if not skip_pool:
            ps_p0 = psC.tile([P, G], f32, tag="p0")
            ps_p1 = psC.tile([P, G], f32, tag="p1")
            ps_pc = psB.tile([1, G], f32, tag="vec")
            for nb in range(BPC):
                psel_t = stream.tile([P, G], f32, tag="psel_t")
                nc.sync.dma_start(out=psel_t[:], in_=d_psel[:, nb * G:(nb + 1) * G])
                pssl = psel_t[:]
                nc.tensor.matmul(out=ps_p0[:], lhsT=h_sb[:, nb * H:nb * H + P],
                                 rhs=pssl, start=(nb == 0), stop=(nb == BPC - 1))
                nc.tensor.matmul(out=ps_p1[:], lhsT=h_sb[:, nb * H + P:(nb + 1) * H],
                                 rhs=pssl, start=(nb == 0), stop=(nb == BPC - 1))
                nc.tensor.matmul(out=ps_pc[:], lhsT=ones128[:],
                                 rhs=pssl, start=(nb == 0), stop=(nb == BPC - 1))
            g0 = work.tile([P, G], f32, tag="g0")
            g1 = work.tile([P, G], f32, tag="g1")
            cnt = scal[:, 0:G]
            nc.vector.tensor_copy(out=g0[:], in_=ps_p0[:])
            nc.vector.tensor_copy(out=g1[:], in_=ps_p1[:])
            nc.vector.tensor_copy(out=cnt, in_=ps_pc[:])
            nc.sync.dma_start(out=pr_in[0:P, :], in_=g0[:])
            nc.sync.dma_start(out=pr_in[P:2 * P, :], in_=g1[:])
            nc.sync.dma_start(out=pr_in[2 * P:2 * P + 1, :], in_=cnt)
            nc.gpsimd.collective_compute(
                "AllReduce", OP.add, replica_groups=rg,
                ins=[pr_in[:]], outs=[pr_out[:]])
            nc.sync.dma_start(out=g0[:], in_=pr_out[0:P, :])
            nc.sync.dma_start(out=g1[:], in_=pr_out[P:2 * P, :])
            nc.sync.dma_start(out=cnt, in_=pr_out[2 * P:2 * P + 1, :])
            nc.vector.tensor_scalar_max(cnt, cnt, 1.0)
            nc.vector.reciprocal(out=cnt, in_=cnt)
            ps_r = psB.tile([P, G], f32, tag="vec")
            nc.tensor.matmul(out=ps_r[:], lhsT=ones1[:], rhs=cnt, start=True, stop=True)
            rc_rep = work.tile([P, G], f32, tag="rc_rep")
            nc.vector.tensor_copy(out=rc_rep[:], in_=ps_r[:])
            nc.vector.tensor_tensor(out=g0[:], in0=g0[:], in1=rc_rep[:], op=OP.mult)
            nc.vector.tensor_tensor(out=g1[:], in0=g1[:], in1=rc_rep[:], op=OP.mult)

            # MLP head (transposed: weights are lhsT, graphs along free dim)
            ps1 = psB.tile([P, G], f32, tag="vec")
            nc.tensor.matmul(out=ps1[:], lhsT=w1_sb[:, 0:P], rhs=g0[:], start=True, stop=False)
            nc.tensor.matmul(out=ps1[:], lhsT=w1_sb[:, P:2 * P], rhs=g1[:], start=False, stop=True)
            y1 = work.tile([P, G], f32, tag="y1")
            nc.scalar.activation(out=y1[:], in_=ps1[:], func=FT.Relu, bias=b1_sb[:, 0:1])
            ps2 = psB.tile([64, G], f32, tag="vec")
            nc.tensor.matmul(out=ps2[:], lhsT=w2_sb[:], rhs=y1[:], start=True, stop=True)
            y2 = work.tile([64, G], f32, tag="y2")
            nc.scalar.activation(out=y2[:], in_=ps2[:], func=FT.Relu, bias=b2_sb[:, 0:1])
            ps3 = psB.tile([1, G], f32, tag="vec")
            nc.tensor.matmul(out=ps3[:], lhsT=w3_sb[:], rhs=y2[:], start=True, stop=True)
            y3 = work.tile([1, G], f32, tag="y3")
            nc.vector.tensor_scalar_add(y3[:], ps3[:], b3_sb[0:1, 0:1])
            nc.sync.dma_start(out=d_out[:], in_=y3[:])

    nc.compile()
    return nc


# --------------------------------------------------------------------------
# entry point
# --------------------------------------------------------------------------

def kernel(x, edge_index, batch_ids, emb, W, b, gamma, beta,
           mlp_W1, mlp_b1, mlp_W2, mlp_b2, mlp_W3, mlp_b3,
           _trace=False, _trace_kwargs=None):
    # NB: reference BN subtracts the per-channel mean, so the additive bias b
    # cancels exactly and is not needed by the device program.
    T_blk, in_maps = _preprocess(x, edge_index, batch_ids, emb, W, gamma, beta,
                                 mlp_W1, mlp_b1, mlp_W2, mlp_b2, mlp_W3, mlp_b3)
    import os
    variant = os.environ.get("KVARIANT", "full")
    key = (T_blk, variant)
    if key not in _compiled:
        _compiled[key] = _build(T_blk, variant)
    nc = _compiled[key]
    kw = {}
    if _trace:
        kw = dict(trace=True, **(_trace_kwargs or {}))
    res = run_bass_kernel_spmd(nc, in_maps, core_ids=list(range(NCORE)), **kw)
    out = np.asarray(res.results[0]["out"], np.float32).reshape(G, 1)
    kernel._last_results = res
    return out



# revision 5
# speedup vs baseline: 1.3460x; 1.3460x over previous
"""Trainium2 Bass kernel for HIVNet GCN message passing (8-core SPMD).

v2 strategy (vs v1 baseline at 2.29ms):
  - Pad N=10000 nodes to 10240 = 80 blocks x 128; core c owns 10 dst-blocks.
  - Per layer: hws = (h*nrm) @ W[l] on the owned shard (bf16), AllGather into
    a DRAM table on every core, bulk-load the full table into SBUF.
  - Aggregation is HYBRID per dst-block:
      * N_DENSE blocks: dense one-hot adjacency matmuls on TensorE
        (80 src-chunk matmuls accumulated in PSUM, A streamed from HBM,
        prefetched under the AllGather) -- zero GPSIMD cost.
      * remaining blocks: SWDGE dma_gather of DEDUPED (src,dst-block) rows
        (multiplicity folded into the sel weights) + one-hot sel matmuls.
    Split chosen to balance Pool-engine (8.2ns/row) vs TensorE (213ns/matmul).
  - BatchNorm: partial sums/sumsq -> stats replicated to 128 rows so the
    2KB AllReduce becomes a 256KB RDH AllReduce (79us -> ~16us); scale/shift
    broadcast via rank-1 TensorE matmul; relu + residual on DVE.
  - Readout: graph mean-pool via one-hot pool matrices, 257-row AllReduce,
    3-layer MLP computed redundantly on every core.
"""

import os
import sys

sys.path.insert(0, "/opt/trn_rl_repo")

from contextlib import ExitStack

import numpy as np
import ml_dtypes

from concourse import bass, mybir, bacc, tile, library_config
from concourse.bass_utils import run_bass_kernel_spmd
from concourse.masks import make_identity

NCORE = 8
P = 128
H = 256
L = 4
NF = 9
G = 256
N = 10000
BPC = 10                # dst blocks per core
NPC = BPC * P           # 1280 nodes per core
NPAD = NCORE * NPC      # 10240
NCHUNK = NPAD // P      # 80 src chunks
BN_EPS = 1e-5

N_DENSE = int(os.environ.get("KDENSE", "6"))   # dense-adjacency blocks per core
N_GATH = BPC - N_DENSE

f32 = mybir.dt.float32
bf16 = mybir.dt.bfloat16
i16 = mybir.dt.int16
bfnp = ml_dtypes.bfloat16

FT = mybir.ActivationFunctionType
OP = mybir.AluOpType

_compiled = {}


# --------------------------------------------------------------------------
# host-side structural preprocessing
# --------------------------------------------------------------------------

def _preprocess(x, edge_index, batch_ids, emb, W, gamma, beta,
                mlp_W1, mlp_b1, mlp_W2, mlp_b2, mlp_W3, mlp_b3):
    src = np.asarray(edge_index[0], np.int64)
    dst = np.asarray(edge_index[1], np.int64)
    # self loops for every real node (weight nrm[d]^2 folds in)
    src_all = np.concatenate([src, np.arange(N, dtype=np.int64)])
    dst_all = np.concatenate([dst, np.arange(N, dtype=np.int64)])
    order = np.argsort(dst_all, kind="stable")
    s_sorted = src_all[order]
    d_sorted = dst_all[order]

    deg = np.bincount(dst_all, minlength=NPAD).astype(np.float64)  # incl self

    nblk = NCORE * BPC
    starts = np.searchsorted(d_sorted, np.arange(nblk) * P)
    ends = np.searchsorted(d_sorted, (np.arange(nblk) + 1) * P)

    # ---- per-block edge structures ------------------------------------
    # dense blocks (local idx < N_DENSE): A counts [NPAD, P]
    # gathered blocks: deduped (src, dst_local) with multiplicity
    A_blocks = {}          # (c, nb) -> [P, NCHUNK*P] bf16
    uniq_blocks = {}       # (c, nb) -> (srcs_u, dloc_u, cnt)
    T_g = 1
    for g in range(nblk):
        c, nb = divmod(g, BPC)
        e_s = s_sorted[starts[g]:ends[g]]
        e_d = d_sorted[starts[g]:ends[g]] - g * P
        if nb < N_DENSE:
            A = np.zeros((NPAD, P), np.float32)
            np.add.at(A, (e_s, e_d), 1.0)
            A_blocks[(c, nb)] = np.ascontiguousarray(
                A.reshape(NCHUNK, P, P).transpose(1, 0, 2).reshape(P, NCHUNK * P)
            ).astype(bfnp)
        else:
            key = e_s * P + e_d
            uk, cnt = np.unique(key, return_counts=True)
            uniq_blocks[(c, nb)] = (uk // P, uk % P, cnt)
            T_g = max(T_g, (len(uk) + P - 1) // P)

    NI = T_g * P
    NIB = NI // 16

    # graph pool one-hot [node, graph]
    bids = np.asarray(batch_ids, np.int64)
    psel_full = np.zeros((NPAD, G), np.float32)
    psel_full[np.arange(N), bids] = 1.0

    x_np = np.zeros((NPAD, NF), np.float32)
    x_np[:N] = np.asarray(x, np.float64)

    # shared parameter tensors (layout for device)
    Wf = np.asarray(W, np.float32)                       # [L,H,H]
    W_lhsT = Wf.reshape(L, 2, P, H).transpose(2, 0, 1, 3).reshape(P, L * 2 * H)
    gb = np.concatenate([np.asarray(gamma, np.float32).reshape(-1),
                         np.asarray(beta, np.float32).reshape(-1)])[None, :]
    embf = np.asarray(emb, np.float32)
    emb0 = np.ascontiguousarray(embf[:, 0, :])
    emb1 = np.ascontiguousarray(embf[:, 1, :])
    w1 = np.asarray(mlp_W1, np.float32).reshape(2, P, P).transpose(1, 0, 2).reshape(P, 2 * P)
    w2 = np.asarray(mlp_W2, np.float32)
    w3 = np.asarray(mlp_W3, np.float32)
    b1 = np.asarray(mlp_b1, np.float32).reshape(P, 1)
    b2 = np.asarray(mlp_b2, np.float32).reshape(64, 1)
    b3 = np.asarray(mlp_b3, np.float32).reshape(1, 1)

    in_maps = []
    for c in range(NCORE):
        lo, hi = c * NPC, (c + 1) * NPC

        # dense adjacency stream: [P, N_DENSE * NCHUNK * P]
        if N_DENSE:
            Ac = np.concatenate([A_blocks[(c, nb)] for nb in range(N_DENSE)], axis=1)
        else:
            Ac = np.zeros((P, 1), bfnp)

        # gathered blocks: sel [P, N_GATH*T_g*P] bf16, idx [P, N_GATH*NIB] i16
        # pad unused slots with idx 0 (gathers real row 0, sel weight 0) --
        # idx -1 would skip the DMA and leave garbage (possibly NaN) in the
        # tile, and 0 * NaN = NaN would poison the PSUM accumulation.
        selc = np.zeros((N_GATH, T_g, P, P), np.float32)
        idxc = np.zeros((N_GATH, NI), np.int16)
        for j in range(N_GATH):
            nb = N_DENSE + j
            su, du, cnt = uniq_blocks[(c, nb)]
            n = len(su)
            idxc[j, :n] = su.astype(np.int16)
            selc[j].reshape(T_g * P, P)[np.arange(n), du] = cnt
        selc = np.ascontiguousarray(
            selc.reshape(N_GATH * T_g, P, P).transpose(1, 0, 2)
        ).reshape(P, N_GATH * T_g * P)
        idxw = idxc.reshape(N_GATH, NIB, 16).transpose(0, 2, 1)   # [NG,16,NIB]
        idxw = np.tile(idxw, (1, 8, 1))                           # [NG,128,NIB]
        idxw = np.ascontiguousarray(idxw.transpose(1, 0, 2)).reshape(P, N_GATH * NIB)

        degc = deg[lo:hi].reshape(BPC, P).T
        maskc = (degc > 0).astype(np.float32)
        degc = np.maximum(degc, 1.0).astype(np.float32)

        pselc = psel_full[lo:hi].reshape(BPC, P, G)
        pselc = np.ascontiguousarray(pselc.transpose(1, 0, 2)).reshape(P, BPC * G)

        xTc = np.ascontiguousarray(x_np[lo:hi].T)

        in_maps.append(dict(
            A=Ac, selw=selc.astype(bfnp), idx=idxw.astype(np.int16),
            xT=xTc, deg=degc, mask=maskc, psel=pselc,
            W=W_lhsT.astype(bfnp), gb=gb, emb0=emb0, emb1=emb1,
            w1=w1, w2=w2, w3=w3, b1=b1, b2=b2, b3=b3,
        ))
    return T_g, in_maps


# --------------------------------------------------------------------------
# device program
# --------------------------------------------------------------------------

def _build(T_g):
    NI = T_g * P
    NIB = NI // 16
    nc = bacc.Bacc(None, target_bir_lowering=False)

    d_A = nc.dram_tensor("A", [P, max(N_DENSE * NCHUNK * P, 1)], bf16,
                         kind="ExternalInput")
    d_sel = nc.dram_tensor("selw", [P, N_GATH * T_g * P], bf16, kind="ExternalInput")
    d_idx = nc.dram_tensor("idx", [P, N_GATH * NIB], i16, kind="ExternalInput")
    d_xT = nc.dram_tensor("xT", [NF, NPC], f32, kind="ExternalInput")
    d_deg = nc.dram_tensor("deg", [P, BPC], f32, kind="ExternalInput")
    d_mask = nc.dram_tensor("mask", [P, BPC], f32, kind="ExternalInput")
    d_psel = nc.dram_tensor("psel", [P, BPC * G], f32, kind="ExternalInput")
    d_W = nc.dram_tensor("W", [P, L * 2 * H], bf16, kind="ExternalInput")
    d_gb = nc.dram_tensor("gb", [1, 2 * L * H], f32, kind="ExternalInput")
    d_emb0 = nc.dram_tensor("emb0", [NF, H], f32, kind="ExternalInput")
    d_emb1 = nc.dram_tensor("emb1", [NF, H], f32, kind="ExternalInput")
    d_w1 = nc.dram_tensor("w1", [P, 2 * P], f32, kind="ExternalInput")
    d_w2 = nc.dram_tensor("w2", [P, 64], f32, kind="ExternalInput")
    d_w3 = nc.dram_tensor("w3", [64, 1], f32, kind="ExternalInput")
    d_b1 = nc.dram_tensor("b1", [P, 1], f32, kind="ExternalInput")
    d_b2 = nc.dram_tensor("b2", [64, 1], f32, kind="ExternalInput")
    d_b3 = nc.dram_tensor("b3", [1, 1], f32, kind="ExternalInput")
    d_out = nc.dram_tensor("out", [1, G], f32, kind="ExternalOutput")

    rg = [list(range(NCORE))]

    with tile.TileContext(nc) as tc, ExitStack() as ctx:
        pers = ctx.enter_context(tc.tile_pool(name="pers", bufs=1))
        psA = ctx.enter_context(tc.tile_pool(name="psA", bufs=2, space="PSUM"))
        psB = ctx.enter_context(tc.tile_pool(name="psB", bufs=2, space="PSUM"))
        psC = ctx.enter_context(tc.tile_pool(name="psC", bufs=1, space="PSUM"))
        apool = ctx.enter_context(tc.tile_pool(name="apool", bufs=2))
        gpool = ctx.enter_context(tc.tile_pool(name="gpool", bufs=2))
        work = ctx.enter_context(tc.tile_pool(name="work", bufs=2))
        stream = ctx.enter_context(tc.tile_pool(name="stream", bufs=2))
        dram = ctx.enter_context(tc.tile_pool(name="dram", bufs=2, space="DRAM"))

        # ---- persistent SBUF state -------------------------------------
        sel_sb = pers.tile([P, N_GATH * T_g * P], bf16, tag="sel")
        idx_sb = pers.tile([P, N_GATH * NIB], i16, tag="idx")
        deg_sb = pers.tile([P, BPC], f32, tag="deg")
        mask_sb = pers.tile([P, BPC], f32, tag="mask")
        W_sb = pers.tile([P, L * 2 * H], bf16, tag="W")
        gb_sb = pers.tile([1, 2 * L * H], f32, tag="gb")
        emb0_sb = pers.tile([NF, H], f32, tag="emb0")
        emb1_sb = pers.tile([NF, H], f32, tag="emb1")
        w1_sb = pers.tile([P, 2 * P], f32, tag="w1")
        w2_sb = pers.tile([P, 64], f32, tag="w2")
        w3_sb = pers.tile([64, 1], f32, tag="w3")
        b1_sb = pers.tile([P, 1], f32, tag="b1")
        b2_sb = pers.tile([64, 1], f32, tag="b2")
        b3_sb = pers.tile([1, 1], f32, tag="b3")

        tab_sb = pers.tile([P, NCHUNK * H], bf16, tag="tab")
        h_sb = pers.tile([P, BPC * H], f32, tag="h")
        hsT_sb = pers.tile([P, BPC * 2 * P], bf16, tag="hsT")
        hws_sb = pers.tile([P, BPC * H], bf16, tag="hws")
        t_all = pers.tile([P, BPC * H], f32, tag="t_all")
        nrm_sb = pers.tile([P, BPC], f32, tag="nrm")
        acc_s = pers.tile([P, H], f32, tag="acc_s")
        acc_q = pers.tile([P, H], f32, tag="acc_q")
        D_sb = pers.tile([NF, H], f32, tag="D")
        base_rep = pers.tile([P, H], f32, tag="base_rep")
        a_rep = pers.tile([P, H], f32, tag="a_rep")
        c_rep = pers.tile([P, H], f32, tag="c_rep")
        ident_bf = pers.tile([P, P], bf16, tag="ident")
        ones9 = pers.tile([NF, 1], f32, tag="ones9")
        ones1 = pers.tile([1, P], f32, tag="ones1")
        ones128 = pers.tile([P, 1], f32, tag="ones128")
        stv = pers.tile([1, 2 * H], f32, tag="stv")
        scal = pers.tile([1, 8 * H], f32, tag="scal")

        # ---- DRAM bounce buffers ---------------------------------------
        ag_in = dram.tile([NPC, H], bf16, tag="ag_in")
        ag_out = dram.tile([NPAD, H], bf16, tag="ag_out")
        ar_in = dram.tile([P, 2 * H], f32, tag="ar_in")
        ar_out = dram.tile([P, 2 * H], f32, tag="ar_out")
        pr_in = dram.tile([2 * P + 1, G], f32, tag="pr_in")
        pr_out = dram.tile([2 * P + 1, G], f32, tag="pr_out")

        # ---- input loads ------------------------------------------------
        for t, d in [(sel_sb, d_sel), (idx_sb, d_idx),
                     (deg_sb, d_deg), (mask_sb, d_mask),
                     (W_sb, d_W), (gb_sb, d_gb), (emb0_sb, d_emb0),
                     (emb1_sb, d_emb1), (w1_sb, d_w1), (w2_sb, d_w2),
                     (w3_sb, d_w3), (b1_sb, d_b1), (b2_sb, d_b2),
                     (b3_sb, d_b3)]:
            nc.sync.dma_start(out=t[:], in_=d[:])

        nc.gpsimd.load_library(library_config.mlp)
        make_identity(nc, ident_bf[:])
        nc.vector.memset(ones9[:], 1.0)
        nc.vector.memset(ones1[:], 1.0)
        nc.vector.memset(ones128[:], 1.0)

        # nrm = rsqrt(deg) * mask
        rdeg = work.tile([P, BPC], f32, tag="rdeg")
        nc.vector.reciprocal(out=rdeg[:], in_=deg_sb[:])
        nc.scalar.activation(out=rdeg[:], in_=rdeg[:], func=FT.Sqrt)
        nc.vector.tensor_tensor(out=nrm_sb[:], in0=rdeg[:], in1=mask_sb[:], op=OP.mult)

        # encoder prep: D = emb1 - emb0 ; base = ones9^T @ emb0, broadcast
        nc.vector.tensor_tensor(out=D_sb[:], in0=emb1_sb[:], in1=emb0_sb[:], op=OP.subtract)
        ps_b = psB.tile([1, H], f32, tag="vec")
        nc.tensor.matmul(out=ps_b[:], lhsT=ones9[:], rhs=emb0_sb[:], start=True, stop=True)
        bvec = scal[:, 0:H]
        nc.vector.tensor_copy(out=bvec, in_=ps_b[:])
        ps_br = psB.tile([P, H], f32, tag="vec")
        nc.tensor.matmul(out=ps_br[:], lhsT=ones1[:], rhs=bvec, start=True, stop=True)
        nc.vector.tensor_copy(out=base_rep[:], in_=ps_br[:])

        def hslice(nb):
            return h_sb[:, nb * H:(nb + 1) * H]

        def emit_hs_transpose(nb):
            """hs = h*nrm (bf16), transpose both 128-halves into hsT_sb."""
            hs_bf = work.tile([P, H], bf16, tag="hs_bf")
            nc.vector.tensor_scalar_mul(hs_bf[:], hslice(nb), nrm_sb[:, nb:nb + 1])
            for k in range(2):
                pst = psB.tile([P, P], bf16, tag="pst")
                nc.tensor.transpose(out=pst[:], in_=hs_bf[:, k * P:(k + 1) * P],
                                    identity=ident_bf[:])
                nc.vector.tensor_copy(out=hsT_sb[:, (nb * 2 + k) * P:(nb * 2 + k + 1) * P],
                                      in_=pst[:])

        # encoder: h0 = base + xT^T @ D  (per block)
        for nb in range(BPC):
            xT_t = stream.tile([NF, P], f32, tag="xT_t")
            nc.sync.dma_start(out=xT_t[:], in_=d_xT[:, nb * P:(nb + 1) * P])
            ps_h = psA.tile([P, H], f32, tag="mm")
            nc.tensor.matmul(out=ps_h[:], lhsT=xT_t[:],
                             rhs=D_sb[:], start=True, stop=True)
            nc.vector.tensor_tensor(out=hslice(nb), in0=ps_h[:], in1=base_rep[:], op=OP.add)
            emit_hs_transpose(nb)

        # ---- layers -----------------------------------------------------
        for l in range(L):
            # GEMM hws = hs @ W[l]
            for nb in range(BPC):
                ps_g = psA.tile([P, H], f32, tag="mm")
                for k in range(2):
                    nc.tensor.matmul(
                        out=ps_g[:],
                        lhsT=hsT_sb[:, (nb * 2 + k) * P:(nb * 2 + k + 1) * P],
                        rhs=W_sb[:, (l * 2 + k) * H:(l * 2 + k + 1) * H],
                        start=(k == 0), stop=(k == 1))
                nc.vector.tensor_copy(out=hws_sb[:, nb * H:(nb + 1) * H], in_=ps_g[:])
                nc.sync.dma_start(out=ag_in[nb * P:(nb + 1) * P, :],
                                  in_=hws_sb[:, nb * H:(nb + 1) * H])
            nc.gpsimd.collective_compute(
                "AllGather", OP.bypass, replica_groups=rg,
                ins=[ag_in[:]], outs=[ag_out[:]])

            # bulk-load the full table into SBUF: tab_sb[p, c*H:(c+1)*H] =
            # ag_out row c*128+p
            nc.sync.dma_start(
                out=tab_sb[:].rearrange("p (c h) -> p c h", h=H),
                in_=ag_out[:].rearrange("(c p) h -> p c h", p=P))

            nc.vector.memset(acc_s[:], 0.0)
            nc.vector.memset(acc_q[:], 0.0)

            def post_block(nb, ps_t):
                """t = nrm*psum; accumulate BN stats."""
                tsl = t_all[:, nb * H:(nb + 1) * H]
                nc.scalar.activation(out=tsl, in_=ps_t[:], func=FT.Copy,
                                     scale=nrm_sb[:, nb:nb + 1])
                sq = work.tile([P, H], f32, tag="tmp")
                nc.vector.tensor_tensor(out=sq[:], in0=tsl, in1=tsl, op=OP.mult)
                nc.vector.tensor_tensor(out=acc_s[:], in0=acc_s[:], in1=tsl, op=OP.add)
                nc.vector.tensor_tensor(out=acc_q[:], in0=acc_q[:], in1=sq[:], op=OP.add)

            # gathered blocks: issue SWDGE gathers first (Pool engine runs
            # them while TensorE does the dense blocks)
            T0 = (T_g + 1) // 2
            chunks = [(0, T0), (T0, T_g)] if T_g > 1 else [(0, T_g)]
            gath_tiles = []
            for j in range(N_GATH):
                gts = []
                for (j0, j1) in chunks:
                    gath = gpool.tile([P, T0 * H], bf16, tag="gath")
                    nc.gpsimd.dma_gather(
                        out_ap=gath[:, :(j1 - j0) * H].rearrange("p (t h) -> p t h", h=H),
                        in_ap=ag_out[:],
                        idxs_ap=idx_sb[:, j * NIB + j0 * 8:j * NIB + j1 * 8],
                        num_idxs=(j1 - j0) * P, num_idxs_reg=(j1 - j0) * P,
                        elem_size=H, single_packet=False)
                    gts.append(gath)
                gath_tiles.append(gts)

            # dense blocks on TensorE (A streamed in half-blocks of 40 chunks
            # to halve the SBUF footprint; bufs=2 double-buffers the stream)
            HC = NCHUNK // 2
            for nb in range(N_DENSE):
                ps_t = psA.tile([P, H], f32, tag="mm")
                for half in range(2):
                    a_t = apool.tile([P, HC * P], bf16, tag="A")
                    base = (nb * NCHUNK + half * HC) * P
                    nc.sync.dma_start(out=a_t[:], in_=d_A[:, base:base + HC * P])
                    for cc in range(HC):
                        c = half * HC + cc
                        nc.tensor.matmul(
                            out=ps_t[:],
                            lhsT=a_t[:, cc * P:(cc + 1) * P],
                            rhs=tab_sb[:, c * H:(c + 1) * H],
                            start=(c == 0), stop=(c == NCHUNK - 1))
                post_block(nb, ps_t)

            # gathered blocks: sel matmul chains
            for j in range(N_GATH):
                nb = N_DENSE + j
                gts = gath_tiles[j]
                ps_t = psA.tile([P, H], f32, tag="mm")
                for t in range(T_g):
                    ci = 0 if t < T0 else 1
                    tt = t if t < T0 else t - T0
                    nc.tensor.matmul(
                        out=ps_t[:],
                        lhsT=sel_sb[:, (j * T_g + t) * P:(j * T_g + t + 1) * P],
                        rhs=gts[ci][:, tt * H:(tt + 1) * H],
                        start=(t == 0), stop=(t == T_g - 1))
                post_block(nb, ps_t)

            # stats: cross-partition reduce, replicate to 128 rows, AllReduce
            ps_s = psB.tile([1, 2 * H], f32, tag="vec")
            nc.tensor.matmul(out=ps_s[:, 0:H], lhsT=ones128[:], rhs=acc_s[:],
                             start=True, stop=True)
            nc.tensor.matmul(out=ps_s[:, H:2 * H], lhsT=ones128[:], rhs=acc_q[:],
                             start=True, stop=True)
            st_sb = scal[:, 6 * H:8 * H]
            nc.vector.tensor_copy(out=st_sb, in_=ps_s[:])
            # replicate to [128, 2H] so the AllReduce picks the fast RDH path
            st_rep = work.tile([P, 2 * H], f32, tag="strep")
            for half in range(2):
                ps_r2 = psB.tile([P, H], f32, tag="vec")
                nc.tensor.matmul(out=ps_r2[:], lhsT=ones1[:],
                                 rhs=st_sb[:, half * H:(half + 1) * H],
                                 start=True, stop=True)
                nc.vector.tensor_copy(out=st_rep[:, half * H:(half + 1) * H],
                                      in_=ps_r2[:])
            nc.sync.dma_start(out=ar_in[:], in_=st_rep[:])
            nc.gpsimd.collective_compute(
                "AllReduce", OP.add, replica_groups=rg,
                ins=[ar_in[:]], outs=[ar_out[:]])
            nc.sync.dma_start(out=stv[:], in_=ar_out[0:1, :])

            # a = gamma*istd ; c = beta - mu*a
            mu = scal[:, H:2 * H]
            var = scal[:, 2 * H:3 * H]
            av = scal[:, 3 * H:4 * H]
            cv = scal[:, 4 * H:5 * H]
            msq = scal[:, 5 * H:6 * H]
            nc.vector.tensor_scalar_mul(mu, stv[:, 0:H], 1.0 / N)
            nc.vector.tensor_scalar_mul(var, stv[:, H:2 * H], 1.0 / N)
            nc.vector.tensor_tensor(out=msq, in0=mu, in1=mu, op=OP.mult)
            nc.vector.tensor_tensor(out=var, in0=var, in1=msq, op=OP.subtract)
            nc.vector.tensor_scalar_add(var, var, BN_EPS)
            nc.vector.reciprocal(out=var, in_=var)
            nc.scalar.activation(out=var, in_=var, func=FT.Sqrt)  # istd
            nc.vector.tensor_tensor(out=av, in0=var,
                                    in1=gb_sb[:, l * H:(l + 1) * H], op=OP.mult)
            nc.vector.tensor_tensor(out=msq, in0=mu, in1=av, op=OP.mult)
            nc.vector.tensor_tensor(out=cv, in0=gb_sb[:, (L + l) * H:(L + l + 1) * H],
                                    in1=msq, op=OP.subtract)
            ps_a = psB.tile([P, H], f32, tag="vec")
            nc.tensor.matmul(out=ps_a[:], lhsT=ones1[:], rhs=av, start=True, stop=True)
            nc.vector.tensor_copy(out=a_rep[:], in_=ps_a[:])
            ps_c = psB.tile([P, H], f32, tag="vec")
            nc.tensor.matmul(out=ps_c[:], lhsT=ones1[:], rhs=cv, start=True, stop=True)
            nc.vector.tensor_copy(out=c_rep[:], in_=ps_c[:])

            # h = relu(t*a + c) + h ; prepare hsT for next layer
            for nb in range(BPC):
                tsl = t_all[:, nb * H:(nb + 1) * H]
                u = work.tile([P, H], f32, tag="tmp")
                nc.vector.tensor_tensor(out=u[:], in0=tsl, in1=a_rep[:], op=OP.mult)
                nc.vector.tensor_tensor(out=u[:], in0=u[:], in1=c_rep[:], op=OP.add)
                r = work.tile([P, H], f32, tag="tmp2")
                nc.scalar.activation(out=r[:], in_=u[:], func=FT.Relu)
                nc.vector.tensor_tensor(out=hslice(nb), in0=hslice(nb), in1=r[:], op=OP.add)
                if l < L - 1:
                    emit_hs_transpose(nb)

        # ---- pooling ----------------------------------------------------
        ps_p0 = psC.tile([P, G], f32, tag="p0")
        ps_p1 = psC.tile([P, G], f32, tag="p1")
        ps_pc = psB.tile([1, G], f32, tag="vec")
        for nb in range(BPC):
            psel_t = stream.tile([P, G], f32, tag="psel_t")
            nc.sync.dma_start(out=psel_t[:], in_=d_psel[:, nb * G:(nb + 1) * G])
            pssl = psel_t[:]
            nc.tensor.matmul(out=ps_p0[:], lhsT=h_sb[:, nb * H:nb * H + P],
                             rhs=pssl, start=(nb == 0), stop=(nb == BPC - 1))
            nc.tensor.matmul(out=ps_p1[:], lhsT=h_sb[:, nb * H + P:(nb + 1) * H],
                             rhs=pssl, start=(nb == 0), stop=(nb == BPC - 1))
            nc.tensor.matmul(out=ps_pc[:], lhsT=ones128[:],
                             rhs=pssl, start=(nb == 0), stop=(nb == BPC - 1))
        g0 = work.tile([P, G], f32, tag="g0")
        g1 = work.tile([P, G], f32, tag="g1")
        cnt = scal[:, 0:G]
        nc.vector.tensor_copy(out=g0[:], in_=ps_p0[:])
        nc.vector.tensor_copy(out=g1[:], in_=ps_p1[:])
        nc.vector.tensor_copy(out=cnt, in_=ps_pc[:])
        nc.sync.dma_start(out=pr_in[0:P, :], in_=g0[:])
        nc.sync.dma_start(out=pr_in[P:2 * P, :], in_=g1[:])
        nc.sync.dma_start(out=pr_in[2 * P:2 * P + 1, :], in_=cnt)
        nc.gpsimd.collective_compute(
            "AllReduce", OP.add, replica_groups=rg,
            ins=[pr_in[:]], outs=[pr_out[:]])
        nc.sync.dma_start(out=g0[:], in_=pr_out[0:P, :])
        nc.sync.dma_start(out=g1[:], in_=pr_out[P:2 * P, :])
        nc.sync.dma_start(out=cnt, in_=pr_out[2 * P:2 * P + 1, :])
        nc.vector.tensor_scalar_max(cnt, cnt, 1.0)
        nc.vector.reciprocal(out=cnt, in_=cnt)
        ps_r = psB.tile([P, G], f32, tag="vec")
        nc.tensor.matmul(out=ps_r[:], lhsT=ones1[:], rhs=cnt, start=True, stop=True)
        rc_rep = work.tile([P, G], f32, tag="rc_rep")
        nc.vector.tensor_copy(out=rc_rep[:], in_=ps_r[:])
        nc.vector.tensor_tensor(out=g0[:], in0=g0[:], in1=rc_rep[:], op=OP.mult)
        nc.vector.tensor_tensor(out=g1[:], in0=g1[:], in1=rc_rep[:], op=OP.mult)

        # MLP head (transposed: weights are lhsT, graphs along free dim)
        ps1 = psB.tile([P, G], f32, tag="vec")
        nc.tensor.matmul(out=ps1[:], lhsT=w1_sb[:, 0:P], rhs=g0[:], start=True, stop=False)
        nc.tensor.matmul(out=ps1[:], lhsT=w1_sb[:, P:2 * P], rhs=g1[:], start=False, stop=True)
        y1 = work.tile([P, G], f32, tag="y1")
        nc.scalar.activation(out=y1[:], in_=ps1[:], func=FT.Relu, bias=b1_sb[:, 0:1])
        ps2 = psB.tile([64, G], f32, tag="vec")
        nc.tensor.matmul(out=ps2[:], lhsT=w2_sb[:], rhs=y1[:], start=True, stop=True)
        y2 = work.tile([64, G], f32, tag="y2")
        nc.scalar.activation(out=y2[:], in_=ps2[:], func=FT.Relu, bias=b2_sb[:, 0:1])
        ps3 = psB.tile([1, G], f32, tag="vec")
        nc.tensor.matmul(out=ps3[:], lhsT=w3_sb[:], rhs=y2[:], start=True, stop=True)
        y3 = work.tile([1, G], f32, tag="y3")
        nc.vector.tensor_scalar_add(y3[:], ps3[:], b3_sb[0:1, 0:1])
        nc.sync.dma_start(out=d_out[:], in_=y3[:])

    nc.compile()
    return nc


# --------------------------------------------------------------------------
# entry point
# --------------------------------------------------------------------------

def kernel(x, edge_index, batch_ids, emb, W, b, gamma, beta,
           mlp_W1, mlp_b1, mlp_W2, mlp_b2, mlp_W3, mlp_b3,
           _trace=False, _trace_kwargs=None):
    # NB: reference BN subtracts the per-channel mean, so the additive bias b
    # cancels exactly and is not needed by the device program.
    T_g, in_maps = _preprocess(x, edge_index, batch_ids, emb, W, gamma, beta,
                               mlp_W1, mlp_b1, mlp_W2, mlp_b2, mlp_W3, mlp_b3)
    key = (T_g, N_DENSE)
    if key not in _compiled:
        _compiled[key] = _build(T_g)
    nc = _compiled[key]
    kw = {}
    if _trace:
        kw = dict(trace=True, **(_trace_kwargs or {}))
    res = run_bass_kernel_spmd(nc, in_maps, core_ids=list(range(NCORE)), **kw)
    out = np.asarray(res.results[0]["out"], np.float32).reshape(G, 1)
    kernel._last_results = res
    return out


# revision 8
# speedup vs baseline: 1.5258x; 1.1336x over previous
"""Trainium2 Bass kernel for HIVNet GCN message passing (8-core SPMD).

v2 strategy (vs v1 baseline at 2.29ms):
  - Pad N=10000 nodes to 10240 = 80 blocks x 128; core c owns 10 dst-blocks.
  - Per layer: hws = (h*nrm) @ W[l] on the owned shard (bf16), AllGather into
    a DRAM table on every core, bulk-load the full table into SBUF.
  - Aggregation is HYBRID per dst-block:
      * N_DENSE blocks: dense one-hot adjacency matmuls on TensorE
        (80 src-chunk matmuls accumulated in PSUM, A streamed from HBM,
        prefetched under the AllGather) -- zero GPSIMD cost.
      * remaining blocks: SWDGE dma_gather of DEDUPED (src,dst-block) rows
        (multiplicity folded into the sel weights) + one-hot sel matmuls.
    Split chosen to balance Pool-engine (8.2ns/row) vs TensorE (213ns/matmul).
  - BatchNorm: partial sums/sumsq -> stats replicated to 128 rows so the
    2KB AllReduce becomes a 256KB RDH AllReduce (79us -> ~16us); scale/shift
    broadcast via rank-1 TensorE matmul; relu + residual on DVE.
  - Readout: graph mean-pool via one-hot pool matrices, 257-row AllReduce,
    3-layer MLP computed redundantly on every core.
"""

import os
import sys

sys.path.insert(0, "/opt/trn_rl_repo")

from contextlib import ExitStack

import numpy as np
import ml_dtypes

from concourse import bass, mybir, bacc, tile, library_config
from concourse.bass_utils import run_bass_kernel_spmd
from concourse.masks import make_identity

NCORE = 8
P = 128
H = 256
L = 4
NF = 9
G = 256
N = 10000
BPC = 10                # dst blocks per core
NPC = BPC * P           # 1280 nodes per core
NPAD = NCORE * NPC      # 10240
NCHUNK = NPAD // P      # 80 src chunks
BN_EPS = 1e-5

N_DENSE = int(os.environ.get("KDENSE", "7"))   # dense-adjacency blocks per core
N_GATH = BPC - N_DENSE

f32 = mybir.dt.float32
bf16 = mybir.dt.bfloat16
i16 = mybir.dt.int16
bfnp = ml_dtypes.bfloat16

FT = mybir.ActivationFunctionType
OP = mybir.AluOpType

_compiled = {}


# --------------------------------------------------------------------------
# host-side structural preprocessing
# --------------------------------------------------------------------------

def _preprocess(x, edge_index, batch_ids, emb, W, gamma, beta,
                mlp_W1, mlp_b1, mlp_W2, mlp_b2, mlp_W3, mlp_b3):
    src = np.asarray(edge_index[0], np.int64)
    dst = np.asarray(edge_index[1], np.int64)
    # self loops for every real node (weight nrm[d]^2 folds in)
    src_all = np.concatenate([src, np.arange(N, dtype=np.int64)])
    dst_all = np.concatenate([dst, np.arange(N, dtype=np.int64)])
    order = np.argsort(dst_all, kind="stable")
    s_sorted = src_all[order]
    d_sorted = dst_all[order]

    deg = np.bincount(dst_all, minlength=NPAD).astype(np.float64)  # incl self

    nblk = NCORE * BPC
    starts = np.searchsorted(d_sorted, np.arange(nblk) * P)
    ends = np.searchsorted(d_sorted, (np.arange(nblk) + 1) * P)

    # ---- per-block edge structures ------------------------------------
    # dense blocks (local idx < N_DENSE): A counts [NPAD, P]
    # gathered blocks: deduped (src, dst_local) with multiplicity
    A_blocks = {}          # (c, nb) -> [P, NCHUNK*P] bf16
    uniq_blocks = {}       # (c, nb) -> (srcs_u, dloc_u, cnt)
    T_g = 1
    for g in range(nblk):
        c, nb = divmod(g, BPC)
        e_s = s_sorted[starts[g]:ends[g]]
        e_d = d_sorted[starts[g]:ends[g]] - g * P
        if nb < N_DENSE:
            A = np.zeros((NPAD, P), np.float32)
            np.add.at(A, (e_s, e_d), 1.0)
            A_blocks[(c, nb)] = np.ascontiguousarray(
                A.reshape(NCHUNK, P, P).transpose(1, 0, 2).reshape(P, NCHUNK * P)
            ).astype(bfnp)
        else:
            key = e_s * P + e_d
            uk, cnt = np.unique(key, return_counts=True)
            uniq_blocks[(c, nb)] = (uk // P, uk % P, cnt)
            T_g = max(T_g, (len(uk) + P - 1) // P)

    NI = T_g * P
    NIB = NI // 16

    # graph pool one-hot [node, graph]
    bids = np.asarray(batch_ids, np.int64)
    psel_full = np.zeros((NPAD, G), np.float32)
    psel_full[np.arange(N), bids] = 1.0

    x_np = np.zeros((NPAD, NF), np.float32)
    x_np[:N] = np.asarray(x, np.float64)

    # shared parameter tensors (layout for device)
    Wf = np.asarray(W, np.float32)                       # [L,H,H]
    W_lhsT = Wf.reshape(L, 2, P, H).transpose(2, 0, 1, 3).reshape(P, L * 2 * H)
    gb = np.concatenate([np.asarray(gamma, np.float32).reshape(-1),
                         np.asarray(beta, np.float32).reshape(-1)])[None, :]
    embf = np.asarray(emb, np.float32)
    emb0 = np.ascontiguousarray(embf[:, 0, :])
    emb1 = np.ascontiguousarray(embf[:, 1, :])
    w1 = np.asarray(mlp_W1, np.float32).reshape(2, P, P).transpose(1, 0, 2).reshape(P, 2 * P)
    w2 = np.asarray(mlp_W2, np.float32)
    w3 = np.asarray(mlp_W3, np.float32)
    b1 = np.asarray(mlp_b1, np.float32).reshape(P, 1)
    b2 = np.asarray(mlp_b2, np.float32).reshape(64, 1)
    b3 = np.asarray(mlp_b3, np.float32).reshape(1, 1)

    in_maps = []
    for c in range(NCORE):
        lo, hi = c * NPC, (c + 1) * NPC

        # dense adjacency stream: [P, N_DENSE * NCHUNK * P]
        if N_DENSE:
            Ac = np.concatenate([A_blocks[(c, nb)] for nb in range(N_DENSE)], axis=1)
        else:
            Ac = np.zeros((P, 1), bfnp)

        # gathered blocks: sel [P, N_GATH*T_g*P] bf16, idx [P, N_GATH*NIB] i16
        # pad unused slots with idx 0 (gathers real row 0, sel weight 0) --
        # idx -1 would skip the DMA and leave garbage (possibly NaN) in the
        # tile, and 0 * NaN = NaN would poison the PSUM accumulation.
        selc = np.zeros((N_GATH, T_g, P, P), np.float32)
        idxc = np.zeros((N_GATH, NI), np.int16)
        for j in range(N_GATH):
            nb = N_DENSE + j
            su, du, cnt = uniq_blocks[(c, nb)]
            n = len(su)
            idxc[j, :n] = su.astype(np.int16)
            selc[j].reshape(T_g * P, P)[np.arange(n), du] = cnt
        selc = np.ascontiguousarray(
            selc.reshape(N_GATH * T_g, P, P).transpose(1, 0, 2)
        ).reshape(P, N_GATH * T_g * P)
        idxw = idxc.reshape(N_GATH, NIB, 16).transpose(0, 2, 1)   # [NG,16,NIB]
        idxw = np.tile(idxw, (1, 8, 1))                           # [NG,128,NIB]
        idxw = np.ascontiguousarray(idxw.transpose(1, 0, 2)).reshape(P, N_GATH * NIB)

        degc = deg[lo:hi].reshape(BPC, P).T
        maskc = (degc > 0).astype(np.float32)
        degc = np.maximum(degc, 1.0).astype(np.float32)

        pselc = psel_full[lo:hi].reshape(BPC, P, G)
        pselc = np.ascontiguousarray(pselc.transpose(1, 0, 2)).reshape(P, BPC * G)

        xTc = np.ascontiguousarray(x_np[lo:hi].T)

        in_maps.append(dict(
            A=Ac, selw=selc.astype(bfnp), idx=idxw.astype(np.int16),
            xT=xTc, deg=degc, mask=maskc, psel=pselc,
            W=W_lhsT.astype(bfnp), gb=gb, emb0=emb0, emb1=emb1,
            w1=w1, w2=w2, w3=w3, b1=b1, b2=b2, b3=b3,
        ))
    return T_g, in_maps


# --------------------------------------------------------------------------
# device program
# --------------------------------------------------------------------------

def _build(T_g):
    NI = T_g * P
    NIB = NI // 16
    nc = bacc.Bacc(None, target_bir_lowering=False)

    d_A = nc.dram_tensor("A", [P, max(N_DENSE * NCHUNK * P, 1)], bf16,
                         kind="ExternalInput")
    d_sel = nc.dram_tensor("selw", [P, N_GATH * T_g * P], bf16, kind="ExternalInput")
    d_idx = nc.dram_tensor("idx", [P, N_GATH * NIB], i16, kind="ExternalInput")
    d_xT = nc.dram_tensor("xT", [NF, NPC], f32, kind="ExternalInput")
    d_deg = nc.dram_tensor("deg", [P, BPC], f32, kind="ExternalInput")
    d_mask = nc.dram_tensor("mask", [P, BPC], f32, kind="ExternalInput")
    d_psel = nc.dram_tensor("psel", [P, BPC * G], f32, kind="ExternalInput")
    d_W = nc.dram_tensor("W", [P, L * 2 * H], bf16, kind="ExternalInput")
    d_gb = nc.dram_tensor("gb", [1, 2 * L * H], f32, kind="ExternalInput")
    d_emb0 = nc.dram_tensor("emb0", [NF, H], f32, kind="ExternalInput")
    d_emb1 = nc.dram_tensor("emb1", [NF, H], f32, kind="ExternalInput")
    d_w1 = nc.dram_tensor("w1", [P, 2 * P], f32, kind="ExternalInput")
    d_w2 = nc.dram_tensor("w2", [P, 64], f32, kind="ExternalInput")
    d_w3 = nc.dram_tensor("w3", [64, 1], f32, kind="ExternalInput")
    d_b1 = nc.dram_tensor("b1", [P, 1], f32, kind="ExternalInput")
    d_b2 = nc.dram_tensor("b2", [64, 1], f32, kind="ExternalInput")
    d_b3 = nc.dram_tensor("b3", [1, 1], f32, kind="ExternalInput")
    d_out = nc.dram_tensor("out", [1, G], f32, kind="ExternalOutput")

    rg = [list(range(NCORE))]

    with tile.TileContext(nc) as tc, ExitStack() as ctx:
        pers = ctx.enter_context(tc.tile_pool(name="pers", bufs=1))
        psA = ctx.enter_context(tc.tile_pool(name="psA", bufs=2, space="PSUM"))
        psB = ctx.enter_context(tc.tile_pool(name="psB", bufs=2, space="PSUM"))
        psC = ctx.enter_context(tc.tile_pool(name="psC", bufs=1, space="PSUM"))
        apool = ctx.enter_context(tc.tile_pool(name="apool", bufs=2))
        gpool = ctx.enter_context(tc.tile_pool(name="gpool", bufs=3))
        work = ctx.enter_context(tc.tile_pool(name="work", bufs=2))
        stream = ctx.enter_context(tc.tile_pool(name="stream", bufs=2))
        dram = ctx.enter_context(tc.tile_pool(name="dram", bufs=2, space="DRAM"))

        # ---- persistent SBUF state -------------------------------------
        sel_sb = pers.tile([P, N_GATH * T_g * P], bf16, tag="sel")
        idx_sb = pers.tile([P, N_GATH * NIB], i16, tag="idx")
        deg_sb = pers.tile([P, BPC], f32, tag="deg")
        mask_sb = pers.tile([P, BPC], f32, tag="mask")
        W_sb = pers.tile([P, L * 2 * H], bf16, tag="W")
        gb_sb = pers.tile([1, 2 * L * H], f32, tag="gb")
        emb0_sb = pers.tile([NF, H], f32, tag="emb0")
        emb1_sb = pers.tile([NF, H], f32, tag="emb1")
        w1_sb = pers.tile([P, 2 * P], f32, tag="w1")
        w2_sb = pers.tile([P, 64], f32, tag="w2")
        w3_sb = pers.tile([64, 1], f32, tag="w3")
        b1_sb = pers.tile([P, 1], f32, tag="b1")
        b2_sb = pers.tile([64, 1], f32, tag="b2")
        b3_sb = pers.tile([1, 1], f32, tag="b3")

        tab_sb = pers.tile([P, NCHUNK * H], bf16, tag="tab")
        h_sb = pers.tile([P, BPC * H], f32, tag="h")
        hsT_sb = pers.tile([P, BPC * 2 * P], bf16, tag="hsT")
        hws_sb = pers.tile([P, BPC * H], bf16, tag="hws")
        t_all = pers.tile([P, BPC * H], f32, tag="t_all")
        nrm_sb = pers.tile([P, BPC], f32, tag="nrm")
        acc_s = pers.tile([P, H], f32, tag="acc_s")
        acc_q = pers.tile([P, H], f32, tag="acc_q")
        D_sb = pers.tile([NF, H], f32, tag="D")
        base_rep = pers.tile([P, H], f32, tag="base_rep")
        a_rep = pers.tile([P, H], f32, tag="a_rep")
        c_rep = pers.tile([P, H], f32, tag="c_rep")
        ident_bf = pers.tile([P, P], bf16, tag="ident")
        ones9 = pers.tile([NF, 1], f32, tag="ones9")
        ones1 = pers.tile([1, P], f32, tag="ones1")
        ones128 = pers.tile([P, 1], f32, tag="ones128")
        stv = pers.tile([1, 2 * H], f32, tag="stv")
        scal = pers.tile([1, 8 * H], f32, tag="scal")

        # ---- DRAM bounce buffers ---------------------------------------
        ag_in = dram.tile([NPC, H], bf16, tag="ag_in")
        ag_out = dram.tile([NPAD, H], bf16, tag="ag_out")
        ar_in = dram.tile([P, 2 * H], f32, tag="ar_in")
        ar_out = dram.tile([P, 2 * H], f32, tag="ar_out")
        pr_in = dram.tile([2 * P + 1, G], f32, tag="pr_in")
        pr_out = dram.tile([2 * P + 1, G], f32, tag="pr_out")

        # ---- input loads ------------------------------------------------
        for t, d in [(sel_sb, d_sel), (idx_sb, d_idx),
                     (deg_sb, d_deg), (mask_sb, d_mask),
                     (W_sb, d_W), (gb_sb, d_gb), (emb0_sb, d_emb0),
                     (emb1_sb, d_emb1), (w1_sb, d_w1), (w2_sb, d_w2),
                     (w3_sb, d_w3), (b1_sb, d_b1), (b2_sb, d_b2),
                     (b3_sb, d_b3)]:
            nc.sync.dma_start(out=t[:], in_=d[:])

        nc.gpsimd.load_library(library_config.mlp)
        make_identity(nc, ident_bf[:])
        nc.vector.memset(ones9[:], 1.0)
        nc.vector.memset(ones1[:], 1.0)
        nc.vector.memset(ones128[:], 1.0)

        # nrm = rsqrt(deg) * mask
        rdeg = work.tile([P, BPC], f32, tag="rdeg")
        nc.vector.reciprocal(out=rdeg[:], in_=deg_sb[:])
        nc.scalar.activation(out=rdeg[:], in_=rdeg[:], func=FT.Sqrt)
        nc.vector.tensor_tensor(out=nrm_sb[:], in0=rdeg[:], in1=mask_sb[:], op=OP.mult)

        # encoder prep: D = emb1 - emb0 ; base = ones9^T @ emb0, broadcast
        nc.vector.tensor_tensor(out=D_sb[:], in0=emb1_sb[:], in1=emb0_sb[:], op=OP.subtract)
        ps_b = psB.tile([1, H], f32, tag="vec")
        nc.tensor.matmul(out=ps_b[:], lhsT=ones9[:], rhs=emb0_sb[:], start=True, stop=True)
        bvec = scal[:, 0:H]
        nc.vector.tensor_copy(out=bvec, in_=ps_b[:])
        ps_br = psB.tile([P, H], f32, tag="vec")
        nc.tensor.matmul(out=ps_br[:], lhsT=ones1[:], rhs=bvec, start=True, stop=True)
        nc.vector.tensor_copy(out=base_rep[:], in_=ps_br[:])

        def hslice(nb):
            return h_sb[:, nb * H:(nb + 1) * H]

        def emit_hs_transpose(nb):
            """hs = h*nrm (bf16), transpose both 128-halves into hsT_sb."""
            hs_bf = work.tile([P, H], bf16, tag="hs_bf")
            nc.vector.tensor_scalar_mul(hs_bf[:], hslice(nb), nrm_sb[:, nb:nb + 1])
            for k in range(2):
                pst = psB.tile([P, P], bf16, tag="pst")
                nc.tensor.transpose(out=pst[:], in_=hs_bf[:, k * P:(k + 1) * P],
                                    identity=ident_bf[:])
                nc.vector.tensor_copy(out=hsT_sb[:, (nb * 2 + k) * P:(nb * 2 + k + 1) * P],
                                      in_=pst[:])

        # encoder: h0 = base + xT^T @ D  (per block)
        for nb in range(BPC):
            xT_t = stream.tile([NF, P], f32, tag="xT_t")
            nc.sync.dma_start(out=xT_t[:], in_=d_xT[:, nb * P:(nb + 1) * P])
            ps_h = psA.tile([P, H], f32, tag="mm")
            nc.tensor.matmul(out=ps_h[:], lhsT=xT_t[:],
                             rhs=D_sb[:], start=True, stop=True)
            nc.vector.tensor_tensor(out=hslice(nb), in0=ps_h[:], in1=base_rep[:], op=OP.add)
            emit_hs_transpose(nb)

        # ---- layers -----------------------------------------------------
        for l in range(L):
            # GEMM hws = hs @ W[l]
            for nb in range(BPC):
                ps_g = psA.tile([P, H], f32, tag="mm")
                for k in range(2):
                    nc.tensor.matmul(
                        out=ps_g[:],
                        lhsT=hsT_sb[:, (nb * 2 + k) * P:(nb * 2 + k + 1) * P],
                        rhs=W_sb[:, (l * 2 + k) * H:(l * 2 + k + 1) * H],
                        start=(k == 0), stop=(k == 1))
                nc.vector.tensor_copy(out=hws_sb[:, nb * H:(nb + 1) * H], in_=ps_g[:])
                nc.sync.dma_start(out=ag_in[nb * P:(nb + 1) * P, :],
                                  in_=hws_sb[:, nb * H:(nb + 1) * H])
            nc.gpsimd.collective_compute(
                "AllGather", OP.bypass, replica_groups=rg,
                ins=[ag_in[:]], outs=[ag_out[:]])

            # bulk-load the full table into SBUF: tab_sb[p, c*H:(c+1)*H] =
            # ag_out row c*128+p
            nc.sync.dma_start(
                out=tab_sb[:].rearrange("p (c h) -> p c h", h=H),
                in_=ag_out[:].rearrange("(c p) h -> p c h", p=P))

            nc.vector.memset(acc_s[:], 0.0)
            nc.vector.memset(acc_q[:], 0.0)

            def post_block(nb, ps_t):
                """t = nrm*psum; accumulate BN stats."""
                tsl = t_all[:, nb * H:(nb + 1) * H]
                nc.scalar.activation(out=tsl, in_=ps_t[:], func=FT.Copy,
                                     scale=nrm_sb[:, nb:nb + 1])
                sq = work.tile([P, H], f32, tag="tmp")
                nc.vector.tensor_tensor(out=sq[:], in0=tsl, in1=tsl, op=OP.mult)
                nc.vector.tensor_tensor(out=acc_s[:], in0=acc_s[:], in1=tsl, op=OP.add)
                nc.vector.tensor_tensor(out=acc_q[:], in0=acc_q[:], in1=sq[:], op=OP.add)

            # gathered blocks: issue SWDGE gathers up front (Pool engine runs
            # them while TensorE does the dense blocks); sel chains are
            # interleaved between dense chains so gpool bufs recycle promptly.
            T0 = (T_g + 1) // 2
            chunks = [(0, T0), (T0, T_g)] if T_g > 1 else [(0, T_g)]

            def emit_gather(j):
                gts = []
                for (j0, j1) in chunks:
                    gath = gpool.tile([P, T0 * H], bf16, tag="gath")
                    nc.gpsimd.dma_gather(
                        out_ap=gath[:, :(j1 - j0) * H].rearrange("p (t h) -> p t h", h=H),
                        in_ap=ag_out[:],
                        idxs_ap=idx_sb[:, j * NIB + j0 * 8:j * NIB + j1 * 8],
                        num_idxs=(j1 - j0) * P, num_idxs_reg=(j1 - j0) * P,
                        elem_size=H, single_packet=False)
                    gts.append(gath)
                return gts

            def emit_sel_chain(j, gts):
                nb = N_DENSE + j
                ps_t = psA.tile([P, H], f32, tag="mm")
                for t in range(T_g):
                    ci = 0 if t < T0 else 1
                    tt = t if t < T0 else t - T0
                    nc.tensor.matmul(
                        out=ps_t[:],
                        lhsT=sel_sb[:, (j * T_g + t) * P:(j * T_g + t + 1) * P],
                        rhs=gts[ci][:, tt * H:(tt + 1) * H],
                        start=(t == 0), stop=(t == T_g - 1))
                post_block(nb, ps_t)

            def emit_dense_chain(nb):
                # A streamed in half-blocks of 40 chunks (SBUF footprint)
                HC = NCHUNK // 2
                ps_t = psA.tile([P, H], f32, tag="mm")
                for half in range(2):
                    a_t = apool.tile([P, HC * P], bf16, tag="A")
                    base = (nb * NCHUNK + half * HC) * P
                    nc.sync.dma_start(out=a_t[:], in_=d_A[:, base:base + HC * P])
                    for cc in range(HC):
                        c = half * HC + cc
                        nc.tensor.matmul(
                            out=ps_t[:],
                            lhsT=a_t[:, cc * P:(cc + 1) * P],
                            rhs=tab_sb[:, c * H:(c + 1) * H],
                            start=(c == 0), stop=(c == NCHUNK - 1))
                post_block(nb, ps_t)

            # interleave: gathers 0,1 in flight; after each dense chain,
            # drain one ready gather with its sel chain and issue the next.
            pending = []
            next_g = 0
            if N_GATH and next_g < N_GATH:
                pending.append((next_g, emit_gather(next_g))); next_g += 1
            for nb in range(N_DENSE):
                emit_dense_chain(nb)
                if next_g < N_GATH:
                    pending.append((next_g, emit_gather(next_g))); next_g += 1
                if pending and (len(pending) >= 2 or nb >= N_DENSE - 1):
                    j, gts = pending.pop(0)
                    emit_sel_chain(j, gts)
            while next_g < N_GATH:
                pending.append((next_g, emit_gather(next_g))); next_g += 1
            for j, gts in pending:
                emit_sel_chain(j, gts)

            # stats: cross-partition reduce, replicate to 128 rows, AllReduce
            ps_s = psB.tile([1, 2 * H], f32, tag="vec")
            nc.tensor.matmul(out=ps_s[:, 0:H], lhsT=ones128[:], rhs=acc_s[:],
                             start=True, stop=True)
            nc.tensor.matmul(out=ps_s[:, H:2 * H], lhsT=ones128[:], rhs=acc_q[:],
                             start=True, stop=True)
            st_sb = scal[:, 6 * H:8 * H]
            nc.vector.tensor_copy(out=st_sb, in_=ps_s[:])
            # replicate to [128, 2H] so the AllReduce picks the fast RDH path
            st_rep = work.tile([P, 2 * H], f32, tag="strep")
            for half in range(2):
                ps_r2 = psB.tile([P, H], f32, tag="vec")
                nc.tensor.matmul(out=ps_r2[:], lhsT=ones1[:],
                                 rhs=st_sb[:, half * H:(half + 1) * H],
                                 start=True, stop=True)
                nc.vector.tensor_copy(out=st_rep[:, half * H:(half + 1) * H],
                                      in_=ps_r2[:])
            nc.sync.dma_start(out=ar_in[:], in_=st_rep[:])
            nc.gpsimd.collective_compute(
                "AllReduce", OP.add, replica_groups=rg,
                ins=[ar_in[:]], outs=[ar_out[:]])
            nc.sync.dma_start(out=stv[:], in_=ar_out[0:1, :])

            # a = gamma*istd ; c = beta - mu*a
            mu = scal[:, H:2 * H]
            var = scal[:, 2 * H:3 * H]
            av = scal[:, 3 * H:4 * H]
            cv = scal[:, 4 * H:5 * H]
            msq = scal[:, 5 * H:6 * H]
            nc.vector.tensor_scalar_mul(mu, stv[:, 0:H], 1.0 / N)
            nc.vector.tensor_scalar_mul(var, stv[:, H:2 * H], 1.0 / N)
            nc.vector.tensor_tensor(out=msq, in0=mu, in1=mu, op=OP.mult)
            nc.vector.tensor_tensor(out=var, in0=var, in1=msq, op=OP.subtract)
            nc.vector.tensor_scalar_add(var, var, BN_EPS)
            nc.vector.reciprocal(out=var, in_=var)
            nc.scalar.activation(out=var, in_=var, func=FT.Sqrt)  # istd
            nc.vector.tensor_tensor(out=av, in0=var,
                                    in1=gb_sb[:, l * H:(l + 1) * H], op=OP.mult)
            nc.vector.tensor_tensor(out=msq, in0=mu, in1=av, op=OP.mult)
            nc.vector.tensor_tensor(out=cv, in0=gb_sb[:, (L + l) * H:(L + l + 1) * H],
                                    in1=msq, op=OP.subtract)
            ps_a = psB.tile([P, H], f32, tag="vec")
            nc.tensor.matmul(out=ps_a[:], lhsT=ones1[:], rhs=av, start=True, stop=True)
            nc.vector.tensor_copy(out=a_rep[:], in_=ps_a[:])
            ps_c = psB.tile([P, H], f32, tag="vec")
            nc.tensor.matmul(out=ps_c[:], lhsT=ones1[:], rhs=cv, start=True, stop=True)
            nc.vector.tensor_copy(out=c_rep[:], in_=ps_c[:])

            # h = relu(t*a + c) + h ; prepare hsT for next layer
            for nb in range(BPC):
                tsl = t_all[:, nb * H:(nb + 1) * H]
                u = work.tile([P, H], f32, tag="tmp")
                nc.vector.tensor_tensor(out=u[:], in0=tsl, in1=a_rep[:], op=OP.mult)
                nc.vector.tensor_tensor(out=u[:], in0=u[:], in1=c_rep[:], op=OP.add)
                r = work.tile([P, H], f32, tag="tmp2")
                nc.scalar.activation(out=r[:], in_=u[:], func=FT.Relu)
                nc.vector.tensor_tensor(out=hslice(nb), in0=hslice(nb), in1=r[:], op=OP.add)
                if l < L - 1:
                    emit_hs_transpose(nb)

        # ---- pooling ----------------------------------------------------
        ps_p0 = psC.tile([P, G], f32, tag="p0")
        ps_p1 = psC.tile([P, G], f32, tag="p1")
        ps_pc = psB.tile([1, G], f32, tag="vec")
        for nb in range(BPC):
            psel_t = stream.tile([P, G], f32, tag="psel_t")
            nc.sync.dma_start(out=psel_t[:], in_=d_psel[:, nb * G:(nb + 1) * G])
            pssl = psel_t[:]
            nc.tensor.matmul(out=ps_p0[:], lhsT=h_sb[:, nb * H:nb * H + P],
                             rhs=pssl, start=(nb == 0), stop=(nb == BPC - 1))
            nc.tensor.matmul(out=ps_p1[:], lhsT=h_sb[:, nb * H + P:(nb + 1) * H],
                             rhs=pssl, start=(nb == 0), stop=(nb == BPC - 1))
            nc.tensor.matmul(out=ps_pc[:], lhsT=ones128[:],
                             rhs=pssl, start=(nb == 0), stop=(nb == BPC - 1))
        g0 = work.tile([P, G], f32, tag="g0")
        g1 = work.tile([P, G], f32, tag="g1")
        cnt = scal[:, 0:G]
        nc.vector.tensor_copy(out=g0[:], in_=ps_p0[:])
        nc.vector.tensor_copy(out=g1[:], in_=ps_p1[:])
        nc.vector.tensor_copy(out=cnt, in_=ps_pc[:])
        nc.sync.dma_start(out=pr_in[0:P, :], in_=g0[:])
        nc.sync.dma_start(out=pr_in[P:2 * P, :], in_=g1[:])
        nc.sync.dma_start(out=pr_in[2 * P:2 * P + 1, :], in_=cnt)
        nc.gpsimd.collective_compute(
            "AllReduce", OP.add, replica_groups=rg,
            ins=[pr_in[:]], outs=[pr_out[:]])
        nc.sync.dma_start(out=g0[:], in_=pr_out[0:P, :])
        nc.sync.dma_start(out=g1[:], in_=pr_out[P:2 * P, :])
        nc.sync.dma_start(out=cnt, in_=pr_out[2 * P:2 * P + 1, :])
        nc.vector.tensor_scalar_max(cnt, cnt, 1.0)
        nc.vector.reciprocal(out=cnt, in_=cnt)
        ps_r = psB.tile([P, G], f32, tag="vec")
        nc.tensor.matmul(out=ps_r[:], lhsT=ones1[:], rhs=cnt, start=True, stop=True)
        rc_rep = work.tile([P, G], f32, tag="rc_rep")
        nc.vector.tensor_copy(out=rc_rep[:], in_=ps_r[:])
        nc.vector.tensor_tensor(out=g0[:], in0=g0[:], in1=rc_rep[:], op=OP.mult)
        nc.vector.tensor_tensor(out=g1[:], in0=g1[:], in1=rc_rep[:], op=OP.mult)

        # MLP head (transposed: weights are lhsT, graphs along free dim)
        ps1 = psB.tile([P, G], f32, tag="vec")
        nc.tensor.matmul(out=ps1[:], lhsT=w1_sb[:, 0:P], rhs=g0[:], start=True, stop=False)
        nc.tensor.matmul(out=ps1[:], lhsT=w1_sb[:, P:2 * P], rhs=g1[:], start=False, stop=True)
        y1 = work.tile([P, G], f32, tag="y1")
        nc.scalar.activation(out=y1[:], in_=ps1[:], func=FT.Relu, bias=b1_sb[:, 0:1])
        ps2 = psB.tile([64, G], f32, tag="vec")
        nc.tensor.matmul(out=ps2[:], lhsT=w2_sb[:], rhs=y1[:], start=True, stop=True)
        y2 = work.tile([64, G], f32, tag="y2")
        nc.scalar.activation(out=y2[:], in_=ps2[:], func=FT.Relu, bias=b2_sb[:, 0:1])
        ps3 = psB.tile([1, G], f32, tag="vec")
        nc.tensor.matmul(out=ps3[:], lhsT=w3_sb[:], rhs=y2[:], start=True, stop=True)
        y3 = work.tile([1, G], f32, tag="y3")
        nc.vector.tensor_scalar_add(y3[:], ps3[:], b3_sb[0:1, 0:1])
        nc.sync.dma_start(out=d_out[:], in_=y3[:])

    nc.compile()
    return nc


# --------------------------------------------------------------------------
# entry point
# --------------------------------------------------------------------------

def kernel(x, edge_index, batch_ids, emb, W, b, gamma, beta,
           mlp_W1, mlp_b1, mlp_W2, mlp_b2, mlp_W3, mlp_b3,
           _trace=False, _trace_kwargs=None):
    # NB: reference BN subtracts the per-channel mean, so the additive bias b
    # cancels exactly and is not needed by the device program.
    T_g, in_maps = _preprocess(x, edge_index, batch_ids, emb, W, gamma, beta,
                               mlp_W1, mlp_b1, mlp_W2, mlp_b2, mlp_W3, mlp_b3)
    key = (T_g, N_DENSE)
    if key not in _compiled:
        _compiled[key] = _build(T_g)
    nc = _compiled[key]
    kw = {}
    if _trace:
        kw = dict(trace=True, **(_trace_kwargs or {}))
    res = run_bass_kernel_spmd(nc, in_maps, core_ids=list(range(NCORE)), **kw)
    out = np.asarray(res.results[0]["out"], np.float32).reshape(G, 1)
    kernel._last_results = res
    return out


# revision 11
# speedup vs baseline: 1.5436x; 1.0116x over previous
"""Trainium2 Bass kernel for HIVNet GCN message passing (8-core SPMD).

v2 strategy (vs v1 baseline at 2.29ms):
  - Pad N=10000 nodes to 10240 = 80 blocks x 128; core c owns 10 dst-blocks.
  - Per layer: hws = (h*nrm) @ W[l] on the owned shard (bf16), AllGather into
    a DRAM table on every core, bulk-load the full table into SBUF.
  - Aggregation is HYBRID per dst-block:
      * N_DENSE blocks: dense one-hot adjacency matmuls on TensorE
        (80 src-chunk matmuls accumulated in PSUM, A streamed from HBM,
        prefetched under the AllGather) -- zero GPSIMD cost.
      * remaining blocks: SWDGE dma_gather of DEDUPED (src,dst-block) rows
        (multiplicity folded into the sel weights) + one-hot sel matmuls.
    Split chosen to balance Pool-engine (8.2ns/row) vs TensorE (213ns/matmul).
  - BatchNorm: partial sums/sumsq -> stats replicated to 128 rows so the
    2KB AllReduce becomes a 256KB RDH AllReduce (79us -> ~16us); scale/shift
    broadcast via rank-1 TensorE matmul; relu + residual on DVE.
  - Readout: graph mean-pool via one-hot pool matrices, 257-row AllReduce,
    3-layer MLP computed redundantly on every core.
"""

import os
import sys

sys.path.insert(0, "/opt/trn_rl_repo")

from contextlib import ExitStack

import numpy as np
import ml_dtypes

from concourse import bass, mybir, bacc, tile, library_config
from concourse.bass_utils import run_bass_kernel_spmd
from concourse.masks import make_identity

NCORE = 8
P = 128
H = 256
L = 4
NF = 9
G = 256
N = 10000
BPC = 10                # dst blocks per core
NPC = BPC * P           # 1280 nodes per core
NPAD = NCORE * NPC      # 10240
NCHUNK = NPAD // P      # 80 src chunks
BN_EPS = 1e-5

N_DENSE = int(os.environ.get("KDENSE", "7"))   # dense-adjacency blocks per core
N_GATH = BPC - N_DENSE

f32 = mybir.dt.float32
bf16 = mybir.dt.bfloat16
i16 = mybir.dt.int16
bfnp = ml_dtypes.bfloat16

FT = mybir.ActivationFunctionType
OP = mybir.AluOpType

_compiled = {}


# --------------------------------------------------------------------------
# host-side structural preprocessing
# --------------------------------------------------------------------------

def _preprocess(x, edge_index, batch_ids, emb, W, gamma, beta,
                mlp_W1, mlp_b1, mlp_W2, mlp_b2, mlp_W3, mlp_b3):
    src = np.asarray(edge_index[0], np.int64)
    dst = np.asarray(edge_index[1], np.int64)
    # self loops for every real node (weight nrm[d]^2 folds in)
    src_all = np.concatenate([src, np.arange(N, dtype=np.int64)])
    dst_all = np.concatenate([dst, np.arange(N, dtype=np.int64)])
    order = np.argsort(dst_all, kind="stable")
    s_sorted = src_all[order]
    d_sorted = dst_all[order]

    deg = np.bincount(dst_all, minlength=NPAD).astype(np.float64)  # incl self

    nblk = NCORE * BPC
    starts = np.searchsorted(d_sorted, np.arange(nblk) * P)
    ends = np.searchsorted(d_sorted, (np.arange(nblk) + 1) * P)

    # ---- per-block edge structures ------------------------------------
    # dense blocks (local idx < N_DENSE): A counts [NPAD, P]
    # gathered blocks: deduped (src, dst_local) with multiplicity
    A_blocks = {}          # (c, nb) -> [P, NCHUNK*P] bf16
    uniq_blocks = {}       # (c, nb) -> (srcs_u, dloc_u, cnt)
    T_g = 1
    for g in range(nblk):
        c, nb = divmod(g, BPC)
        e_s = s_sorted[starts[g]:ends[g]]
        e_d = d_sorted[starts[g]:ends[g]] - g * P
        if nb < N_DENSE:
            A = np.zeros((NPAD, P), np.float32)
            np.add.at(A, (e_s, e_d), 1.0)
            A_blocks[(c, nb)] = np.ascontiguousarray(
                A.reshape(NCHUNK, P, P).transpose(1, 0, 2).reshape(P, NCHUNK * P)
            ).astype(bfnp)
        else:
            key = e_s * P + e_d
            uk, cnt = np.unique(key, return_counts=True)
            uniq_blocks[(c, nb)] = (uk // P, uk % P, cnt)
            T_g = max(T_g, (len(uk) + P - 1) // P)

    NI = T_g * P
    NIB = NI // 16

    # graph pool one-hot [node, graph]
    bids = np.asarray(batch_ids, np.int64)
    psel_full = np.zeros((NPAD, G), np.float32)
    psel_full[np.arange(N), bids] = 1.0

    x_np = np.zeros((NPAD, NF), np.float32)
    x_np[:N] = np.asarray(x, np.float64)

    # shared parameter tensors (layout for device)
    Wf = np.asarray(W, np.float32)                       # [L,H,H]
    W_lhsT = Wf.reshape(L, 2, P, H).transpose(2, 0, 1, 3).reshape(P, L * 2 * H)
    gb = np.concatenate([np.asarray(gamma, np.float32).reshape(-1),
                         np.asarray(beta, np.float32).reshape(-1)])[None, :]
    embf = np.asarray(emb, np.float32)
    emb0 = np.ascontiguousarray(embf[:, 0, :])
    emb1 = np.ascontiguousarray(embf[:, 1, :])
    w1 = np.asarray(mlp_W1, np.float32).reshape(2, P, P).transpose(1, 0, 2).reshape(P, 2 * P)
    w2 = np.asarray(mlp_W2, np.float32)
    w3 = np.asarray(mlp_W3, np.float32)
    b1 = np.asarray(mlp_b1, np.float32).reshape(P, 1)
    b2 = np.asarray(mlp_b2, np.float32).reshape(64, 1)
    b3 = np.asarray(mlp_b3, np.float32).reshape(1, 1)

    in_maps = []
    for c in range(NCORE):
        lo, hi = c * NPC, (c + 1) * NPC

        # dense adjacency stream: [P, N_DENSE * NCHUNK * P]
        if N_DENSE:
            Ac = np.concatenate([A_blocks[(c, nb)] for nb in range(N_DENSE)], axis=1)
        else:
            Ac = np.zeros((P, 1), bfnp)

        # gathered blocks: sel [P, N_GATH*T_g*P] bf16, idx [P, N_GATH*NIB] i16
        # pad unused slots with idx 0 (gathers real row 0, sel weight 0) --
        # idx -1 would skip the DMA and leave garbage (possibly NaN) in the
        # tile, and 0 * NaN = NaN would poison the PSUM accumulation.
        selc = np.zeros((N_GATH, T_g, P, P), np.float32)
        idxc = np.zeros((N_GATH, NI), np.int16)
        for j in range(N_GATH):
            nb = N_DENSE + j
            su, du, cnt = uniq_blocks[(c, nb)]
            n = len(su)
            idxc[j, :n] = su.astype(np.int16)
            selc[j].reshape(T_g * P, P)[np.arange(n), du] = cnt
        selc = np.ascontiguousarray(
            selc.reshape(N_GATH * T_g, P, P).transpose(1, 0, 2)
        ).reshape(P, N_GATH * T_g * P)
        idxw = idxc.reshape(N_GATH, NIB, 16).transpose(0, 2, 1)   # [NG,16,NIB]
        idxw = np.tile(idxw, (1, 8, 1))                           # [NG,128,NIB]
        idxw = np.ascontiguousarray(idxw.transpose(1, 0, 2)).reshape(P, N_GATH * NIB)

        degc = deg[lo:hi].reshape(BPC, P).T
        maskc = (degc > 0).astype(np.float32)
        degc = np.maximum(degc, 1.0).astype(np.float32)

        pselc = psel_full[lo:hi].reshape(BPC, P, G)
        pselc = np.ascontiguousarray(pselc.transpose(1, 0, 2)).reshape(P, BPC * G)

        xTc = np.ascontiguousarray(x_np[lo:hi].T)

        in_maps.append(dict(
            A=Ac, selw=selc.astype(bfnp), idx=idxw.astype(np.int16),
            xT=xTc, deg=degc, mask=maskc, psel=pselc,
            W=W_lhsT.astype(bfnp), gb=gb, emb0=emb0, emb1=emb1,
            w1=w1, w2=w2, w3=w3, b1=b1, b2=b2, b3=b3,
        ))
    return T_g, in_maps


# --------------------------------------------------------------------------
# device program
# --------------------------------------------------------------------------

def _build(T_g):
    NI = T_g * P
    NIB = NI // 16
    nc = bacc.Bacc(None, target_bir_lowering=False)

    d_A = nc.dram_tensor("A", [P, max(N_DENSE * NCHUNK * P, 1)], bf16,
                         kind="ExternalInput")
    d_sel = nc.dram_tensor("selw", [P, N_GATH * T_g * P], bf16, kind="ExternalInput")
    d_idx = nc.dram_tensor("idx", [P, N_GATH * NIB], i16, kind="ExternalInput")
    d_xT = nc.dram_tensor("xT", [NF, NPC], f32, kind="ExternalInput")
    d_deg = nc.dram_tensor("deg", [P, BPC], f32, kind="ExternalInput")
    d_mask = nc.dram_tensor("mask", [P, BPC], f32, kind="ExternalInput")
    d_psel = nc.dram_tensor("psel", [P, BPC * G], f32, kind="ExternalInput")
    d_W = nc.dram_tensor("W", [P, L * 2 * H], bf16, kind="ExternalInput")
    d_gb = nc.dram_tensor("gb", [1, 2 * L * H], f32, kind="ExternalInput")
    d_emb0 = nc.dram_tensor("emb0", [NF, H], f32, kind="ExternalInput")
    d_emb1 = nc.dram_tensor("emb1", [NF, H], f32, kind="ExternalInput")
    d_w1 = nc.dram_tensor("w1", [P, 2 * P], f32, kind="ExternalInput")
    d_w2 = nc.dram_tensor("w2", [P, 64], f32, kind="ExternalInput")
    d_w3 = nc.dram_tensor("w3", [64, 1], f32, kind="ExternalInput")
    d_b1 = nc.dram_tensor("b1", [P, 1], f32, kind="ExternalInput")
    d_b2 = nc.dram_tensor("b2", [64, 1], f32, kind="ExternalInput")
    d_b3 = nc.dram_tensor("b3", [1, 1], f32, kind="ExternalInput")
    d_out = nc.dram_tensor("out", [1, G], f32, kind="ExternalOutput")

    rg = [list(range(NCORE))]

    with tile.TileContext(nc) as tc, ExitStack() as ctx:
        pers = ctx.enter_context(tc.tile_pool(name="pers", bufs=1))
        psA = ctx.enter_context(tc.tile_pool(name="psA", bufs=2, space="PSUM"))
        psB = ctx.enter_context(tc.tile_pool(name="psB", bufs=2, space="PSUM"))
        psC = ctx.enter_context(tc.tile_pool(name="psC", bufs=1, space="PSUM"))
        apool = ctx.enter_context(tc.tile_pool(name="apool", bufs=3))
        gpool = ctx.enter_context(tc.tile_pool(name="gpool", bufs=2))
        work = ctx.enter_context(tc.tile_pool(name="work", bufs=2))
        stream = ctx.enter_context(tc.tile_pool(name="stream", bufs=2))
        dram = ctx.enter_context(tc.tile_pool(name="dram", bufs=2, space="DRAM"))

        # ---- persistent SBUF state -------------------------------------
        sel_sb = pers.tile([P, N_GATH * T_g * P], bf16, tag="sel")
        idx_sb = pers.tile([P, N_GATH * NIB], i16, tag="idx")
        deg_sb = pers.tile([P, BPC], f32, tag="deg")
        mask_sb = pers.tile([P, BPC], f32, tag="mask")
        W_sb = pers.tile([P, L * 2 * H], bf16, tag="W")
        gb_sb = pers.tile([1, 2 * L * H], f32, tag="gb")
        emb0_sb = pers.tile([NF, H], f32, tag="emb0")
        emb1_sb = pers.tile([NF, H], f32, tag="emb1")
        w1_sb = pers.tile([P, 2 * P], f32, tag="w1")
        w2_sb = pers.tile([P, 64], f32, tag="w2")
        w3_sb = pers.tile([64, 1], f32, tag="w3")
        b1_sb = pers.tile([P, 1], f32, tag="b1")
        b2_sb = pers.tile([64, 1], f32, tag="b2")
        b3_sb = pers.tile([1, 1], f32, tag="b3")

        tab_sb = pers.tile([P, NCHUNK * H], bf16, tag="tab")
        h_sb = pers.tile([P, BPC * H], f32, tag="h")
        hsT_sb = pers.tile([P, BPC * 2 * P], bf16, tag="hsT")
        hws_sb = pers.tile([P, BPC * H], bf16, tag="hws")
        t_all = pers.tile([P, BPC * H], f32, tag="t_all")
        nrm_sb = pers.tile([P, BPC], f32, tag="nrm")
        acc_s = pers.tile([P, H], f32, tag="acc_s")
        acc_q = pers.tile([P, H], f32, tag="acc_q")
        D_sb = pers.tile([NF, H], f32, tag="D")
        base_rep = pers.tile([P, H], f32, tag="base_rep")
        a_rep = pers.tile([P, H], f32, tag="a_rep")
        c_rep = pers.tile([P, H], f32, tag="c_rep")
        ident_bf = pers.tile([P, P], bf16, tag="ident")
        ones9 = pers.tile([NF, 1], f32, tag="ones9")
        ones1 = pers.tile([1, P], f32, tag="ones1")
        ones128 = pers.tile([P, 1], f32, tag="ones128")
        stv = pers.tile([1, 2 * H], f32, tag="stv")
        scal = pers.tile([1, 8 * H], f32, tag="scal")

        # ---- DRAM bounce buffers ---------------------------------------
        ag_in = dram.tile([NPC, H], bf16, tag="ag_in")
        ag_out = dram.tile([NPAD, H], bf16, tag="ag_out")
        ar_in = dram.tile([P, 2 * H], f32, tag="ar_in")
        ar_out = dram.tile([P, 2 * H], f32, tag="ar_out")
        pr_in = dram.tile([2 * P + 1, G], f32, tag="pr_in")
        pr_out = dram.tile([2 * P + 1, G], f32, tag="pr_out")

        # ---- input loads ------------------------------------------------
        for t, d in [(sel_sb, d_sel), (idx_sb, d_idx),
                     (deg_sb, d_deg), (mask_sb, d_mask),
                     (W_sb, d_W), (gb_sb, d_gb), (emb0_sb, d_emb0),
                     (emb1_sb, d_emb1), (w1_sb, d_w1), (w2_sb, d_w2),
                     (w3_sb, d_w3), (b1_sb, d_b1), (b2_sb, d_b2),
                     (b3_sb, d_b3)]:
            nc.sync.dma_start(out=t[:], in_=d[:])

        nc.gpsimd.load_library(library_config.mlp)
        make_identity(nc, ident_bf[:])
        nc.vector.memset(ones9[:], 1.0)
        nc.vector.memset(ones1[:], 1.0)
        nc.vector.memset(ones128[:], 1.0)

        # nrm = rsqrt(deg) * mask
        rdeg = work.tile([P, BPC], f32, tag="rdeg")
        nc.vector.reciprocal(out=rdeg[:], in_=deg_sb[:])
        nc.scalar.activation(out=rdeg[:], in_=rdeg[:], func=FT.Sqrt)
        nc.vector.tensor_tensor(out=nrm_sb[:], in0=rdeg[:], in1=mask_sb[:], op=OP.mult)

        # encoder prep: D = emb1 - emb0 ; base = ones9^T @ emb0, broadcast
        nc.vector.tensor_tensor(out=D_sb[:], in0=emb1_sb[:], in1=emb0_sb[:], op=OP.subtract)
        ps_b = psB.tile([1, H], f32, tag="vec")
        nc.tensor.matmul(out=ps_b[:], lhsT=ones9[:], rhs=emb0_sb[:], start=True, stop=True)
        bvec = scal[:, 0:H]
        nc.vector.tensor_copy(out=bvec, in_=ps_b[:])
        ps_br = psB.tile([P, H], f32, tag="vec")
        nc.tensor.matmul(out=ps_br[:], lhsT=ones1[:], rhs=bvec, start=True, stop=True)
        nc.vector.tensor_copy(out=base_rep[:], in_=ps_br[:])

        def hslice(nb):
            return h_sb[:, nb * H:(nb + 1) * H]

        def emit_hs_transpose(nb):
            """hs = h*nrm (bf16), transpose both 128-halves into hsT_sb."""
            hs_bf = work.tile([P, H], bf16, tag="hs_bf")
            nc.vector.tensor_scalar_mul(hs_bf[:], hslice(nb), nrm_sb[:, nb:nb + 1])
            for k in range(2):
                pst = psB.tile([P, P], bf16, tag="pst")
                nc.tensor.transpose(out=pst[:], in_=hs_bf[:, k * P:(k + 1) * P],
                                    identity=ident_bf[:])
                nc.vector.tensor_copy(out=hsT_sb[:, (nb * 2 + k) * P:(nb * 2 + k + 1) * P],
                                      in_=pst[:])

        # encoder: h0 = base + xT^T @ D  (per block)
        for nb in range(BPC):
            xT_t = stream.tile([NF, P], f32, tag="xT_t")
            nc.sync.dma_start(out=xT_t[:], in_=d_xT[:, nb * P:(nb + 1) * P])
            ps_h = psA.tile([P, H], f32, tag="mm")
            nc.tensor.matmul(out=ps_h[:], lhsT=xT_t[:],
                             rhs=D_sb[:], start=True, stop=True)
            nc.vector.tensor_tensor(out=hslice(nb), in0=ps_h[:], in1=base_rep[:], op=OP.add)
            emit_hs_transpose(nb)

        # ---- layers -----------------------------------------------------
        for l in range(L):
            # GEMM hws = hs @ W[l]
            for nb in range(BPC):
                ps_g = psA.tile([P, H], f32, tag="mm")
                for k in range(2):
                    nc.tensor.matmul(
                        out=ps_g[:],
                        lhsT=hsT_sb[:, (nb * 2 + k) * P:(nb * 2 + k + 1) * P],
                        rhs=W_sb[:, (l * 2 + k) * H:(l * 2 + k + 1) * H],
                        start=(k == 0), stop=(k == 1))
                nc.vector.tensor_copy(out=hws_sb[:, nb * H:(nb + 1) * H], in_=ps_g[:])
                nc.sync.dma_start(out=ag_in[nb * P:(nb + 1) * P, :],
                                  in_=hws_sb[:, nb * H:(nb + 1) * H])

            # prefetch the first dense block's A halves BEFORE the collective
            # so the loads run under the AllGather (they have no data deps)
            HC = NCHUNK // 2
            a_fifo = []

            def a_prefetch(nh):
                a_t = apool.tile([P, HC * P], bf16, tag="A")
                nc.sync.dma_start(out=a_t[:], in_=d_A[:, nh * HC * P:(nh + 1) * HC * P])
                a_fifo.append(a_t)

            n_halves = 2 * N_DENSE
            for nh in range(min(2, n_halves)):
                a_prefetch(nh)

            nc.gpsimd.collective_compute(
                "AllGather", OP.bypass, replica_groups=rg,
                ins=[ag_in[:]], outs=[ag_out[:]])

            # bulk-load the full table into SBUF: tab_sb[p, c*H:(c+1)*H] =
            # ag_out row c*128+p
            nc.sync.dma_start(
                out=tab_sb[:].rearrange("p (c h) -> p c h", h=H),
                in_=ag_out[:].rearrange("(c p) h -> p c h", p=P))

            nc.vector.memset(acc_s[:], 0.0)
            nc.vector.memset(acc_q[:], 0.0)

            def post_block(nb, ps_t):
                """t = nrm*psum; accumulate BN stats."""
                tsl = t_all[:, nb * H:(nb + 1) * H]
                nc.scalar.activation(out=tsl, in_=ps_t[:], func=FT.Copy,
                                     scale=nrm_sb[:, nb:nb + 1])
                sq = work.tile([P, H], f32, tag="tmp")
                nc.vector.tensor_tensor(out=sq[:], in0=tsl, in1=tsl, op=OP.mult)
                nc.vector.tensor_tensor(out=acc_s[:], in0=acc_s[:], in1=tsl, op=OP.add)
                nc.vector.tensor_tensor(out=acc_q[:], in0=acc_q[:], in1=sq[:], op=OP.add)

            # gathered blocks: issue SWDGE gathers up front (Pool engine runs
            # them while TensorE does the dense blocks); sel chains are
            # interleaved between dense chains so gpool bufs recycle promptly.
            T0 = (T_g + 1) // 2
            chunks = [(0, T0), (T0, T_g)] if T_g > 1 else [(0, T_g)]

            def emit_gather(j):
                gts = []
                for (j0, j1) in chunks:
                    gath = gpool.tile([P, T0 * H], bf16, tag="gath")
                    nc.gpsimd.dma_gather(
                        out_ap=gath[:, :(j1 - j0) * H].rearrange("p (t h) -> p t h", h=H),
                        in_ap=ag_out[:],
                        idxs_ap=idx_sb[:, j * NIB + j0 * 8:j * NIB + j1 * 8],
                        num_idxs=(j1 - j0) * P, num_idxs_reg=(j1 - j0) * P,
                        elem_size=H, single_packet=False)
                    gts.append(gath)
                return gts

            def emit_sel_chain(j, gts):
                nb = N_DENSE + j
                ps_t = psA.tile([P, H], f32, tag="mm")
                for t in range(T_g):
                    ci = 0 if t < T0 else 1
                    tt = t if t < T0 else t - T0
                    nc.tensor.matmul(
                        out=ps_t[:],
                        lhsT=sel_sb[:, (j * T_g + t) * P:(j * T_g + t + 1) * P],
                        rhs=gts[ci][:, tt * H:(tt + 1) * H],
                        start=(t == 0), stop=(t == T_g - 1))
                post_block(nb, ps_t)

            def emit_dense_chain(nb):
                # A streamed in half-blocks of 40 chunks, software-pipelined:
                # the load for half h+2 is issued before consuming half h.
                ps_t = psA.tile([P, H], f32, tag="mm")
                for half in range(2):
                    nh = nb * 2 + half
                    if nh + 2 < n_halves:
                        a_prefetch(nh + 2)
                    a_t = a_fifo.pop(0)
                    for cc in range(HC):
                        c = half * HC + cc
                        nc.tensor.matmul(
                            out=ps_t[:],
                            lhsT=a_t[:, cc * P:(cc + 1) * P],
                            rhs=tab_sb[:, c * H:(c + 1) * H],
                            start=(c == 0), stop=(c == NCHUNK - 1))
                post_block(nb, ps_t)

            # interleave: gathers 0,1 in flight; after each dense chain,
            # drain one ready gather with its sel chain and issue the next.
            pending = []
            next_g = 0
            if N_GATH and next_g < N_GATH:
                pending.append((next_g, emit_gather(next_g))); next_g += 1
            for nb in range(N_DENSE):
                emit_dense_chain(nb)
                if next_g < N_GATH:
                    pending.append((next_g, emit_gather(next_g))); next_g += 1
                if pending and (len(pending) >= 2 or nb >= N_DENSE - 1):
                    j, gts = pending.pop(0)
                    emit_sel_chain(j, gts)
            while next_g < N_GATH:
                pending.append((next_g, emit_gather(next_g))); next_g += 1
            for j, gts in pending:
                emit_sel_chain(j, gts)

            # stats: cross-partition reduce, replicate to 128 rows, AllReduce
            ps_s = psB.tile([1, 2 * H], f32, tag="vec")
            nc.tensor.matmul(out=ps_s[:, 0:H], lhsT=ones128[:], rhs=acc_s[:],
                             start=True, stop=True)
            nc.tensor.matmul(out=ps_s[:, H:2 * H], lhsT=ones128[:], rhs=acc_q[:],
                             start=True, stop=True)
            st_sb = scal[:, 6 * H:8 * H]
            nc.vector.tensor_copy(out=st_sb, in_=ps_s[:])
            # replicate to [128, 2H] so the AllReduce picks the fast RDH path
            st_rep = work.tile([P, 2 * H], f32, tag="strep")
            for half in range(2):
                ps_r2 = psB.tile([P, H], f32, tag="vec")
                nc.tensor.matmul(out=ps_r2[:], lhsT=ones1[:],
                                 rhs=st_sb[:, half * H:(half + 1) * H],
                                 start=True, stop=True)
                nc.vector.tensor_copy(out=st_rep[:, half * H:(half + 1) * H],
                                      in_=ps_r2[:])
            nc.sync.dma_start(out=ar_in[:], in_=st_rep[:])
            nc.gpsimd.collective_compute(
                "AllReduce", OP.add, replica_groups=rg,
                ins=[ar_in[:]], outs=[ar_out[:]])
            nc.sync.dma_start(out=stv[:], in_=ar_out[0:1, :])

            # a = gamma*istd ; c = beta - mu*a
            mu = scal[:, H:2 * H]
            var = scal[:, 2 * H:3 * H]
            av = scal[:, 3 * H:4 * H]
            cv = scal[:, 4 * H:5 * H]
            msq = scal[:, 5 * H:6 * H]
            nc.vector.tensor_scalar_mul(mu, stv[:, 0:H], 1.0 / N)
            nc.vector.tensor_scalar_mul(var, stv[:, H:2 * H], 1.0 / N)
            nc.vector.tensor_tensor(out=msq, in0=mu, in1=mu, op=OP.mult)
            nc.vector.tensor_tensor(out=var, in0=var, in1=msq, op=OP.subtract)
            nc.vector.tensor_scalar_add(var, var, BN_EPS)
            nc.vector.reciprocal(out=var, in_=var)
            nc.scalar.activation(out=var, in_=var, func=FT.Sqrt)  # istd
            nc.vector.tensor_tensor(out=av, in0=var,
                                    in1=gb_sb[:, l * H:(l + 1) * H], op=OP.mult)
            nc.vector.tensor_tensor(out=msq, in0=mu, in1=av, op=OP.mult)
            nc.vector.tensor_tensor(out=cv, in0=gb_sb[:, (L + l) * H:(L + l + 1) * H],
                                    in1=msq, op=OP.subtract)
            ps_a = psB.tile([P, H], f32, tag="vec")
            nc.tensor.matmul(out=ps_a[:], lhsT=ones1[:], rhs=av, start=True, stop=True)
            nc.vector.tensor_copy(out=a_rep[:], in_=ps_a[:])
            ps_c = psB.tile([P, H], f32, tag="vec")
            nc.tensor.matmul(out=ps_c[:], lhsT=ones1[:], rhs=cv, start=True, stop=True)
            nc.vector.tensor_copy(out=c_rep[:], in_=ps_c[:])

            # h = relu(t*a + c) + h ; prepare hsT for next layer
            for nb in range(BPC):
                tsl = t_all[:, nb * H:(nb + 1) * H]
                u = work.tile([P, H], f32, tag="tmp")
                nc.vector.tensor_tensor(out=u[:], in0=tsl, in1=a_rep[:], op=OP.mult)
                nc.vector.tensor_tensor(out=u[:], in0=u[:], in1=c_rep[:], op=OP.add)
                r = work.tile([P, H], f32, tag="tmp2")
                nc.scalar.activation(out=r[:], in_=u[:], func=FT.Relu)
                nc.vector.tensor_tensor(out=hslice(nb), in0=hslice(nb), in1=r[:], op=OP.add)
                if l < L - 1:
                    emit_hs_transpose(nb)

        # ---- pooling ----------------------------------------------------
        ps_p0 = psC.tile([P, G], f32, tag="p0")
        ps_p1 = psC.tile([P, G], f32, tag="p1")
        ps_pc = psB.tile([1, G], f32, tag="vec")
        for nb in range(BPC):
            psel_t = stream.tile([P, G], f32, tag="psel_t")
            nc.sync.dma_start(out=psel_t[:], in_=d_psel[:, nb * G:(nb + 1) * G])
            pssl = psel_t[:]
            nc.tensor.matmul(out=ps_p0[:], lhsT=h_sb[:, nb * H:nb * H + P],
                             rhs=pssl, start=(nb == 0), stop=(nb == BPC - 1))
            nc.tensor.matmul(out=ps_p1[:], lhsT=h_sb[:, nb * H + P:(nb + 1) * H],
                             rhs=pssl, start=(nb == 0), stop=(nb == BPC - 1))
            nc.tensor.matmul(out=ps_pc[:], lhsT=ones128[:],
                             rhs=pssl, start=(nb == 0), stop=(nb == BPC - 1))
        g0 = work.tile([P, G], f32, tag="g0")
        g1 = work.tile([P, G], f32, tag="g1")
        cnt = scal[:, 0:G]
        nc.vector.tensor_copy(out=g0[:], in_=ps_p0[:])
        nc.vector.tensor_copy(out=g1[:], in_=ps_p1[:])
        nc.vector.tensor_copy(out=cnt, in_=ps_pc[:])
        nc.sync.dma_start(out=pr_in[0:P, :], in_=g0[:])
        nc.sync.dma_start(out=pr_in[P:2 * P, :], in_=g1[:])
        nc.sync.dma_start(out=pr_in[2 * P:2 * P + 1, :], in_=cnt)
        nc.gpsimd.collective_compute(
            "AllReduce", OP.add, replica_groups=rg,
            ins=[pr_in[:]], outs=[pr_out[:]])
        nc.sync.dma_start(out=g0[:], in_=pr_out[0:P, :])
        nc.sync.dma_start(out=g1[:], in_=pr_out[P:2 * P, :])
        nc.sync.dma_start(out=cnt, in_=pr_out[2 * P:2 * P + 1, :])
        nc.vector.tensor_scalar_max(cnt, cnt, 1.0)
        nc.vector.reciprocal(out=cnt, in_=cnt)
        ps_r = psB.tile([P, G], f32, tag="vec")
        nc.tensor.matmul(out=ps_r[:], lhsT=ones1[:], rhs=cnt, start=True, stop=True)
        rc_rep = work.tile([P, G], f32, tag="rc_rep")
        nc.vector.tensor_copy(out=rc_rep[:], in_=ps_r[:])
        nc.vector.tensor_tensor(out=g0[:], in0=g0[:], in1=rc_rep[:], op=OP.mult)
        nc.vector.tensor_tensor(out=g1[:], in0=g1[:], in1=rc_rep[:], op=OP.mult)

        # MLP head (transposed: weights are lhsT, graphs along free dim)
        ps1 = psB.tile([P, G], f32, tag="vec")
        nc.tensor.matmul(out=ps1[:], lhsT=w1_sb[:, 0:P], rhs=g0[:], start=True, stop=False)
        nc.tensor.matmul(out=ps1[:], lhsT=w1_sb[:, P:2 * P], rhs=g1[:], start=False, stop=True)
        y1 = work.tile([P, G], f32, tag="y1")
        nc.scalar.activation(out=y1[:], in_=ps1[:], func=FT.Relu, bias=b1_sb[:, 0:1])
        ps2 = psB.tile([64, G], f32, tag="vec")
        nc.tensor.matmul(out=ps2[:], lhsT=w2_sb[:], rhs=y1[:], start=True, stop=True)
        y2 = work.tile([64, G], f32, tag="y2")
        nc.scalar.activation(out=y2[:], in_=ps2[:], func=FT.Relu, bias=b2_sb[:, 0:1])
        ps3 = psB.tile([1, G], f32, tag="vec")
        nc.tensor.matmul(out=ps3[:], lhsT=w3_sb[:], rhs=y2[:], start=True, stop=True)
        y3 = work.tile([1, G], f32, tag="y3")
        nc.vector.tensor_scalar_add(y3[:], ps3[:], b3_sb[0:1, 0:1])
        nc.sync.dma_start(out=d_out[:], in_=y3[:])

    nc.compile()
    return nc


# --------------------------------------------------------------------------
# entry point
# --------------------------------------------------------------------------

def kernel(x, edge_index, batch_ids, emb, W, b, gamma, beta,
           mlp_W1, mlp_b1, mlp_W2, mlp_b2, mlp_W3, mlp_b3,
           _trace=False, _trace_kwargs=None):
    # NB: reference BN subtracts the per-channel mean, so the additive bias b
    # cancels exactly and is not needed by the device program.
    T_g, in_maps = _preprocess(x, edge_index, batch_ids, emb, W, gamma, beta,
                               mlp_W1, mlp_b1, mlp_W2, mlp_b2, mlp_W3, mlp_b3)
    key = (T_g, N_DENSE)
    if key not in _compiled:
        _compiled[key] = _build(T_g)
    nc = _compiled[key]
    kw = {}
    if _trace:
        kw = dict(trace=True, **(_trace_kwargs or {}))
    res = run_bass_kernel_spmd(nc, in_maps, core_ids=list(range(NCORE)), **kw)
    out = np.asarray(res.results[0]["out"], np.float32).reshape(G, 1)
    kernel._last_results = res
    return out


# revision 16
# speedup vs baseline: 1.5470x; 1.0022x over previous
"""Trainium2 Bass kernel for HIVNet GCN message passing (8-core SPMD).

v2 strategy (vs v1 baseline at 2.29ms):
  - Pad N=10000 nodes to 10240 = 80 blocks x 128; core c owns 10 dst-blocks.
  - Per layer: hws = (h*nrm) @ W[l] on the owned shard (bf16), AllGather into
    a DRAM table on every core, bulk-load the full table into SBUF.
  - Aggregation is HYBRID per dst-block:
      * N_DENSE blocks: dense one-hot adjacency matmuls on TensorE
        (80 src-chunk matmuls accumulated in PSUM, A streamed from HBM,
        prefetched under the AllGather) -- zero GPSIMD cost.
      * remaining blocks: SWDGE dma_gather of DEDUPED (src,dst-block) rows
        (multiplicity folded into the sel weights) + one-hot sel matmuls.
    Split chosen to balance Pool-engine (8.2ns/row) vs TensorE (213ns/matmul).
  - BatchNorm: partial sums/sumsq -> stats replicated to 128 rows so the
    2KB AllReduce becomes a 256KB RDH AllReduce (79us -> ~16us); scale/shift
    broadcast via rank-1 TensorE matmul; relu + residual on DVE.
  - Readout: graph mean-pool via one-hot pool matrices, 257-row AllReduce,
    3-layer MLP computed redundantly on every core.
"""

import os
import sys

sys.path.insert(0, "/opt/trn_rl_repo")

from contextlib import ExitStack

import numpy as np
import ml_dtypes

from concourse import bass, mybir, bacc, tile, library_config
from concourse.bass_utils import run_bass_kernel_spmd
from concourse.masks import make_identity

NCORE = 8
P = 128
H = 256
L = 4
NF = 9
G = 256
N = 10000
BPC = 10                # dst blocks per core
NPC = BPC * P           # 1280 nodes per core
NPAD = NCORE * NPC      # 10240
NCHUNK = NPAD // P      # 80 src chunks
BN_EPS = 1e-5

N_DENSE = int(os.environ.get("KDENSE", "7"))   # dense-adjacency blocks per core
N_GATH = BPC - N_DENSE

f32 = mybir.dt.float32
bf16 = mybir.dt.bfloat16
i16 = mybir.dt.int16
bfnp = ml_dtypes.bfloat16

FT = mybir.ActivationFunctionType
OP = mybir.AluOpType

_compiled = {}


# --------------------------------------------------------------------------
# host-side structural preprocessing
# --------------------------------------------------------------------------

def _preprocess(x, edge_index, batch_ids, emb, W, gamma, beta,
                mlp_W1, mlp_b1, mlp_W2, mlp_b2, mlp_W3, mlp_b3):
    src = np.asarray(edge_index[0], np.int64)
    dst = np.asarray(edge_index[1], np.int64)
    # self loops for every real node (weight nrm[d]^2 folds in)
    src_all = np.concatenate([src, np.arange(N, dtype=np.int64)])
    dst_all = np.concatenate([dst, np.arange(N, dtype=np.int64)])
    order = np.argsort(dst_all, kind="stable")
    s_sorted = src_all[order]
    d_sorted = dst_all[order]

    deg = np.bincount(dst_all, minlength=NPAD).astype(np.float64)  # incl self

    nblk = NCORE * BPC
    starts = np.searchsorted(d_sorted, np.arange(nblk) * P)
    ends = np.searchsorted(d_sorted, (np.arange(nblk) + 1) * P)

    # ---- per-block edge structures ------------------------------------
    # dense blocks (local idx < N_DENSE): A counts [NPAD, P]
    # gathered blocks: deduped (src, dst_local) with multiplicity
    A_blocks = {}          # (c, nb) -> [P, NCHUNK*P] bf16
    uniq_blocks = {}       # (c, nb) -> (srcs_u, dloc_u, cnt)
    T_g = 1
    for g in range(nblk):
        c, nb = divmod(g, BPC)
        e_s = s_sorted[starts[g]:ends[g]]
        e_d = d_sorted[starts[g]:ends[g]] - g * P
        if nb < N_DENSE:
            A = np.zeros((NPAD, P), np.float32)
            np.add.at(A, (e_s, e_d), 1.0)
            A_blocks[(c, nb)] = np.ascontiguousarray(
                A.reshape(NCHUNK, P, P).transpose(1, 0, 2).reshape(P, NCHUNK * P)
            ).astype(bfnp)
        else:
            key = e_s * P + e_d
            uk, cnt = np.unique(key, return_counts=True)
            uniq_blocks[(c, nb)] = (uk // P, uk % P, cnt)
            T_g = max(T_g, (len(uk) + P - 1) // P)

    NI = T_g * P
    NIB = NI // 16

    # graph pool one-hot [node, graph]
    bids = np.asarray(batch_ids, np.int64)
    psel_full = np.zeros((NPAD, G), np.float32)
    psel_full[np.arange(N), bids] = 1.0

    x_np = np.zeros((NPAD, NF), np.float32)
    x_np[:N] = np.asarray(x, np.float64)

    # shared parameter tensors (layout for device)
    Wf = np.asarray(W, np.float32)                       # [L,H,H]
    W_lhsT = Wf.reshape(L, 2, P, H).transpose(2, 0, 1, 3).reshape(P, L * 2 * H)
    gb = np.concatenate([np.asarray(gamma, np.float32).reshape(-1),
                         np.asarray(beta, np.float32).reshape(-1)])[None, :]
    embf = np.asarray(emb, np.float32)
    emb0 = np.ascontiguousarray(embf[:, 0, :])
    emb1 = np.ascontiguousarray(embf[:, 1, :])
    w1 = np.asarray(mlp_W1, np.float32).reshape(2, P, P).transpose(1, 0, 2).reshape(P, 2 * P)
    w2 = np.asarray(mlp_W2, np.float32)
    w3 = np.asarray(mlp_W3, np.float32)
    b1 = np.asarray(mlp_b1, np.float32).reshape(P, 1)
    b2 = np.asarray(mlp_b2, np.float32).reshape(64, 1)
    b3 = np.asarray(mlp_b3, np.float32).reshape(1, 1)

    in_maps = []
    for c in range(NCORE):
        lo, hi = c * NPC, (c + 1) * NPC

        # dense adjacency stream: [P, N_DENSE * NCHUNK * P]
        if N_DENSE:
            Ac = np.concatenate([A_blocks[(c, nb)] for nb in range(N_DENSE)], axis=1)
        else:
            Ac = np.zeros((P, 1), bfnp)

        # gathered blocks: sel [P, N_GATH*T_g*P] bf16, idx [P, N_GATH*NIB] i16
        # pad unused slots with idx 0 (gathers real row 0, sel weight 0) --
        # idx -1 would skip the DMA and leave garbage (possibly NaN) in the
        # tile, and 0 * NaN = NaN would poison the PSUM accumulation.
        # Node n lives at table row perm(n) = core*1280 + (n%128)*10 +
        # (n//128)%10 under the partition-major AllGather layout.
        selc = np.zeros((N_GATH, T_g, P, P), np.float32)
        idxc = np.zeros((N_GATH, NI), np.int16)
        for j in range(N_GATH):
            nb = N_DENSE + j
            su, du, cnt = uniq_blocks[(c, nb)]
            n = len(su)
            perm = (su // NPC) * NPC + (su % P) * BPC + (su // P) % BPC
            idxc[j, :n] = perm.astype(np.int16)
            selc[j].reshape(T_g * P, P)[np.arange(n), du] = cnt
        selc = np.ascontiguousarray(
            selc.reshape(N_GATH * T_g, P, P).transpose(1, 0, 2)
        ).reshape(P, N_GATH * T_g * P)
        idxw = idxc.reshape(N_GATH, NIB, 16).transpose(0, 2, 1)   # [NG,16,NIB]
        idxw = np.tile(idxw, (1, 8, 1))                           # [NG,128,NIB]
        idxw = np.ascontiguousarray(idxw.transpose(1, 0, 2)).reshape(P, N_GATH * NIB)

        degc = deg[lo:hi].reshape(BPC, P).T
        maskc = (degc > 0).astype(np.float32)
        degc = np.maximum(degc, 1.0).astype(np.float32)

        pselc = psel_full[lo:hi].reshape(BPC, P, G)
        pselc = np.ascontiguousarray(pselc.transpose(1, 0, 2)).reshape(P, BPC * G)

        xTc = np.ascontiguousarray(x_np[lo:hi].T)

        in_maps.append(dict(
            A=Ac, selw=selc.astype(bfnp), idx=idxw.astype(np.int16),
            xT=xTc, deg=degc, mask=maskc, psel=pselc,
            W=W_lhsT.astype(bfnp), gb=gb, emb0=emb0, emb1=emb1,
            w1=w1, w2=w2, w3=w3, b1=b1, b2=b2, b3=b3,
        ))
    return T_g, in_maps


# --------------------------------------------------------------------------
# device program
# --------------------------------------------------------------------------

def _build(T_g):
    NI = T_g * P
    NIB = NI // 16
    nc = bacc.Bacc(None, target_bir_lowering=False)

    d_A = nc.dram_tensor("A", [P, max(N_DENSE * NCHUNK * P, 1)], bf16,
                         kind="ExternalInput")
    d_sel = nc.dram_tensor("selw", [P, N_GATH * T_g * P], bf16, kind="ExternalInput")
    d_idx = nc.dram_tensor("idx", [P, N_GATH * NIB], i16, kind="ExternalInput")
    d_xT = nc.dram_tensor("xT", [NF, NPC], f32, kind="ExternalInput")
    d_deg = nc.dram_tensor("deg", [P, BPC], f32, kind="ExternalInput")
    d_mask = nc.dram_tensor("mask", [P, BPC], f32, kind="ExternalInput")
    d_psel = nc.dram_tensor("psel", [P, BPC * G], f32, kind="ExternalInput")
    d_W = nc.dram_tensor("W", [P, L * 2 * H], bf16, kind="ExternalInput")
    d_gb = nc.dram_tensor("gb", [1, 2 * L * H], f32, kind="ExternalInput")
    d_emb0 = nc.dram_tensor("emb0", [NF, H], f32, kind="ExternalInput")
    d_emb1 = nc.dram_tensor("emb1", [NF, H], f32, kind="ExternalInput")
    d_w1 = nc.dram_tensor("w1", [P, 2 * P], f32, kind="ExternalInput")
    d_w2 = nc.dram_tensor("w2", [P, 64], f32, kind="ExternalInput")
    d_w3 = nc.dram_tensor("w3", [64, 1], f32, kind="ExternalInput")
    d_b1 = nc.dram_tensor("b1", [P, 1], f32, kind="ExternalInput")
    d_b2 = nc.dram_tensor("b2", [64, 1], f32, kind="ExternalInput")
    d_b3 = nc.dram_tensor("b3", [1, 1], f32, kind="ExternalInput")
    d_out = nc.dram_tensor("out", [1, G], f32, kind="ExternalOutput")

    rg = [list(range(NCORE))]

    with tile.TileContext(nc) as tc, ExitStack() as ctx:
        pers = ctx.enter_context(tc.tile_pool(name="pers", bufs=1))
        psA = ctx.enter_context(tc.tile_pool(name="psA", bufs=2, space="PSUM"))
        psB = ctx.enter_context(tc.tile_pool(name="psB", bufs=2, space="PSUM"))
        psC = ctx.enter_context(tc.tile_pool(name="psC", bufs=1, space="PSUM"))
        apool = ctx.enter_context(tc.tile_pool(name="apool", bufs=3))
        gpool = ctx.enter_context(tc.tile_pool(name="gpool", bufs=2))
        work = ctx.enter_context(tc.tile_pool(name="work", bufs=2))
        stream = ctx.enter_context(tc.tile_pool(name="stream", bufs=2))
        dram = ctx.enter_context(tc.tile_pool(name="dram", bufs=2, space="DRAM"))

        # ---- persistent SBUF state -------------------------------------
        sel_sb = pers.tile([P, N_GATH * T_g * P], bf16, tag="sel")
        idx_sb = pers.tile([P, N_GATH * NIB], i16, tag="idx")
        deg_sb = pers.tile([P, BPC], f32, tag="deg")
        mask_sb = pers.tile([P, BPC], f32, tag="mask")
        W_sb = pers.tile([P, L * 2 * H], bf16, tag="W")
        gb_sb = pers.tile([1, 2 * L * H], f32, tag="gb")
        emb0_sb = pers.tile([NF, H], f32, tag="emb0")
        emb1_sb = pers.tile([NF, H], f32, tag="emb1")
        w1_sb = pers.tile([P, 2 * P], f32, tag="w1")
        w2_sb = pers.tile([P, 64], f32, tag="w2")
        w3_sb = pers.tile([64, 1], f32, tag="w3")
        b1_sb = pers.tile([P, 1], f32, tag="b1")
        b2_sb = pers.tile([64, 1], f32, tag="b2")
        b3_sb = pers.tile([1, 1], f32, tag="b3")

        tab_sb = pers.tile([P, NCHUNK * H], bf16, tag="tab")
        h_sb = pers.tile([P, BPC * H], f32, tag="h")
        hsT_sb = pers.tile([P, BPC * 2 * P], bf16, tag="hsT")
        hws_sb = pers.tile([P, BPC * H], bf16, tag="hws")
        t_all = pers.tile([P, BPC * H], f32, tag="t_all")
        nrm_sb = pers.tile([P, BPC], f32, tag="nrm")
        acc_s = pers.tile([P, H], f32, tag="acc_s")
        acc_q = pers.tile([P, H], f32, tag="acc_q")
        D_sb = pers.tile([NF, H], f32, tag="D")
        base_rep = pers.tile([P, H], f32, tag="base_rep")
        a_rep = pers.tile([P, H], f32, tag="a_rep")
        c_rep = pers.tile([P, H], f32, tag="c_rep")
        ident_bf = pers.tile([P, P], bf16, tag="ident")
        ones9 = pers.tile([NF, 1], f32, tag="ones9")
        ones1 = pers.tile([1, P], f32, tag="ones1")
        ones128 = pers.tile([P, 1], f32, tag="ones128")
        stv = pers.tile([1, 2 * H], f32, tag="stv")
        scal = pers.tile([1, 8 * H], f32, tag="scal")

        # ---- DRAM bounce buffers ---------------------------------------
        # AllGather payload is partition-major: ag_in[p, nb*H:(nb+1)*H] =
        # hws[nb*128+p] (hws_sb's natural layout). ag_out row c*128+p then
        # holds core c's 10 chunks for partition p as one 5KB contiguous run,
        # so the SBUF table load is 1024 big descriptors instead of 10240
        # 512B scattered ones. Global chunk g lands at column g*256 either
        # way ((c*10+nb)*256), and the flat buffer still reads as [10240,256]
        # rows under the permutation n -> c*1280 + p*10 + nb.
        ag_in = dram.tile([P, BPC * H], bf16, tag="ag_in")
        ag_out = dram.tile([NCORE * P, BPC * H], bf16, tag="ag_out")
        ar_in = dram.tile([P, 2 * H], f32, tag="ar_in")
        ar_out = dram.tile([P, 2 * H], f32, tag="ar_out")
        pr_in = dram.tile([2 * P + 1, G], f32, tag="pr_in")
        pr_out = dram.tile([2 * P + 1, G], f32, tag="pr_out")

        # ---- input loads ------------------------------------------------
        for t, d in [(sel_sb, d_sel), (idx_sb, d_idx),
                     (deg_sb, d_deg), (mask_sb, d_mask),
                     (W_sb, d_W), (gb_sb, d_gb), (emb0_sb, d_emb0),
                     (emb1_sb, d_emb1), (w1_sb, d_w1), (w2_sb, d_w2),
                     (w3_sb, d_w3), (b1_sb, d_b1), (b2_sb, d_b2),
                     (b3_sb, d_b3)]:
            nc.sync.dma_start(out=t[:], in_=d[:])

        nc.gpsimd.load_library(library_config.mlp)
        make_identity(nc, ident_bf[:])
        nc.vector.memset(ones9[:], 1.0)
        nc.vector.memset(ones1[:], 1.0)
        nc.vector.memset(ones128[:], 1.0)

        # nrm = rsqrt(deg) * mask
        rdeg = work.tile([P, BPC], f32, tag="rdeg")
        nc.vector.reciprocal(out=rdeg[:], in_=deg_sb[:])
        nc.scalar.activation(out=rdeg[:], in_=rdeg[:], func=FT.Sqrt)
        nc.vector.tensor_tensor(out=nrm_sb[:], in0=rdeg[:], in1=mask_sb[:], op=OP.mult)

        # encoder prep: D = emb1 - emb0 ; base = ones9^T @ emb0, broadcast
        nc.vector.tensor_tensor(out=D_sb[:], in0=emb1_sb[:], in1=emb0_sb[:], op=OP.subtract)
        ps_b = psB.tile([1, H], f32, tag="vec")
        nc.tensor.matmul(out=ps_b[:], lhsT=ones9[:], rhs=emb0_sb[:], start=True, stop=True)
        bvec = scal[:, 0:H]
        nc.vector.tensor_copy(out=bvec, in_=ps_b[:])
        ps_br = psB.tile([P, H], f32, tag="vec")
        nc.tensor.matmul(out=ps_br[:], lhsT=ones1[:], rhs=bvec, start=True, stop=True)
        nc.vector.tensor_copy(out=base_rep[:], in_=ps_br[:])

        def hslice(nb):
            return h_sb[:, nb * H:(nb + 1) * H]

        def emit_hs_transpose(nb):
            """hs = h*nrm (bf16), transpose both 128-halves into hsT_sb."""
            hs_bf = work.tile([P, H], bf16, tag="hs_bf")
            nc.vector.tensor_scalar_mul(hs_bf[:], hslice(nb), nrm_sb[:, nb:nb + 1])
            for k in range(2):
                pst = psB.tile([P, P], bf16, tag="pst")
                nc.tensor.transpose(out=pst[:], in_=hs_bf[:, k * P:(k + 1) * P],
                                    identity=ident_bf[:])
                nc.vector.tensor_copy(out=hsT_sb[:, (nb * 2 + k) * P:(nb * 2 + k + 1) * P],
                                      in_=pst[:])

        # encoder: h0 = base + xT^T @ D  (per block)
        for nb in range(BPC):
            xT_t = stream.tile([NF, P], f32, tag="xT_t")
            nc.sync.dma_start(out=xT_t[:], in_=d_xT[:, nb * P:(nb + 1) * P])
            ps_h = psA.tile([P, H], f32, tag="mm")
            nc.tensor.matmul(out=ps_h[:], lhsT=xT_t[:],
                             rhs=D_sb[:], start=True, stop=True)
            nc.vector.tensor_tensor(out=hslice(nb), in0=ps_h[:], in1=base_rep[:], op=OP.add)
            emit_hs_transpose(nb)

        # ---- layers -----------------------------------------------------
        for l in range(L):
            # GEMM hws = hs @ W[l]
            for nb in range(BPC):
                ps_g = psA.tile([P, H], f32, tag="mm")
                for k in range(2):
                    nc.tensor.matmul(
                        out=ps_g[:],
                        lhsT=hsT_sb[:, (nb * 2 + k) * P:(nb * 2 + k + 1) * P],
                        rhs=W_sb[:, (l * 2 + k) * H:(l * 2 + k + 1) * H],
                        start=(k == 0), stop=(k == 1))
                nc.vector.tensor_copy(out=hws_sb[:, nb * H:(nb + 1) * H], in_=ps_g[:])
            nc.sync.dma_start(out=ag_in[:], in_=hws_sb[:])

            # prefetch the first dense block's A halves BEFORE the collective
            # so the loads run under the AllGather (they have no data deps)
            HC = NCHUNK // 2
            a_fifo = []

            def a_prefetch(nh):
                a_t = apool.tile([P, HC * P], bf16, tag="A")
                nc.sync.dma_start(out=a_t[:], in_=d_A[:, nh * HC * P:(nh + 1) * HC * P])
                a_fifo.append(a_t)

            n_halves = 2 * N_DENSE
            for nh in range(min(2, n_halves)):
                a_prefetch(nh)

            nc.gpsimd.collective_compute(
                "AllGather", OP.bypass, replica_groups=rg,
                ins=[ag_in[:]], outs=[ag_out[:]])

            # bulk-load the full table into SBUF: partition p gets core c's
            # 5KB run at columns [c*BPC*H, (c+1)*BPC*H) -- 1024 descriptors.
            nc.sync.dma_start(
                out=tab_sb[:].rearrange("p (c w) -> p c w", c=NCORE),
                in_=ag_out[:].rearrange("(c p) w -> p c w", p=P))

            nc.vector.memset(acc_s[:], 0.0)
            nc.vector.memset(acc_q[:], 0.0)

            def post_block(nb, ps_t):
                """t = nrm*psum; accumulate BN stats."""
                tsl = t_all[:, nb * H:(nb + 1) * H]
                nc.scalar.activation(out=tsl, in_=ps_t[:], func=FT.Copy,
                                     scale=nrm_sb[:, nb:nb + 1])
                sq = work.tile([P, H], f32, tag="tmp")
                nc.vector.tensor_tensor(out=sq[:], in0=tsl, in1=tsl, op=OP.mult)
                nc.vector.tensor_tensor(out=acc_s[:], in0=acc_s[:], in1=tsl, op=OP.add)
                nc.vector.tensor_tensor(out=acc_q[:], in0=acc_q[:], in1=sq[:], op=OP.add)

            # gathered blocks: issue SWDGE gathers up front (Pool engine runs
            # them while TensorE does the dense blocks); sel chains are
            # interleaved between dense chains so gpool bufs recycle promptly.
            T0 = (T_g + 1) // 2
            chunks = [(0, T0), (T0, T_g)] if T_g > 1 else [(0, T_g)]

            def emit_gather(j):
                gts = []
                for (j0, j1) in chunks:
                    gath = gpool.tile([P, T0 * H], bf16, tag="gath")
                    nc.gpsimd.dma_gather(
                        out_ap=gath[:, :(j1 - j0) * H].rearrange("p (t h) -> p t h", h=H),
                        in_ap=ag_out[:].rearrange("c (n h) -> (c n) h", h=H),
                        idxs_ap=idx_sb[:, j * NIB + j0 * 8:j * NIB + j1 * 8],
                        num_idxs=(j1 - j0) * P, num_idxs_reg=(j1 - j0) * P,
                        elem_size=H, single_packet=False)
                    gts.append(gath)
                return gts

            def emit_sel_chain(j, gts):
                nb = N_DENSE + j
                ps_t = psA.tile([P, H], f32, tag="mm")
                for t in range(T_g):
                    ci = 0 if t < T0 else 1
                    tt = t if t < T0 else t - T0
                    nc.tensor.matmul(
                        out=ps_t[:],
                        lhsT=sel_sb[:, (j * T_g + t) * P:(j * T_g + t + 1) * P],
                        rhs=gts[ci][:, tt * H:(tt + 1) * H],
                        start=(t == 0), stop=(t == T_g - 1))
                post_block(nb, ps_t)

            def emit_dense_chain(nb):
                # A streamed in half-blocks of 40 chunks, software-pipelined:
                # the load for half h+2 is issued before consuming half h.
                ps_t = psA.tile([P, H], f32, tag="mm")
                for half in range(2):
                    nh = nb * 2 + half
                    if nh + 2 < n_halves:
                        a_prefetch(nh + 2)
                    a_t = a_fifo.pop(0)
                    for cc in range(HC):
                        c = half * HC + cc
                        nc.tensor.matmul(
                            out=ps_t[:],
                            lhsT=a_t[:, cc * P:(cc + 1) * P],
                            rhs=tab_sb[:, c * H:(c + 1) * H],
                            start=(c == 0), stop=(c == NCHUNK - 1))
                post_block(nb, ps_t)

            # interleave: gathers 0,1 in flight; after each dense chain,
            # drain one ready gather with its sel chain and issue the next.
            pending = []
            next_g = 0
            if N_GATH and next_g < N_GATH:
                pending.append((next_g, emit_gather(next_g))); next_g += 1
            for nb in range(N_DENSE):
                emit_dense_chain(nb)
                if next_g < N_GATH:
                    pending.append((next_g, emit_gather(next_g))); next_g += 1
                if pending and (len(pending) >= 2 or nb >= N_DENSE - 1):
                    j, gts = pending.pop(0)
                    emit_sel_chain(j, gts)
            while next_g < N_GATH:
                pending.append((next_g, emit_gather(next_g))); next_g += 1
            for j, gts in pending:
                emit_sel_chain(j, gts)

            # stats: cross-partition reduce, replicate to 128 rows, AllReduce
            ps_s = psB.tile([1, 2 * H], f32, tag="vec")
            nc.tensor.matmul(out=ps_s[:, 0:H], lhsT=ones128[:], rhs=acc_s[:],
                             start=True, stop=True)
            nc.tensor.matmul(out=ps_s[:, H:2 * H], lhsT=ones128[:], rhs=acc_q[:],
                             start=True, stop=True)
            st_sb = scal[:, 6 * H:8 * H]
            nc.vector.tensor_copy(out=st_sb, in_=ps_s[:])
            # replicate to [128, 2H] so the AllReduce picks the fast RDH path
            st_rep = work.tile([P, 2 * H], f32, tag="strep")
            for half in range(2):
                ps_r2 = psB.tile([P, H], f32, tag="vec")
                nc.tensor.matmul(out=ps_r2[:], lhsT=ones1[:],
                                 rhs=st_sb[:, half * H:(half + 1) * H],
                                 start=True, stop=True)
                nc.vector.tensor_copy(out=st_rep[:, half * H:(half + 1) * H],
                                      in_=ps_r2[:])
            nc.sync.dma_start(out=ar_in[:], in_=st_rep[:])
            nc.gpsimd.collective_compute(
                "AllReduce", OP.add, replica_groups=rg,
                ins=[ar_in[:]], outs=[ar_out[:]])
            nc.sync.dma_start(out=stv[:], in_=ar_out[0:1, :])

            # a = gamma*istd ; c = beta - mu*a
            mu = scal[:, H:2 * H]
            var = scal[:, 2 * H:3 * H]
            av = scal[:, 3 * H:4 * H]
            cv = scal[:, 4 * H:5 * H]
            msq = scal[:, 5 * H:6 * H]
            nc.vector.tensor_scalar_mul(mu, stv[:, 0:H], 1.0 / N)
            nc.vector.tensor_scalar_mul(var, stv[:, H:2 * H], 1.0 / N)
            nc.vector.tensor_tensor(out=msq, in0=mu, in1=mu, op=OP.mult)
            nc.vector.tensor_tensor(out=var, in0=var, in1=msq, op=OP.subtract)
            nc.vector.tensor_scalar_add(var, var, BN_EPS)
            nc.vector.reciprocal(out=var, in_=var)
            nc.scalar.activation(out=var, in_=var, func=FT.Sqrt)  # istd
            nc.vector.tensor_tensor(out=av, in0=var,
                                    in1=gb_sb[:, l * H:(l + 1) * H], op=OP.mult)
            nc.vector.tensor_tensor(out=msq, in0=mu, in1=av, op=OP.mult)
            nc.vector.tensor_tensor(out=cv, in0=gb_sb[:, (L + l) * H:(L + l + 1) * H],
                                    in1=msq, op=OP.subtract)
            ps_a = psB.tile([P, H], f32, tag="vec")
            nc.tensor.matmul(out=ps_a[:], lhsT=ones1[:], rhs=av, start=True, stop=True)
            nc.vector.tensor_copy(out=a_rep[:], in_=ps_a[:])
            ps_c = psB.tile([P, H], f32, tag="vec")
            nc.tensor.matmul(out=ps_c[:], lhsT=ones1[:], rhs=cv, start=True, stop=True)
            nc.vector.tensor_copy(out=c_rep[:], in_=ps_c[:])

            # h = relu(t*a + c) + h ; prepare hsT for next layer
            for nb in range(BPC):
                tsl = t_all[:, nb * H:(nb + 1) * H]
                u = work.tile([P, H], f32, tag="tmp")
                nc.vector.tensor_tensor(out=u[:], in0=tsl, in1=a_rep[:], op=OP.mult)
                nc.vector.tensor_tensor(out=u[:], in0=u[:], in1=c_rep[:], op=OP.add)
                r = work.tile([P, H], f32, tag="tmp2")
                nc.scalar.activation(out=r[:], in_=u[:], func=FT.Relu)
                nc.vector.tensor_tensor(out=hslice(nb), in0=hslice(nb), in1=r[:], op=OP.add)
                if l < L - 1:
                    emit_hs_transpose(nb)

        # ---- pooling ----------------------------------------------------
        ps_p0 = psC.tile([P, G], f32, tag="p0")
        ps_p1 = psC.tile([P, G], f32, tag="p1")
        ps_pc = psB.tile([1, G], f32, tag="vec")
        for nb in range(BPC):
            psel_t = stream.tile([P, G], f32, tag="psel_t")
            nc.sync.dma_start(out=psel_t[:], in_=d_psel[:, nb * G:(nb + 1) * G])
            pssl = psel_t[:]
            nc.tensor.matmul(out=ps_p0[:], lhsT=h_sb[:, nb * H:nb * H + P],
                             rhs=pssl, start=(nb == 0), stop=(nb == BPC - 1))
            nc.tensor.matmul(out=ps_p1[:], lhsT=h_sb[:, nb * H + P:(nb + 1) * H],
                             rhs=pssl, start=(nb == 0), stop=(nb == BPC - 1))
            nc.tensor.matmul(out=ps_pc[:], lhsT=ones128[:],
                             rhs=pssl, start=(nb == 0), stop=(nb == BPC - 1))
        g0 = work.tile([P, G], f32, tag="g0")
        g1 = work.tile([P, G], f32, tag="g1")
        cnt = scal[:, 0:G]
        nc.vector.tensor_copy(out=g0[:], in_=ps_p0[:])
        nc.vector.tensor_copy(out=g1[:], in_=ps_p1[:])
        nc.vector.tensor_copy(out=cnt, in_=ps_pc[:])
        nc.sync.dma_start(out=pr_in[0:P, :], in_=g0[:])
        nc.sync.dma_start(out=pr_in[P:2 * P, :], in_=g1[:])
        nc.sync.dma_start(out=pr_in[2 * P:2 * P + 1, :], in_=cnt)
        nc.gpsimd.collective_compute(
            "AllReduce", OP.add, replica_groups=rg,
            ins=[pr_in[:]], outs=[pr_out[:]])
        nc.sync.dma_start(out=g0[:], in_=pr_out[0:P, :])
        nc.sync.dma_start(out=g1[:], in_=pr_out[P:2 * P, :])
        nc.sync.dma_start(out=cnt, in_=pr_out[2 * P:2 * P + 1, :])
        nc.vector.tensor_scalar_max(cnt, cnt, 1.0)
        nc.vector.reciprocal(out=cnt, in_=cnt)
        ps_r = psB.tile([P, G], f32, tag="vec")
        nc.tensor.matmul(out=ps_r[:], lhsT=ones1[:], rhs=cnt, start=True, stop=True)
        rc_rep = work.tile([P, G], f32, tag="rc_rep")
        nc.vector.tensor_copy(out=rc_rep[:], in_=ps_r[:])
        nc.vector.tensor_tensor(out=g0[:], in0=g0[:], in1=rc_rep[:], op=OP.mult)
        nc.vector.tensor_tensor(out=g1[:], in0=g1[:], in1=rc_rep[:], op=OP.mult)

        # MLP head (transposed: weights are lhsT, graphs along free dim)
        ps1 = psB.tile([P, G], f32, tag="vec")
        nc.tensor.matmul(out=ps1[:], lhsT=w1_sb[:, 0:P], rhs=g0[:], start=True, stop=False)
        nc.tensor.matmul(out=ps1[:], lhsT=w1_sb[:, P:2 * P], rhs=g1[:], start=False, stop=True)
        y1 = work.tile([P, G], f32, tag="y1")
        nc.scalar.activation(out=y1[:], in_=ps1[:], func=FT.Relu, bias=b1_sb[:, 0:1])
        ps2 = psB.tile([64, G], f32, tag="vec")
        nc.tensor.matmul(out=ps2[:], lhsT=w2_sb[:], rhs=y1[:], start=True, stop=True)
        y2 = work.tile([64, G], f32, tag="y2")
        nc.scalar.activation(out=y2[:], in_=ps2[:], func=FT.Relu, bias=b2_sb[:, 0:1])
        ps3 = psB.tile([1, G], f32, tag="vec")
        nc.tensor.matmul(out=ps3[:], lhsT=w3_sb[:], rhs=y2[:], start=True, stop=True)
        y3 = work.tile([1, G], f32, tag="y3")
        nc.vector.tensor_scalar_add(y3[:], ps3[:], b3_sb[0:1, 0:1])
        nc.sync.dma_start(out=d_out[:], in_=y3[:])

    nc.compile()
    return nc


# --------------------------------------------------------------------------
# entry point
# --------------------------------------------------------------------------

def kernel(x, edge_index, batch_ids, emb, W, b, gamma, beta,
           mlp_W1, mlp_b1, mlp_W2, mlp_b2, mlp_W3, mlp_b3,
           _trace=False, _trace_kwargs=None):
    # NB: reference BN subtracts the per-channel mean, so the additive bias b
    # cancels exactly and is not needed by the device program.
    T_g, in_maps = _preprocess(x, edge_index, batch_ids, emb, W, gamma, beta,
                               mlp_W1, mlp_b1, mlp_W2, mlp_b2, mlp_W3, mlp_b3)
    key = (T_g, N_DENSE)
    if key not in _compiled:
        _compiled[key] = _build(T_g)
    nc = _compiled[key]
    kw = {}
    if _trace:
        kw = dict(trace=True, **(_trace_kwargs or {}))
    res = run_bass_kernel_spmd(nc, in_maps, core_ids=list(range(NCORE)), **kw)
    out = np.asarray(res.results[0]["out"], np.float32).reshape(G, 1)
    kernel._last_results = res
    return out


# revision 19
# speedup vs baseline: 2.3245x; 1.5026x over previous
"""Trainium2 Bass kernel for HIVNet GCN message passing (8-core SPMD).

v2 strategy (vs v1 baseline at 2.29ms):
  - Pad N=10000 nodes to 10240 = 80 blocks x 128; core c owns 10 dst-blocks.
  - Per layer: hws = (h*nrm) @ W[l] on the owned shard (bf16), AllGather into
    a DRAM table on every core, bulk-load the full table into SBUF.
  - Aggregation is HYBRID per dst-block:
      * N_DENSE blocks: dense one-hot adjacency matmuls on TensorE
        (80 src-chunk matmuls accumulated in PSUM, A streamed from HBM,
        prefetched under the AllGather) -- zero GPSIMD cost.
      * remaining blocks: SWDGE dma_gather of DEDUPED (src,dst-block) rows
        (multiplicity folded into the sel weights) + one-hot sel matmuls.
    Split chosen to balance Pool-engine (8.2ns/row) vs TensorE (213ns/matmul).
  - BatchNorm: partial sums/sumsq -> stats replicated to 128 rows so the
    2KB AllReduce becomes a 256KB RDH AllReduce (79us -> ~16us); scale/shift
    broadcast via rank-1 TensorE matmul; relu + residual on DVE.
  - Readout: graph mean-pool via one-hot pool matrices, 257-row AllReduce,
    3-layer MLP computed redundantly on every core.
"""

import os
import sys

sys.path.insert(0, "/opt/trn_rl_repo")

from contextlib import ExitStack

import numpy as np
import ml_dtypes

from concourse import bass, mybir, bacc, tile, library_config
from concourse.bass_utils import run_bass_kernel_spmd
from concourse.masks import make_identity

NCORE = 8
P = 128
H = 256
L = 4
NF = 9
G = 256
N = 10000
BPC = 10                # dst blocks per core
NPC = BPC * P           # 1280 nodes per core
NPAD = NCORE * NPC      # 10240
NCHUNK = NPAD // P      # 80 src chunks
BN_EPS = 1e-5

N_DENSE = int(os.environ.get("KDENSE", "7"))   # dense-adjacency blocks per core
N_GATH = BPC - N_DENSE

f32 = mybir.dt.float32
bf16 = mybir.dt.bfloat16
i16 = mybir.dt.int16
bfnp = ml_dtypes.bfloat16

FT = mybir.ActivationFunctionType
OP = mybir.AluOpType

_compiled = {}


# --------------------------------------------------------------------------
# host-side structural preprocessing
# --------------------------------------------------------------------------

def _preprocess(x, edge_index, batch_ids, emb, W, gamma, beta,
                mlp_W1, mlp_b1, mlp_W2, mlp_b2, mlp_W3, mlp_b3):
    src = np.asarray(edge_index[0], np.int64)
    dst = np.asarray(edge_index[1], np.int64)
    # self loops for every real node (weight nrm[d]^2 folds in)
    src_all = np.concatenate([src, np.arange(N, dtype=np.int64)])
    dst_all = np.concatenate([dst, np.arange(N, dtype=np.int64)])
    order = np.argsort(dst_all, kind="stable")
    s_sorted = src_all[order]
    d_sorted = dst_all[order]

    deg = np.bincount(dst_all, minlength=NPAD).astype(np.float64)  # incl self

    nblk = NCORE * BPC
    starts = np.searchsorted(d_sorted, np.arange(nblk) * P)
    ends = np.searchsorted(d_sorted, (np.arange(nblk) + 1) * P)

    # ---- per-block edge structures ------------------------------------
    # dense blocks (local idx < N_DENSE): A counts [NPAD, P]
    # gathered blocks: deduped (src, dst_local) with multiplicity
    A_blocks = {}          # (c, nb) -> [P, NCHUNK*P] bf16
    uniq_blocks = {}       # (c, nb) -> (srcs_u, dloc_u, cnt)
    T_g = 1
    for g in range(nblk):
        c, nb = divmod(g, BPC)
        e_s = s_sorted[starts[g]:ends[g]]
        e_d = d_sorted[starts[g]:ends[g]] - g * P
        if nb < N_DENSE:
            A = np.zeros((NPAD, P), np.float32)
            np.add.at(A, (e_s, e_d), 1.0)
            A_blocks[(c, nb)] = np.ascontiguousarray(
                A.reshape(NCHUNK, P, P).transpose(1, 0, 2).reshape(P, NCHUNK * P)
            ).astype(bfnp)
        else:
            key = e_s * P + e_d
            uk, cnt = np.unique(key, return_counts=True)
            uniq_blocks[(c, nb)] = (uk // P, uk % P, cnt)
            T_g = max(T_g, (len(uk) + P - 1) // P)

    NI = T_g * P
    NIB = NI // 16

    # graph pool one-hot [node, graph]
    bids = np.asarray(batch_ids, np.int64)
    psel_full = np.zeros((NPAD, G), np.float32)
    psel_full[np.arange(N), bids] = 1.0

    x_np = np.zeros((NPAD, NF), np.float32)
    x_np[:N] = np.asarray(x, np.float64)

    # shared parameter tensors (layout for device)
    Wf = np.asarray(W, np.float32)                       # [L,H,H]
    W_lhsT = Wf.reshape(L, 2, P, H).transpose(2, 0, 1, 3).reshape(P, L * 2 * H)
    gb = np.concatenate([np.asarray(gamma, np.float32).reshape(-1),
                         np.asarray(beta, np.float32).reshape(-1)])[None, :]
    embf = np.asarray(emb, np.float32)
    emb0 = np.ascontiguousarray(embf[:, 0, :])
    emb1 = np.ascontiguousarray(embf[:, 1, :])
    w1 = np.asarray(mlp_W1, np.float32).reshape(2, P, P).transpose(1, 0, 2).reshape(P, 2 * P)
    w2 = np.asarray(mlp_W2, np.float32)
    w3 = np.asarray(mlp_W3, np.float32)
    b1 = np.asarray(mlp_b1, np.float32).reshape(P, 1)
    b2 = np.asarray(mlp_b2, np.float32).reshape(64, 1)
    b3 = np.asarray(mlp_b3, np.float32).reshape(1, 1)

    in_maps = []
    for c in range(NCORE):
        lo, hi = c * NPC, (c + 1) * NPC

        # dense adjacency stream: [P, N_DENSE * NCHUNK * P]
        if N_DENSE:
            Ac = np.concatenate([A_blocks[(c, nb)] for nb in range(N_DENSE)], axis=1)
        else:
            Ac = np.zeros((P, 1), bfnp)
        if N_GATH == 0:
            in_maps.append(dict(
                A=Ac, selw=np.zeros((P, 1), bfnp), idx=np.zeros((P, 8), np.int16),
                xT=np.ascontiguousarray(x_np[lo:hi].T),
                deg=np.maximum(deg[lo:hi].reshape(BPC, P).T, 1.0).astype(np.float32),
                mask=(deg[lo:hi].reshape(BPC, P).T > 0).astype(np.float32),
                psel=np.ascontiguousarray(
                    psel_full[lo:hi].reshape(BPC, P, G).transpose(1, 0, 2)
                ).reshape(P, BPC * G),
                W=W_lhsT.astype(bfnp), gb=gb, emb0=emb0, emb1=emb1,
                w1=w1, w2=w2, w3=w3, b1=b1, b2=b2, b3=b3,
            ))
            continue

        # gathered blocks: sel [P, N_GATH*T_g*P] bf16, idx [P, N_GATH*NIB] i16
        # pad unused slots with idx 0 (gathers real row 0, sel weight 0) --
        # idx -1 would skip the DMA and leave garbage (possibly NaN) in the
        # tile, and 0 * NaN = NaN would poison the PSUM accumulation.
        # Node n lives at table row perm(n) = core*1280 + (n%128)*10 +
        # (n//128)%10 under the partition-major AllGather layout.
        selc = np.zeros((N_GATH, T_g, P, P), np.float32)
        idxc = np.zeros((N_GATH, NI), np.int16)
        for j in range(N_GATH):
            nb = N_DENSE + j
            su, du, cnt = uniq_blocks[(c, nb)]
            n = len(su)
            perm = (su // NPC) * NPC + (su % P) * BPC + (su // P) % BPC
            idxc[j, :n] = perm.astype(np.int16)
            selc[j].reshape(T_g * P, P)[np.arange(n), du] = cnt
        selc = np.ascontiguousarray(
            selc.reshape(N_GATH * T_g, P, P).transpose(1, 0, 2)
        ).reshape(P, N_GATH * T_g * P)
        idxw = idxc.reshape(N_GATH, NIB, 16).transpose(0, 2, 1)   # [NG,16,NIB]
        idxw = np.tile(idxw, (1, 8, 1))                           # [NG,128,NIB]
        idxw = np.ascontiguousarray(idxw.transpose(1, 0, 2)).reshape(P, N_GATH * NIB)

        degc = deg[lo:hi].reshape(BPC, P).T
        maskc = (degc > 0).astype(np.float32)
        degc = np.maximum(degc, 1.0).astype(np.float32)

        pselc = psel_full[lo:hi].reshape(BPC, P, G)
        pselc = np.ascontiguousarray(pselc.transpose(1, 0, 2)).reshape(P, BPC * G)

        xTc = np.ascontiguousarray(x_np[lo:hi].T)

        in_maps.append(dict(
            A=Ac, selw=selc.astype(bfnp), idx=idxw.astype(np.int16),
            xT=xTc, deg=degc, mask=maskc, psel=pselc,
            W=W_lhsT.astype(bfnp), gb=gb, emb0=emb0, emb1=emb1,
            w1=w1, w2=w2, w3=w3, b1=b1, b2=b2, b3=b3,
        ))
    return T_g, in_maps


# --------------------------------------------------------------------------
# device program
# --------------------------------------------------------------------------

def _build(T_g):
    NI = T_g * P
    NIB = NI // 16
    nc = bacc.Bacc(None, target_bir_lowering=False)

    d_A = nc.dram_tensor("A", [P, max(N_DENSE * NCHUNK * P, 1)], bf16,
                         kind="ExternalInput")
    d_sel = nc.dram_tensor("selw", [P, max(N_GATH * T_g * P, 1)], bf16,
                           kind="ExternalInput")
    d_idx = nc.dram_tensor("idx", [P, max(N_GATH * NIB, 8)], i16,
                           kind="ExternalInput")
    d_xT = nc.dram_tensor("xT", [NF, NPC], f32, kind="ExternalInput")
    d_deg = nc.dram_tensor("deg", [P, BPC], f32, kind="ExternalInput")
    d_mask = nc.dram_tensor("mask", [P, BPC], f32, kind="ExternalInput")
    d_psel = nc.dram_tensor("psel", [P, BPC * G], f32, kind="ExternalInput")
    d_W = nc.dram_tensor("W", [P, L * 2 * H], bf16, kind="ExternalInput")
    d_gb = nc.dram_tensor("gb", [1, 2 * L * H], f32, kind="ExternalInput")
    d_emb0 = nc.dram_tensor("emb0", [NF, H], f32, kind="ExternalInput")
    d_emb1 = nc.dram_tensor("emb1", [NF, H], f32, kind="ExternalInput")
    d_w1 = nc.dram_tensor("w1", [P, 2 * P], f32, kind="ExternalInput")
    d_w2 = nc.dram_tensor("w2", [P, 64], f32, kind="ExternalInput")
    d_w3 = nc.dram_tensor("w3", [64, 1], f32, kind="ExternalInput")
    d_b1 = nc.dram_tensor("b1", [P, 1], f32, kind="ExternalInput")
    d_b2 = nc.dram_tensor("b2", [64, 1], f32, kind="ExternalInput")
    d_b3 = nc.dram_tensor("b3", [1, 1], f32, kind="ExternalInput")
    d_out = nc.dram_tensor("out", [1, G], f32, kind="ExternalOutput")

    rg = [list(range(NCORE))]

    with tile.TileContext(nc) as tc, ExitStack() as ctx:
        pers = ctx.enter_context(tc.tile_pool(name="pers", bufs=1))
        psA = ctx.enter_context(tc.tile_pool(name="psA", bufs=2, space="PSUM"))
        psB = ctx.enter_context(tc.tile_pool(name="psB", bufs=2, space="PSUM"))
        psC = ctx.enter_context(tc.tile_pool(name="psC", bufs=1, space="PSUM"))
        apool = ctx.enter_context(tc.tile_pool(name="apool", bufs=3))
        gpool = ctx.enter_context(tc.tile_pool(name="gpool", bufs=2))
        work = ctx.enter_context(tc.tile_pool(name="work", bufs=2))
        stream = ctx.enter_context(tc.tile_pool(name="stream", bufs=2))
        dram = ctx.enter_context(tc.tile_pool(name="dram", bufs=2, space="DRAM"))

        # ---- persistent SBUF state -------------------------------------
        sel_sb = pers.tile([P, max(N_GATH * T_g * P, 1)], bf16, tag="sel")
        idx_sb = pers.tile([P, max(N_GATH * NIB, 8)], i16, tag="idx")
        deg_sb = pers.tile([P, BPC], f32, tag="deg")
        mask_sb = pers.tile([P, BPC], f32, tag="mask")
        W_sb = pers.tile([P, L * 2 * H], bf16, tag="W")
        gb_sb = pers.tile([1, 2 * L * H], f32, tag="gb")
        emb0_sb = pers.tile([NF, H], f32, tag="emb0")
        emb1_sb = pers.tile([NF, H], f32, tag="emb1")
        w1_sb = pers.tile([P, 2 * P], f32, tag="w1")
        w2_sb = pers.tile([P, 64], f32, tag="w2")
        w3_sb = pers.tile([64, 1], f32, tag="w3")
        b1_sb = pers.tile([P, 1], f32, tag="b1")
        b2_sb = pers.tile([64, 1], f32, tag="b2")
        b3_sb = pers.tile([1, 1], f32, tag="b3")

        tab_sb = pers.tile([P, NCHUNK * H], bf16, tag="tab")
        h_sb = pers.tile([P, BPC * H], f32, tag="h")
        hsT_sb = pers.tile([P, BPC * 2 * P], bf16, tag="hsT")
        hws_sb = pers.tile([P, BPC * H], bf16, tag="hws")
        t_all = pers.tile([P, BPC * H], f32, tag="t_all")
        nrm_sb = pers.tile([P, BPC], f32, tag="nrm")
        acc_s = pers.tile([P, H], f32, tag="acc_s")
        acc_q = pers.tile([P, H], f32, tag="acc_q")
        D_sb = pers.tile([NF, H], f32, tag="D")
        base_rep = pers.tile([P, H], f32, tag="base_rep")
        a_rep = pers.tile([P, H], f32, tag="a_rep")
        c_rep = pers.tile([P, H], f32, tag="c_rep")
        ident_bf = pers.tile([P, P], bf16, tag="ident")
        ones9 = pers.tile([NF, 1], f32, tag="ones9")
        ones1 = pers.tile([1, P], f32, tag="ones1")
        ones128 = pers.tile([P, 1], f32, tag="ones128")
        stv = pers.tile([1, 2 * H], f32, tag="stv")
        scal = pers.tile([1, 8 * H], f32, tag="scal")

        # ---- DRAM bounce buffers ---------------------------------------
        # AllGather payload is partition-major: ag_in[p, nb*H:(nb+1)*H] =
        # hws[nb*128+p] (hws_sb's natural layout). ag_out row c*128+p then
        # holds core c's 10 chunks for partition p as one 5KB contiguous run,
        # so the SBUF table load is 1024 big descriptors instead of 10240
        # 512B scattered ones. Global chunk g lands at column g*256 either
        # way ((c*10+nb)*256), and the flat buffer still reads as [10240,256]
        # rows under the permutation n -> c*1280 + p*10 + nb.
        ag_in = dram.tile([P, BPC * H], bf16, tag="ag_in")
        ag_out = dram.tile([NCORE * P, BPC * H], bf16, tag="ag_out")
        ar_in = dram.tile([P, 2 * H], f32, tag="ar_in")
        ar_out = dram.tile([P, 2 * H], f32, tag="ar_out")
        pr_in = dram.tile([2 * P + 1, G], f32, tag="pr_in")
        pr_out = dram.tile([2 * P + 1, G], f32, tag="pr_out")

        # ---- input loads ------------------------------------------------
        for t, d in [(sel_sb, d_sel), (idx_sb, d_idx),
                     (deg_sb, d_deg), (mask_sb, d_mask),
                     (W_sb, d_W), (gb_sb, d_gb), (emb0_sb, d_emb0),
                     (emb1_sb, d_emb1), (w1_sb, d_w1), (w2_sb, d_w2),
                     (w3_sb, d_w3), (b1_sb, d_b1), (b2_sb, d_b2),
                     (b3_sb, d_b3)]:
            nc.sync.dma_start(out=t[:], in_=d[:])

        nc.gpsimd.load_library(library_config.mlp)
        make_identity(nc, ident_bf[:])
        nc.vector.memset(ones9[:], 1.0)
        nc.vector.memset(ones1[:], 1.0)
        nc.vector.memset(ones128[:], 1.0)

        # nrm = rsqrt(deg) * mask
        rdeg = work.tile([P, BPC], f32, tag="rdeg")
        nc.vector.reciprocal(out=rdeg[:], in_=deg_sb[:])
        nc.scalar.activation(out=rdeg[:], in_=rdeg[:], func=FT.Sqrt)
        nc.vector.tensor_tensor(out=nrm_sb[:], in0=rdeg[:], in1=mask_sb[:], op=OP.mult)

        # encoder prep: D = emb1 - emb0 ; base = ones9^T @ emb0, broadcast
        nc.vector.tensor_tensor(out=D_sb[:], in0=emb1_sb[:], in1=emb0_sb[:], op=OP.subtract)
        ps_b = psB.tile([1, H], f32, tag="vec")
        nc.tensor.matmul(out=ps_b[:], lhsT=ones9[:], rhs=emb0_sb[:], start=True, stop=True)
        bvec = scal[:, 0:H]
        nc.vector.tensor_copy(out=bvec, in_=ps_b[:])
        ps_br = psB.tile([P, H], f32, tag="vec")
        nc.tensor.matmul(out=ps_br[:], lhsT=ones1[:], rhs=bvec, start=True, stop=True)
        nc.vector.tensor_copy(out=base_rep[:], in_=ps_br[:])

        def hslice(nb):
            return h_sb[:, nb * H:(nb + 1) * H]

        def emit_hs_transpose(nb):
            """hs = h*nrm (bf16), transpose both 128-halves into hsT_sb."""
            hs_bf = work.tile([P, H], bf16, tag="hs_bf")
            nc.vector.tensor_scalar_mul(hs_bf[:], hslice(nb), nrm_sb[:, nb:nb + 1])
            for k in range(2):
                pst = psB.tile([P, P], bf16, tag="pst")
                nc.tensor.transpose(out=pst[:], in_=hs_bf[:, k * P:(k + 1) * P],
                                    identity=ident_bf[:])
                nc.vector.tensor_copy(out=hsT_sb[:, (nb * 2 + k) * P:(nb * 2 + k + 1) * P],
                                      in_=pst[:])

        # encoder: h0 = base + xT^T @ D  (per block)
        for nb in range(BPC):
            xT_t = stream.tile([NF, P], f32, tag="xT_t")
            nc.sync.dma_start(out=xT_t[:], in_=d_xT[:, nb * P:(nb + 1) * P])
            ps_h = psA.tile([P, H], f32, tag="mm")
            nc.tensor.matmul(out=ps_h[:], lhsT=xT_t[:],
                             rhs=D_sb[:], start=True, stop=True)
            nc.vector.tensor_tensor(out=hslice(nb), in0=ps_h[:], in1=base_rep[:], op=OP.add)
            emit_hs_transpose(nb)

        # ---- layers -----------------------------------------------------
        for l in range(L):
            # GEMM hws = hs @ W[l]
            for nb in range(BPC):
                ps_g = psA.tile([P, H], f32, tag="mm")
                for k in range(2):
                    nc.tensor.matmul(
                        out=ps_g[:],
                        lhsT=hsT_sb[:, (nb * 2 + k) * P:(nb * 2 + k + 1) * P],
                        rhs=W_sb[:, (l * 2 + k) * H:(l * 2 + k + 1) * H],
                        start=(k == 0), stop=(k == 1))
                nc.vector.tensor_copy(out=hws_sb[:, nb * H:(nb + 1) * H], in_=ps_g[:])
            nc.sync.dma_start(out=ag_in[:], in_=hws_sb[:])

            # prefetch the first dense block's A halves BEFORE the collective
            # so the loads run under the AllGather (they have no data deps)
            HC = NCHUNK // 2
            a_fifo = []

            def a_prefetch(nh):
                a_t = apool.tile([P, HC * P], bf16, tag="A")
                nc.sync.dma_start(out=a_t[:], in_=d_A[:, nh * HC * P:(nh + 1) * HC * P])
                a_fifo.append(a_t)

            n_halves = 2 * N_DENSE
            for nh in range(min(2, n_halves)):
                a_prefetch(nh)

            nc.gpsimd.collective_compute(
                "AllGather", OP.bypass, replica_groups=rg,
                ins=[ag_in[:]], outs=[ag_out[:]])

            # bulk-load the full table into SBUF: partition p gets core c's
            # 5KB run at columns [c*BPC*H, (c+1)*BPC*H) -- 1024 descriptors.
            nc.sync.dma_start(
                out=tab_sb[:].rearrange("p (c w) -> p c w", c=NCORE),
                in_=ag_out[:].rearrange("(c p) w -> p c w", p=P))

            nc.vector.memset(acc_s[:], 0.0)
            nc.vector.memset(acc_q[:], 0.0)

            def post_block(nb, ps_t):
                """t = nrm*psum; accumulate BN stats."""
                tsl = t_all[:, nb * H:(nb + 1) * H]
                nc.scalar.activation(out=tsl, in_=ps_t[:], func=FT.Copy,
                                     scale=nrm_sb[:, nb:nb + 1])
                sq = work.tile([P, H], f32, tag="tmp")
                nc.vector.tensor_tensor(out=sq[:], in0=tsl, in1=tsl, op=OP.mult)
                nc.vector.tensor_tensor(out=acc_s[:], in0=acc_s[:], in1=tsl, op=OP.add)
                nc.vector.tensor_tensor(out=acc_q[:], in0=acc_q[:], in1=sq[:], op=OP.add)

            # gathered blocks: issue SWDGE gathers up front (Pool engine runs
            # them while TensorE does the dense blocks); sel chains are
            # interleaved between dense chains so gpool bufs recycle promptly.
            T0 = (T_g + 1) // 2
            chunks = [(0, T0), (T0, T_g)] if T_g > 1 else [(0, T_g)]

            def emit_gather(j):
                gts = []
                for (j0, j1) in chunks:
                    gath = gpool.tile([P, T0 * H], bf16, tag="gath")
                    nc.gpsimd.dma_gather(
                        out_ap=gath[:, :(j1 - j0) * H].rearrange("p (t h) -> p t h", h=H),
                        in_ap=ag_out[:].rearrange("c (n h) -> (c n) h", h=H),
                        idxs_ap=idx_sb[:, j * NIB + j0 * 8:j * NIB + j1 * 8],
                        num_idxs=(j1 - j0) * P, num_idxs_reg=(j1 - j0) * P,
                        elem_size=H, single_packet=False)
                    gts.append(gath)
                return gts

            def emit_sel_chain(j, gts):
                nb = N_DENSE + j
                ps_t = psA.tile([P, H], f32, tag="mm")
                for t in range(T_g):
                    ci = 0 if t < T0 else 1
                    tt = t if t < T0 else t - T0
                    nc.tensor.matmul(
                        out=ps_t[:],
                        lhsT=sel_sb[:, (j * T_g + t) * P:(j * T_g + t + 1) * P],
                        rhs=gts[ci][:, tt * H:(tt + 1) * H],
                        start=(t == 0), stop=(t == T_g - 1))
                post_block(nb, ps_t)

            def emit_dense_chain(nb):
                # A streamed in half-blocks of 40 chunks, software-pipelined:
                # the load for half h+2 is issued before consuming half h.
                ps_t = psA.tile([P, H], f32, tag="mm")
                for half in range(2):
                    nh = nb * 2 + half
                    if nh + 2 < n_halves:
                        a_prefetch(nh + 2)
                    a_t = a_fifo.pop(0)
                    for cc in range(HC):
                        c = half * HC + cc
                        nc.tensor.matmul(
                            out=ps_t[:],
                            lhsT=a_t[:, cc * P:(cc + 1) * P],
                            rhs=tab_sb[:, c * H:(c + 1) * H],
                            start=(c == 0), stop=(c == NCHUNK - 1))
                post_block(nb, ps_t)

            # interleave: gathers 0,1 in flight; after each dense chain,
            # drain one ready gather with its sel chain and issue the next.
            pending = []
            next_g = 0
            if N_GATH and next_g < N_GATH:
                pending.append((next_g, emit_gather(next_g))); next_g += 1
            for nb in range(N_DENSE):
                emit_dense_chain(nb)
                if next_g < N_GATH:
                    pending.append((next_g, emit_gather(next_g))); next_g += 1
                if pending and (len(pending) >= 2 or nb >= N_DENSE - 1):
                    j, gts = pending.pop(0)
                    emit_sel_chain(j, gts)
            while next_g < N_GATH:
                pending.append((next_g, emit_gather(next_g))); next_g += 1
            for j, gts in pending:
                emit_sel_chain(j, gts)

            # stats: cross-partition reduce, replicate to 128 rows, AllReduce
            ps_s = psB.tile([1, 2 * H], f32, tag="vec")
            nc.tensor.matmul(out=ps_s[:, 0:H], lhsT=ones128[:], rhs=acc_s[:],
                             start=True, stop=True)
            nc.tensor.matmul(out=ps_s[:, H:2 * H], lhsT=ones128[:], rhs=acc_q[:],
                             start=True, stop=True)
            st_sb = scal[:, 6 * H:8 * H]
            nc.vector.tensor_copy(out=st_sb, in_=ps_s[:])
            # replicate to [128, 2H] so the AllReduce picks the fast RDH path
            st_rep = work.tile([P, 2 * H], f32, tag="strep")
            for half in range(2):
                ps_r2 = psB.tile([P, H], f32, tag="vec")
                nc.tensor.matmul(out=ps_r2[:], lhsT=ones1[:],
                                 rhs=st_sb[:, half * H:(half + 1) * H],
                                 start=True, stop=True)
                nc.vector.tensor_copy(out=st_rep[:, half * H:(half + 1) * H],
                                      in_=ps_r2[:])
            nc.sync.dma_start(out=ar_in[:], in_=st_rep[:])
            nc.gpsimd.collective_compute(
                "AllReduce", OP.add, replica_groups=rg,
                ins=[ar_in[:]], outs=[ar_out[:]])
            nc.sync.dma_start(out=stv[:], in_=ar_out[0:1, :])

            # a = gamma*istd ; c = beta - mu*a
            mu = scal[:, H:2 * H]
            var = scal[:, 2 * H:3 * H]
            av = scal[:, 3 * H:4 * H]
            cv = scal[:, 4 * H:5 * H]
            msq = scal[:, 5 * H:6 * H]
            nc.vector.tensor_scalar_mul(mu, stv[:, 0:H], 1.0 / N)
            nc.vector.tensor_scalar_mul(var, stv[:, H:2 * H], 1.0 / N)
            nc.vector.tensor_tensor(out=msq, in0=mu, in1=mu, op=OP.mult)
            nc.vector.tensor_tensor(out=var, in0=var, in1=msq, op=OP.subtract)
            nc.vector.tensor_scalar_add(var, var, BN_EPS)
            nc.vector.reciprocal(out=var, in_=var)
            nc.scalar.activation(out=var, in_=var, func=FT.Sqrt)  # istd
            nc.vector.tensor_tensor(out=av, in0=var,
                                    in1=gb_sb[:, l * H:(l + 1) * H], op=OP.mult)
            nc.vector.tensor_tensor(out=msq, in0=mu, in1=av, op=OP.mult)
            nc.vector.tensor_tensor(out=cv, in0=gb_sb[:, (L + l) * H:(L + l + 1) * H],
                                    in1=msq, op=OP.subtract)
            ps_a = psB.tile([P, H], f32, tag="vec")
            nc.tensor.matmul(out=ps_a[:], lhsT=ones1[:], rhs=av, start=True, stop=True)
            nc.vector.tensor_copy(out=a_rep[:], in_=ps_a[:])
            ps_c = psB.tile([P, H], f32, tag="vec")
            nc.tensor.matmul(out=ps_c[:], lhsT=ones1[:], rhs=cv, start=True, stop=True)
            nc.vector.tensor_copy(out=c_rep[:], in_=ps_c[:])

            # h = relu(t*a + c) + h ; prepare hsT for next layer
            for nb in range(BPC):
                tsl = t_all[:, nb * H:(nb + 1) * H]
                u = work.tile([P, H], f32, tag="tmp")
                nc.vector.tensor_tensor(out=u[:], in0=tsl, in1=a_rep[:], op=OP.mult)
                nc.vector.tensor_tensor(out=u[:], in0=u[:], in1=c_rep[:], op=OP.add)
                r = work.tile([P, H], f32, tag="tmp2")
                nc.scalar.activation(out=r[:], in_=u[:], func=FT.Relu)
                nc.vector.tensor_tensor(out=hslice(nb), in0=hslice(nb), in1=r[:], op=OP.add)
                if l < L - 1:
                    emit_hs_transpose(nb)

        # ---- pooling ----------------------------------------------------
        ps_p0 = psC.tile([P, G], f32, tag="p0")
        ps_p1 = psC.tile([P, G], f32, tag="p1")
        ps_pc = psB.tile([1, G], f32, tag="vec")
        for nb in range(BPC):
            psel_t = stream.tile([P, G], f32, tag="psel_t")
            nc.sync.dma_start(out=psel_t[:], in_=d_psel[:, nb * G:(nb + 1) * G])
            pssl = psel_t[:]
            nc.tensor.matmul(out=ps_p0[:], lhsT=h_sb[:, nb * H:nb * H + P],
                             rhs=pssl, start=(nb == 0), stop=(nb == BPC - 1))
            nc.tensor.matmul(out=ps_p1[:], lhsT=h_sb[:, nb * H + P:(nb + 1) * H],
                             rhs=pssl, start=(nb == 0), stop=(nb == BPC - 1))
            nc.tensor.matmul(out=ps_pc[:], lhsT=ones128[:],
                             rhs=pssl, start=(nb == 0), stop=(nb == BPC - 1))
        g0 = work.tile([P, G], f32, tag="g0")
        g1 = work.tile([P, G], f32, tag="g1")
        cnt = scal[:, 0:G]
        nc.vector.tensor_copy(out=g0[:], in_=ps_p0[:])
        nc.vector.tensor_copy(out=g1[:], in_=ps_p1[:])
        nc.vector.tensor_copy(out=cnt, in_=ps_pc[:])
        nc.sync.dma_start(out=pr_in[0:P, :], in_=g0[:])
        nc.sync.dma_start(out=pr_in[P:2 * P, :], in_=g1[:])
        nc.sync.dma_start(out=pr_in[2 * P:2 * P + 1, :], in_=cnt)
        nc.gpsimd.collective_compute(
            "AllReduce", OP.add, replica_groups=rg,
            ins=[pr_in[:]], outs=[pr_out[:]])
        nc.sync.dma_start(out=g0[:], in_=pr_out[0:P, :])
        nc.sync.dma_start(out=g1[:], in_=pr_out[P:2 * P, :])
        nc.sync.dma_start(out=cnt, in_=pr_out[2 * P:2 * P + 1, :])
        nc.vector.tensor_scalar_max(cnt, cnt, 1.0)
        nc.vector.reciprocal(out=cnt, in_=cnt)
        ps_r = psB.tile([P, G], f32, tag="vec")
        nc.tensor.matmul(out=ps_r[:], lhsT=ones1[:], rhs=cnt, start=True, stop=True)
        rc_rep = work.tile([P, G], f32, tag="rc_rep")
        nc.vector.tensor_copy(out=rc_rep[:], in_=ps_r[:])
        nc.vector.tensor_tensor(out=g0[:], in0=g0[:], in1=rc_rep[:], op=OP.mult)
        nc.vector.tensor_tensor(out=g1[:], in0=g1[:], in1=rc_rep[:], op=OP.mult)

        # MLP head (transposed: weights are lhsT, graphs along free dim)
        ps1 = psB.tile([P, G], f32, tag="vec")
        nc.tensor.matmul(out=ps1[:], lhsT=w1_sb[:, 0:P], rhs=g0[:], start=True, stop=False)
        nc.tensor.matmul(out=ps1[:], lhsT=w1_sb[:, P:2 * P], rhs=g1[:], start=False, stop=True)
        y1 = work.tile([P, G], f32, tag="y1")
        nc.scalar.activation(out=y1[:], in_=ps1[:], func=FT.Relu, bias=b1_sb[:, 0:1])
        ps2 = psB.tile([64, G], f32, tag="vec")
        nc.tensor.matmul(out=ps2[:], lhsT=w2_sb[:], rhs=y1[:], start=True, stop=True)
        y2 = work.tile([64, G], f32, tag="y2")
        nc.scalar.activation(out=y2[:], in_=ps2[:], func=FT.Relu, bias=b2_sb[:, 0:1])
        ps3 = psB.tile([1, G], f32, tag="vec")
        nc.tensor.matmul(out=ps3[:], lhsT=w3_sb[:], rhs=y2[:], start=True, stop=True)
        y3 = work.tile([1, G], f32, tag="y3")
        nc.vector.tensor_scalar_add(y3[:], ps3[:], b3_sb[0:1, 0:1])
        nc.sync.dma_start(out=d_out[:], in_=y3[:])

    nc.compile()
    return nc


# --------------------------------------------------------------------------
# entry point
# --------------------------------------------------------------------------

def kernel(x, edge_index, batch_ids, emb, W, b, gamma, beta,
           mlp_W1, mlp_b1, mlp_W2, mlp_b2, mlp_W3, mlp_b3,
           _trace=False, _trace_kwargs=None):
    # NB: reference BN subtracts the per-channel mean, so the additive bias b
    # cancels exactly and is not needed by the device program.
    T_g, in_maps = _preprocess(x, edge_index, batch_ids, emb, W, gamma, beta,
                               mlp_W1, mlp_b1, mlp_W2, mlp_b2, mlp_W3, mlp_b3)
    key = (T_g, N_DENSE)
    if key not in _compiled:
        _compiled[key] = _build(T_g)
    nc = _compiled[key]
    kw = {}
    if _trace:
        kw = dict(trace=True, **(_trace_kwargs or {}))
    res = run_bass_kernel_spmd(nc, in_maps, core_ids=list(range(NCORE)), **kw)
    out = np.asarray(res.results[0]["out"], np.float32).reshape(G, 1)
    kernel._last_results = res
    return out


# revision 21
# speedup vs baseline: 2.3930x; 1.0295x over previous
"""Trainium2 Bass kernel for HIVNet GCN message passing (8-core SPMD).

v6 strategy (baseline 2.29ms -> v5 hybrid 1.48ms -> v6 pure-dense):
  - Pad N=10000 nodes to 10240 = 80 chunks x 128; core c owns 10 dst-blocks
    (global chunks c*10..c*10+9).
  - Per layer: hws = (h*nrm) @ W[l] on the owned shard (bf16), AllGather the
    partition-major table (row c*128+p holds core c's 10 chunks for
    partition p as one contiguous 5KB run), bulk-load into SBUF.
  - Aggregation is PURE dense one-hot adjacency on TensorE: per dst-block an
    80-chunk PSUM-accumulated matmul chain (lhsT = A chunk [128 src x 128
    dst] with edge-multiplicity counts, rhs = table chunk [128 src x 256]).
    A (26MB/core) is streamed from HBM in half-block tiles, software
    pipelined 2 halves ahead; zero GPSIMD, zero scattered DMA (SWDGE
    gathers at 8ns/row descriptor cost were the v1 bottleneck, and their
    512B scattered packets degraded the whole DMA subsystem).
  - The AllGather is split in two half-payload collectives: the second half
    transfers while TensorE chews the first half's chunks (chain order is
    free under PSUM accumulation).
  - BatchNorm: partial sums/sumsq -> stats replicated to 128 rows so the
    2KB AllReduce becomes a 256KB RDH AllReduce (Mesh at 2KB costs 79us);
    scale/shift broadcast via rank-1 TensorE matmul; relu+residual on DVE.
  - Readout: graph mean-pool via preloaded bf16 one-hot pool matrices,
    257-row AllReduce, 3-layer MLP computed redundantly on every core.
"""

import os
import sys

sys.path.insert(0, "/opt/trn_rl_repo")

from contextlib import ExitStack

import numpy as np
import ml_dtypes

from concourse import bass, mybir, bacc, tile, library_config
from concourse.bass_utils import run_bass_kernel_spmd
from concourse.masks import make_identity

NCORE = 8
P = 128
H = 256
L = 4
NF = 9
G = 256
N = 10000
BPC = 10                # dst blocks per core
NPC = BPC * P           # 1280 nodes per core
NPAD = NCORE * NPC      # 10240
NCHUNK = NPAD // P      # 80 src chunks
HB = BPC // 2           # blocks per AllGather half
BN_EPS = 1e-5

f32 = mybir.dt.float32
bf16 = mybir.dt.bfloat16
bfnp = ml_dtypes.bfloat16

FT = mybir.ActivationFunctionType
OP = mybir.AluOpType

_compiled = {}

# chunk consumption order: first-half chunks (block%10 < 5) first, so the
# dense chains can start right after AllGather half A lands
CHUNK_ORDER = [g for g in range(NCHUNK) if g % BPC < HB] + \
              [g for g in range(NCHUNK) if g % BPC >= HB]


# --------------------------------------------------------------------------
# host-side structural preprocessing
# --------------------------------------------------------------------------

def _preprocess(x, edge_index, batch_ids, emb, W, gamma, beta,
                mlp_W1, mlp_b1, mlp_W2, mlp_b2, mlp_W3, mlp_b3):
    src = np.asarray(edge_index[0], np.int64)
    dst = np.asarray(edge_index[1], np.int64)
    # self loops for every real node (weight nrm[d]^2 folds in)
    src_all = np.concatenate([src, np.arange(N, dtype=np.int64)])
    dst_all = np.concatenate([dst, np.arange(N, dtype=np.int64)])
    order = np.argsort(dst_all, kind="stable")
    s_sorted = src_all[order]
    d_sorted = dst_all[order]

    deg = np.bincount(dst_all, minlength=NPAD).astype(np.float64)  # incl self

    nblk = NCORE * BPC
    starts = np.searchsorted(d_sorted, np.arange(nblk) * P)
    ends = np.searchsorted(d_sorted, (np.arange(nblk) + 1) * P)

    # dense adjacency per dst block, chunk-major in CHUNK_ORDER
    A_blocks = {}
    for g in range(nblk):
        c, nb = divmod(g, BPC)
        e_s = s_sorted[starts[g]:ends[g]]
        e_d = d_sorted[starts[g]:ends[g]] - g * P
        A = np.zeros((NPAD, P), np.float32)
        np.add.at(A, (e_s, e_d), 1.0)
        A = A.reshape(NCHUNK, P, P)[CHUNK_ORDER]          # reorder chunks
        A_blocks[(c, nb)] = np.ascontiguousarray(
            A.transpose(1, 0, 2).reshape(P, NCHUNK * P)).astype(bfnp)

    # graph pool one-hot [node, graph] (bf16: values 0/1 exact)
    bids = np.asarray(batch_ids, np.int64)
    psel_full = np.zeros((NPAD, G), np.float32)
    psel_full[np.arange(N), bids] = 1.0

    x_np = np.zeros((NPAD, NF), np.float32)
    x_np[:N] = np.asarray(x, np.float64)

    Wf = np.asarray(W, np.float32)
    W_lhsT = Wf.reshape(L, 2, P, H).transpose(2, 0, 1, 3).reshape(P, L * 2 * H)
    gb = np.concatenate([np.asarray(gamma, np.float32).reshape(-1),
                         np.asarray(beta, np.float32).reshape(-1)])[None, :]
    embf = np.asarray(emb, np.float32)
    emb0 = np.ascontiguousarray(embf[:, 0, :])
    emb1 = np.ascontiguousarray(embf[:, 1, :])
    w1 = np.asarray(mlp_W1, np.float32).reshape(2, P, P).transpose(1, 0, 2).reshape(P, 2 * P)
    w2 = np.asarray(mlp_W2, np.float32)
    w3 = np.asarray(mlp_W3, np.float32)
    b1 = np.asarray(mlp_b1, np.float32).reshape(P, 1)
    b2 = np.asarray(mlp_b2, np.float32).reshape(64, 1)
    b3 = np.asarray(mlp_b3, np.float32).reshape(1, 1)

    in_maps = []
    for c in range(NCORE):
        lo, hi = c * NPC, (c + 1) * NPC
        Ac = np.concatenate([A_blocks[(c, nb)] for nb in range(BPC)], axis=1)

        degc = deg[lo:hi].reshape(BPC, P).T
        maskc = (degc > 0).astype(np.float32)
        degc = np.maximum(degc, 1.0).astype(np.float32)

        pselc = psel_full[lo:hi].reshape(BPC, P, G)
        pselc = np.ascontiguousarray(pselc.transpose(1, 0, 2)).reshape(P, BPC * G)

        in_maps.append(dict(
            A=Ac, xT=np.ascontiguousarray(x_np[lo:hi].T),
            deg=degc, mask=maskc, psel=pselc.astype(bfnp),
            W=W_lhsT.astype(bfnp), gb=gb, emb0=emb0, emb1=emb1,
            w1=w1, w2=w2, w3=w3, b1=b1, b2=b2, b3=b3,
        ))
    return in_maps


# --------------------------------------------------------------------------
# device program
# --------------------------------------------------------------------------

def _build():
    nc = bacc.Bacc(None, target_bir_lowering=False)

    d_A = nc.dram_tensor("A", [P, BPC * NCHUNK * P], bf16, kind="ExternalInput")
    d_xT = nc.dram_tensor("xT", [NF, NPC], f32, kind="ExternalInput")
    d_deg = nc.dram_tensor("deg", [P, BPC], f32, kind="ExternalInput")
    d_mask = nc.dram_tensor("mask", [P, BPC], f32, kind="ExternalInput")
    d_psel = nc.dram_tensor("psel", [P, BPC * G], bf16, kind="ExternalInput")
    d_W = nc.dram_tensor("W", [P, L * 2 * H], bf16, kind="ExternalInput")
    d_gb = nc.dram_tensor("gb", [1, 2 * L * H], f32, kind="ExternalInput")
    d_emb0 = nc.dram_tensor("emb0", [NF, H], f32, kind="ExternalInput")
    d_emb1 = nc.dram_tensor("emb1", [NF, H], f32, kind="ExternalInput")
    d_w1 = nc.dram_tensor("w1", [P, 2 * P], f32, kind="ExternalInput")
    d_w2 = nc.dram_tensor("w2", [P, 64], f32, kind="ExternalInput")
    d_w3 = nc.dram_tensor("w3", [64, 1], f32, kind="ExternalInput")
    d_b1 = nc.dram_tensor("b1", [P, 1], f32, kind="ExternalInput")
    d_b2 = nc.dram_tensor("b2", [64, 1], f32, kind="ExternalInput")
    d_b3 = nc.dram_tensor("b3", [1, 1], f32, kind="ExternalInput")
    d_out = nc.dram_tensor("out", [1, G], f32, kind="ExternalOutput")

    rg = [list(range(NCORE))]
    HW = HB * H          # half payload width per partition (1280 cols)

    with tile.TileContext(nc) as tc, ExitStack() as ctx:
        pers = ctx.enter_context(tc.tile_pool(name="pers", bufs=1))
        psA = ctx.enter_context(tc.tile_pool(name="psA", bufs=2, space="PSUM"))
        psB = ctx.enter_context(tc.tile_pool(name="psB", bufs=2, space="PSUM"))
        psC = ctx.enter_context(tc.tile_pool(name="psC", bufs=1, space="PSUM"))
        apool = ctx.enter_context(tc.tile_pool(name="apool", bufs=3))
        work = ctx.enter_context(tc.tile_pool(name="work", bufs=2))
        stream = ctx.enter_context(tc.tile_pool(name="stream", bufs=2))
        dram = ctx.enter_context(tc.tile_pool(name="dram", bufs=2, space="DRAM"))

        # ---- persistent SBUF state -------------------------------------
        deg_sb = pers.tile([P, BPC], f32, tag="deg")
        mask_sb = pers.tile([P, BPC], f32, tag="mask")
        psel_sb = pers.tile([P, BPC * G], bf16, tag="psel")
        W_sb = pers.tile([P, L * 2 * H], bf16, tag="W")
        gb_sb = pers.tile([1, 2 * L * H], f32, tag="gb")
        emb0_sb = pers.tile([NF, H], f32, tag="emb0")
        emb1_sb = pers.tile([NF, H], f32, tag="emb1")
        w1_sb = pers.tile([P, 2 * P], f32, tag="w1")
        w2_sb = pers.tile([P, 64], f32, tag="w2")
        w3_sb = pers.tile([64, 1], f32, tag="w3")
        b1_sb = pers.tile([P, 1], f32, tag="b1")
        b2_sb = pers.tile([64, 1], f32, tag="b2")
        b3_sb = pers.tile([1, 1], f32, tag="b3")

        tab_sb = pers.tile([P, NCHUNK * H], bf16, tag="tab")
        h_sb = pers.tile([P, BPC * H], f32, tag="h")
        hsT_sb = pers.tile([P, BPC * 2 * P], bf16, tag="hsT")
        hws_sb = pers.tile([P, BPC * H], bf16, tag="hws")
        t_all = pers.tile([P, BPC * H], f32, tag="t_all")
        nrm_sb = pers.tile([P, BPC], f32, tag="nrm")
        acc_s = pers.tile([P, H], f32, tag="acc_s")
        acc_q = pers.tile([P, H], f32, tag="acc_q")
        D_sb = pers.tile([NF, H], f32, tag="D")
        base_rep = pers.tile([P, H], f32, tag="base_rep")
        a_rep = pers.tile([P, H], f32, tag="a_rep")
        c_rep = pers.tile([P, H], f32, tag="c_rep")
        ident_bf = pers.tile([P, P], bf16, tag="ident")
        ones9 = pers.tile([NF, 1], f32, tag="ones9")
        ones1 = pers.tile([1, P], f32, tag="ones1")
        ones128 = pers.tile([P, 1], f32, tag="ones128")
        ones128b = pers.tile([P, 1], bf16, tag="ones128b")
        stv = pers.tile([1, 2 * H], f32, tag="stv")
        scal = pers.tile([1, 8 * H], f32, tag="scal")

        # ---- DRAM bounce buffers ---------------------------------------
        # AllGather halves: ag_inX[p, :] = hws rows for blocks of that half
        # (5 blocks x 256 = 1280 cols, a contiguous 2.5KB run per partition;
        # ag_outX row c*128+p holds core c's half-run for partition p).
        ag_inA = dram.tile([P, HW], bf16, tag="ag_inA")
        ag_inB = dram.tile([P, HW], bf16, tag="ag_inB")
        ag_outA = dram.tile([NCORE * P, HW], bf16, tag="ag_outA")
        ag_outB = dram.tile([NCORE * P, HW], bf16, tag="ag_outB")
        ar_in = dram.tile([P, 2 * H], f32, tag="ar_in")
        ar_out = dram.tile([P, 2 * H], f32, tag="ar_out")
        pr_in = dram.tile([2 * P + 1, G], f32, tag="pr_in")
        pr_out = dram.tile([2 * P + 1, G], f32, tag="pr_out")

        # ---- input loads ------------------------------------------------
        for t, d in [(deg_sb, d_deg), (mask_sb, d_mask), (psel_sb, d_psel),
                     (W_sb, d_W), (gb_sb, d_gb), (emb0_sb, d_emb0),
                     (emb1_sb, d_emb1), (w1_sb, d_w1), (w2_sb, d_w2),
                     (w3_sb, d_w3), (b1_sb, d_b1), (b2_sb, d_b2),
                     (b3_sb, d_b3)]:
            nc.sync.dma_start(out=t[:], in_=d[:])

        make_identity(nc, ident_bf[:])
        nc.vector.memset(ones9[:], 1.0)
        nc.vector.memset(ones1[:], 1.0)
        nc.vector.memset(ones128[:], 1.0)
        nc.vector.memset(ones128b[:], 1.0)

        # nrm = rsqrt(deg) * mask
        rdeg = work.tile([P, BPC], f32, tag="rdeg")
        nc.vector.reciprocal(out=rdeg[:], in_=deg_sb[:])
        nc.scalar.activation(out=rdeg[:], in_=rdeg[:], func=FT.Sqrt)
        nc.vector.tensor_tensor(out=nrm_sb[:], in0=rdeg[:], in1=mask_sb[:], op=OP.mult)

        # encoder prep: D = emb1 - emb0 ; base = ones9^T @ emb0, broadcast
        nc.vector.tensor_tensor(out=D_sb[:], in0=emb1_sb[:], in1=emb0_sb[:], op=OP.subtract)
        ps_b = psB.tile([1, H], f32, tag="vec")
        nc.tensor.matmul(out=ps_b[:], lhsT=ones9[:], rhs=emb0_sb[:], start=True, stop=True)
        bvec = scal[:, 0:H]
        nc.vector.tensor_copy(out=bvec, in_=ps_b[:])
        ps_br = psB.tile([P, H], f32, tag="vec")
        nc.tensor.matmul(out=ps_br[:], lhsT=ones1[:], rhs=bvec, start=True, stop=True)
        nc.vector.tensor_copy(out=base_rep[:], in_=ps_br[:])

        def hslice(nb):
            return h_sb[:, nb * H:(nb + 1) * H]

        def emit_hs_transpose(nb):
            """hs = h*nrm (bf16), transpose both 128-halves into hsT_sb."""
            hs_bf = work.tile([P, H], bf16, tag="hs_bf")
            nc.vector.tensor_scalar_mul(hs_bf[:], hslice(nb), nrm_sb[:, nb:nb + 1])
            for k in range(2):
                pst = psB.tile([P, P], bf16, tag="pst")
                nc.tensor.transpose(out=pst[:], in_=hs_bf[:, k * P:(k + 1) * P],
                                    identity=ident_bf[:])
                nc.vector.tensor_copy(out=hsT_sb[:, (nb * 2 + k) * P:(nb * 2 + k + 1) * P],
                                      in_=pst[:])

        # encoder: h0 = base + xT^T @ D  (per block)
        for nb in range(BPC):
            xT_t = stream.tile([NF, P], f32, tag="xT_t")
            nc.sync.dma_start(out=xT_t[:], in_=d_xT[:, nb * P:(nb + 1) * P])
            ps_h = psA.tile([P, H], f32, tag="mm")
            nc.tensor.matmul(out=ps_h[:], lhsT=xT_t[:],
                             rhs=D_sb[:], start=True, stop=True)
            nc.vector.tensor_tensor(out=hslice(nb), in0=ps_h[:], in1=base_rep[:], op=OP.add)
            emit_hs_transpose(nb)

        HC = NCHUNK // 2     # chunks per A half-tile

        # ---- layers -----------------------------------------------------
        for l in range(L):
            a_fifo = []

            def a_prefetch(nh):
                a_t = apool.tile([P, HC * P], bf16, tag="A")
                nc.sync.dma_start(out=a_t[:], in_=d_A[:, nh * HC * P:(nh + 1) * HC * P])
                a_fifo.append(a_t)

            def emit_gemm(nb):
                ps_g = psA.tile([P, H], f32, tag="mm")
                for k in range(2):
                    nc.tensor.matmul(
                        out=ps_g[:],
                        lhsT=hsT_sb[:, (nb * 2 + k) * P:(nb * 2 + k + 1) * P],
                        rhs=W_sb[:, (l * 2 + k) * H:(l * 2 + k + 1) * H],
                        start=(k == 0), stop=(k == 1))
                nc.vector.tensor_copy(out=hws_sb[:, nb * H:(nb + 1) * H], in_=ps_g[:])

            # GEMM half A -> AllGather A posts while half B GEMMs run
            for nb in range(0, HB):
                emit_gemm(nb)
            nc.sync.dma_start(out=ag_inA[:], in_=hws_sb[:, 0:HW])
            a_prefetch(0)
            a_prefetch(1)
            nc.gpsimd.collective_compute(
                "AllGather", OP.bypass, replica_groups=rg,
                ins=[ag_inA[:]], outs=[ag_outA[:]])
            for nb in range(HB, BPC):
                emit_gemm(nb)
            nc.sync.dma_start(out=ag_inB[:], in_=hws_sb[:, HW:2 * HW])
            nc.gpsimd.collective_compute(
                "AllGather", OP.bypass, replica_groups=rg,
                ins=[ag_inB[:]], outs=[ag_outB[:]])

            # bulk-load both table halves into SBUF (first-half chunks of
            # every core occupy tab columns [0, 40*H), matching CHUNK_ORDER)
            nc.sync.dma_start(
                out=tab_sb[:, 0:NCHUNK * H // 2].rearrange("p (c w) -> p c w", c=NCORE),
                in_=ag_outA[:].rearrange("(c p) w -> p c w", p=P))
            nc.sync.dma_start(
                out=tab_sb[:, NCHUNK * H // 2:].rearrange("p (c w) -> p c w", c=NCORE),
                in_=ag_outB[:].rearrange("(c p) w -> p c w", p=P))

            nc.vector.memset(acc_s[:], 0.0)
            nc.vector.memset(acc_q[:], 0.0)

            # tab column of CHUNK_ORDER[k]: half = k // 40, core = chunk//10,
            # block-in-half = chunk%10 (mod 5) -> with the half-major tab
            # layout this is simply column k*H. (CHUNK_ORDER groups half-A
            # chunks first; within a half, chunks are ordered (c, nb) which
            # is exactly the ag_outX row-major order.)

            def emit_dense_chain(nb):
                ps_t = psA.tile([P, H], f32, tag="mm")
                for half in range(2):
                    nh = nb * 2 + half
                    if nh + 2 < 2 * BPC:
                        a_prefetch(nh + 2)
                    a_t = a_fifo.pop(0)
                    for cc in range(HC):
                        k = half * HC + cc
                        nc.tensor.matmul(
                            out=ps_t[:],
                            lhsT=a_t[:, cc * P:(cc + 1) * P],
                            rhs=tab_sb[:, k * H:(k + 1) * H],
                            start=(k == 0), stop=(k == NCHUNK - 1))
                post_block(nb, ps_t)

            def post_block(nb, ps_t):
                """t = nrm*psum; accumulate BN stats."""
                tsl = t_all[:, nb * H:(nb + 1) * H]
                nc.scalar.activation(out=tsl, in_=ps_t[:], func=FT.Copy,
                                     scale=nrm_sb[:, nb:nb + 1])
                sq = work.tile([P, H], f32, tag="tmp")
                nc.vector.tensor_tensor(out=sq[:], in0=tsl, in1=tsl, op=OP.mult)
                nc.vector.tensor_tensor(out=acc_s[:], in0=acc_s[:], in1=tsl, op=OP.add)
                nc.vector.tensor_tensor(out=acc_q[:], in0=acc_q[:], in1=sq[:], op=OP.add)

            for nb in range(BPC):
                emit_dense_chain(nb)

            # stats: cross-partition reduce, replicate to 128 rows, AllReduce
            ps_s = psB.tile([1, 2 * H], f32, tag="vec")
            nc.tensor.matmul(out=ps_s[:, 0:H], lhsT=ones128[:], rhs=acc_s[:],
                             start=True, stop=True)
            nc.tensor.matmul(out=ps_s[:, H:2 * H], lhsT=ones128[:], rhs=acc_q[:],
                             start=True, stop=True)
            st_sb = scal[:, 6 * H:8 * H]
            nc.vector.tensor_copy(out=st_sb, in_=ps_s[:])
            st_rep = work.tile([P, 2 * H], f32, tag="strep")
            for half in range(2):
                ps_r2 = psB.tile([P, H], f32, tag="vec")
                nc.tensor.matmul(out=ps_r2[:], lhsT=ones1[:],
                                 rhs=st_sb[:, half * H:(half + 1) * H],
                                 start=True, stop=True)
                nc.vector.tensor_copy(out=st_rep[:, half * H:(half + 1) * H],
                                      in_=ps_r2[:])
            nc.sync.dma_start(out=ar_in[:], in_=st_rep[:])
            nc.gpsimd.collective_compute(
                "AllReduce", OP.add, replica_groups=rg,
                ins=[ar_in[:]], outs=[ar_out[:]])
            nc.sync.dma_start(out=stv[:], in_=ar_out[0:1, :])

            # a = gamma*istd ; c = beta - mu*a
            mu = scal[:, H:2 * H]
            var = scal[:, 2 * H:3 * H]
            av = scal[:, 3 * H:4 * H]
            cv = scal[:, 4 * H:5 * H]
            msq = scal[:, 5 * H:6 * H]
            nc.vector.tensor_scalar_mul(mu, stv[:, 0:H], 1.0 / N)
            nc.vector.tensor_scalar_mul(var, stv[:, H:2 * H], 1.0 / N)
            nc.vector.tensor_tensor(out=msq, in0=mu, in1=mu, op=OP.mult)
            nc.vector.tensor_tensor(out=var, in0=var, in1=msq, op=OP.subtract)
            nc.vector.tensor_scalar_add(var, var, BN_EPS)
            nc.vector.reciprocal(out=var, in_=var)
            nc.scalar.activation(out=var, in_=var, func=FT.Sqrt)  # istd
            nc.vector.tensor_tensor(out=av, in0=var,
                                    in1=gb_sb[:, l * H:(l + 1) * H], op=OP.mult)
            nc.vector.tensor_tensor(out=msq, in0=mu, in1=av, op=OP.mult)
            nc.vector.tensor_tensor(out=cv, in0=gb_sb[:, (L + l) * H:(L + l + 1) * H],
                                    in1=msq, op=OP.subtract)
            ps_a = psB.tile([P, H], f32, tag="vec")
            nc.tensor.matmul(out=ps_a[:], lhsT=ones1[:], rhs=av, start=True, stop=True)
            nc.vector.tensor_copy(out=a_rep[:], in_=ps_a[:])
            ps_c = psB.tile([P, H], f32, tag="vec")
            nc.tensor.matmul(out=ps_c[:], lhsT=ones1[:], rhs=cv, start=True, stop=True)
            nc.vector.tensor_copy(out=c_rep[:], in_=ps_c[:])

            # h = relu(t*a + c) + h ; prepare hsT for next layer
            for nb in range(BPC):
                tsl = t_all[:, nb * H:(nb + 1) * H]
                u = work.tile([P, H], f32, tag="tmp")
                nc.vector.tensor_tensor(out=u[:], in0=tsl, in1=a_rep[:], op=OP.mult)
                nc.vector.tensor_tensor(out=u[:], in0=u[:], in1=c_rep[:], op=OP.add)
                r = work.tile([P, H], f32, tag="tmp2")
                nc.scalar.activation(out=r[:], in_=u[:], func=FT.Relu)
                nc.vector.tensor_tensor(out=hslice(nb), in0=hslice(nb), in1=r[:], op=OP.add)
                if l < L - 1:
                    emit_hs_transpose(nb)

        # ---- pooling (bf16 matmuls; psel preloaded) ---------------------
        ps_p0 = psC.tile([P, G], f32, tag="p0")
        ps_p1 = psC.tile([P, G], f32, tag="p1")
        ps_pc = psB.tile([1, G], f32, tag="vec")
        for nb in range(BPC):
            hb_t = work.tile([P, H], bf16, tag="hb")
            nc.vector.tensor_copy(out=hb_t[:], in_=hslice(nb))
            pssl = psel_sb[:, nb * G:(nb + 1) * G]
            nc.tensor.matmul(out=ps_p0[:], lhsT=hb_t[:, 0:P],
                             rhs=pssl, start=(nb == 0), stop=(nb == BPC - 1))
            nc.tensor.matmul(out=ps_p1[:], lhsT=hb_t[:, P:2 * P],
                             rhs=pssl, start=(nb == 0), stop=(nb == BPC - 1))
            nc.tensor.matmul(out=ps_pc[:], lhsT=ones128b[:],
                             rhs=pssl, start=(nb == 0), stop=(nb == BPC - 1))
        g0 = work.tile([P, G], f32, tag="g0")
        g1 = work.tile([P, G], f32, tag="g1")
        cnt = scal[:, 0:G]
        nc.vector.tensor_copy(out=g0[:], in_=ps_p0[:])
        nc.vector.tensor_copy(out=g1[:], in_=ps_p1[:])
        nc.vector.tensor_copy(out=cnt, in_=ps_pc[:])
        nc.sync.dma_start(out=pr_in[0:P, :], in_=g0[:])
        nc.sync.dma_start(out=pr_in[P:2 * P, :], in_=g1[:])
        nc.sync.dma_start(out=pr_in[2 * P:2 * P + 1, :], in_=cnt)
        nc.gpsimd.collective_compute(
            "AllReduce", OP.add, replica_groups=rg,
            ins=[pr_in[:]], outs=[pr_out[:]])
        nc.sync.dma_start(out=g0[:], in_=pr_out[0:P, :])
        nc.sync.dma_start(out=g1[:], in_=pr_out[P:2 * P, :])
        nc.sync.dma_start(out=cnt, in_=pr_out[2 * P:2 * P + 1, :])
        nc.vector.tensor_scalar_max(cnt, cnt, 1.0)
        nc.vector.reciprocal(out=cnt, in_=cnt)
        ps_r = psB.tile([P, G], f32, tag="vec")
        nc.tensor.matmul(out=ps_r[:], lhsT=ones1[:], rhs=cnt, start=True, stop=True)
        rc_rep = work.tile([P, G], f32, tag="rc_rep")
        nc.vector.tensor_copy(out=rc_rep[:], in_=ps_r[:])
        nc.vector.tensor_tensor(out=g0[:], in0=g0[:], in1=rc_rep[:], op=OP.mult)
        nc.vector.tensor_tensor(out=g1[:], in0=g1[:], in1=rc_rep[:], op=OP.mult)

        # MLP head (transposed: weights are lhsT, graphs along free dim)
        ps1 = psB.tile([P, G], f32, tag="vec")
        nc.tensor.matmul(out=ps1[:], lhsT=w1_sb[:, 0:P], rhs=g0[:], start=True, stop=False)
        nc.tensor.matmul(out=ps1[:], lhsT=w1_sb[:, P:2 * P], rhs=g1[:], start=False, stop=True)
        y1 = work.tile([P, G], f32, tag="y1")
        nc.scalar.activation(out=y1[:], in_=ps1[:], func=FT.Relu, bias=b1_sb[:, 0:1])
        ps2 = psB.tile([64, G], f32, tag="vec")
        nc.tensor.matmul(out=ps2[:], lhsT=w2_sb[:], rhs=y1[:], start=True, stop=True)
        y2 = work.tile([64, G], f32, tag="y2")
        nc.scalar.activation(out=y2[:], in_=ps2[:], func=FT.Relu, bias=b2_sb[:, 0:1])
        ps3 = psB.tile([1, G], f32, tag="vec")
        nc.tensor.matmul(out=ps3[:], lhsT=w3_sb[:], rhs=y2[:], start=True, stop=True)
        y3 = work.tile([1, G], f32, tag="y3")
        nc.vector.tensor_scalar_add(y3[:], ps3[:], b3_sb[0:1, 0:1])
        nc.sync.dma_start(out=d_out[:], in_=y3[:])

    nc.compile()
    return nc


# --------------------------------------------------------------------------
# entry point
# --------------------------------------------------------------------------

def kernel(x, edge_index, batch_ids, emb, W, b, gamma, beta,
           mlp_W1, mlp_b1, mlp_W2, mlp_b2, mlp_W3, mlp_b3,
           _trace=False, _trace_kwargs=None):
    # NB: reference BN subtracts the per-channel mean, so the additive bias b
    # cancels exactly and is not needed by the device program.
    in_maps = _preprocess(x, edge_index, batch_ids, emb, W, gamma, beta,
                          mlp_W1, mlp_b1, mlp_W2, mlp_b2, mlp_W3, mlp_b3)
    if "nc" not in _compiled:
        _compiled["nc"] = _build()
    nc = _compiled["nc"]
    kw = {}
    if _trace:
        kw = dict(trace=True, **(_trace_kwargs or {}))
    res = run_bass_kernel_spmd(nc, in_maps, core_ids=list(range(NCORE)), **kw)
    out = np.asarray(res.results[0]["out"], np.float32).reshape(G, 1)
    kernel._last_results = res
    return out


# revision 23
# speedup vs baseline: 2.3957x; 1.0011x over previous
"""Trainium2 Bass kernel for HIVNet GCN message passing (8-core SPMD).

v6 strategy (baseline 2.29ms -> v5 hybrid 1.48ms -> v6 pure-dense):
  - Pad N=10000 nodes to 10240 = 80 chunks x 128; core c owns 10 dst-blocks
    (global chunks c*10..c*10+9).
  - Per layer: hws = (h*nrm) @ W[l] on the owned shard (bf16), AllGather the
    partition-major table (row c*128+p holds core c's 10 chunks for
    partition p as one contiguous 5KB run), bulk-load into SBUF.
  - Aggregation is PURE dense one-hot adjacency on TensorE: per dst-block an
    80-chunk PSUM-accumulated matmul chain (lhsT = A chunk [128 src x 128
    dst] with edge-multiplicity counts, rhs = table chunk [128 src x 256]).
    A (26MB/core) is streamed from HBM in half-block tiles, software
    pipelined 2 halves ahead; zero GPSIMD, zero scattered DMA (SWDGE
    gathers at 8ns/row descriptor cost were the v1 bottleneck, and their
    512B scattered packets degraded the whole DMA subsystem).
  - The AllGather is split in two half-payload collectives: the second half
    transfers while TensorE chews the first half's chunks (chain order is
    free under PSUM accumulation).
  - BatchNorm: partial sums/sumsq -> stats replicated to 128 rows so the
    2KB AllReduce becomes a 256KB RDH AllReduce (Mesh at 2KB costs 79us);
    scale/shift broadcast via rank-1 TensorE matmul; relu+residual on DVE.
  - Readout: graph mean-pool via preloaded bf16 one-hot pool matrices,
    257-row AllReduce, 3-layer MLP computed redundantly on every core.
"""

import os
import sys

sys.path.insert(0, "/opt/trn_rl_repo")

from contextlib import ExitStack

import numpy as np
import ml_dtypes

from concourse import bass, mybir, bacc, tile, library_config
from concourse.bass_utils import run_bass_kernel_spmd
from concourse.masks import make_identity

NCORE = 8
P = 128
H = 256
L = 4
NF = 9
G = 256
N = 10000
BPC = 10                # dst blocks per core
NPC = BPC * P           # 1280 nodes per core
NPAD = NCORE * NPC      # 10240
NCHUNK = NPAD // P      # 80 src chunks
HB = BPC // 2           # blocks per AllGather half
BN_EPS = 1e-5

f32 = mybir.dt.float32
bf16 = mybir.dt.bfloat16
bfnp = ml_dtypes.bfloat16

FT = mybir.ActivationFunctionType
OP = mybir.AluOpType

_compiled = {}

# chunk consumption order: first-half chunks (block%10 < 5) first, so the
# dense chains can start right after AllGather half A lands
CHUNK_ORDER = [g for g in range(NCHUNK) if g % BPC < HB] + \
              [g for g in range(NCHUNK) if g % BPC >= HB]


# --------------------------------------------------------------------------
# host-side structural preprocessing
# --------------------------------------------------------------------------

def _preprocess(x, edge_index, batch_ids, emb, W, gamma, beta,
                mlp_W1, mlp_b1, mlp_W2, mlp_b2, mlp_W3, mlp_b3):
    src = np.asarray(edge_index[0], np.int64)
    dst = np.asarray(edge_index[1], np.int64)
    # self loops for every real node (weight nrm[d]^2 folds in)
    src_all = np.concatenate([src, np.arange(N, dtype=np.int64)])
    dst_all = np.concatenate([dst, np.arange(N, dtype=np.int64)])
    order = np.argsort(dst_all, kind="stable")
    s_sorted = src_all[order]
    d_sorted = dst_all[order]

    deg = np.bincount(dst_all, minlength=NPAD).astype(np.float64)  # incl self

    nblk = NCORE * BPC
    starts = np.searchsorted(d_sorted, np.arange(nblk) * P)
    ends = np.searchsorted(d_sorted, (np.arange(nblk) + 1) * P)

    # dense adjacency per dst block, chunk-major in CHUNK_ORDER
    A_blocks = {}
    for g in range(nblk):
        c, nb = divmod(g, BPC)
        e_s = s_sorted[starts[g]:ends[g]]
        e_d = d_sorted[starts[g]:ends[g]] - g * P
        A = np.zeros((NPAD, P), np.float32)
        np.add.at(A, (e_s, e_d), 1.0)
        A = A.reshape(NCHUNK, P, P)[CHUNK_ORDER]          # reorder chunks
        A_blocks[(c, nb)] = np.ascontiguousarray(
            A.transpose(1, 0, 2).reshape(P, NCHUNK * P)).astype(bfnp)

    # graph pool one-hot [node, graph] (bf16: values 0/1 exact)
    bids = np.asarray(batch_ids, np.int64)
    psel_full = np.zeros((NPAD, G), np.float32)
    psel_full[np.arange(N), bids] = 1.0

    x_np = np.zeros((NPAD, NF), np.float32)
    x_np[:N] = np.asarray(x, np.float64)

    Wf = np.asarray(W, np.float32)
    W_lhsT = Wf.reshape(L, 2, P, H).transpose(2, 0, 1, 3).reshape(P, L * 2 * H)
    gb = np.concatenate([np.asarray(gamma, np.float32).reshape(-1),
                         np.asarray(beta, np.float32).reshape(-1)])[None, :]
    embf = np.asarray(emb, np.float32)
    emb0 = np.ascontiguousarray(embf[:, 0, :])
    emb1 = np.ascontiguousarray(embf[:, 1, :])
    w1 = np.asarray(mlp_W1, np.float32).reshape(2, P, P).transpose(1, 0, 2).reshape(P, 2 * P)
    w2 = np.asarray(mlp_W2, np.float32)
    w3 = np.asarray(mlp_W3, np.float32)
    b1 = np.asarray(mlp_b1, np.float32).reshape(P, 1)
    b2 = np.asarray(mlp_b2, np.float32).reshape(64, 1)
    b3 = np.asarray(mlp_b3, np.float32).reshape(1, 1)

    in_maps = []
    HC = NCHUNK // 2
    for c in range(NCORE):
        lo, hi = c * NPC, (c + 1) * NPC
        Ac = np.concatenate([A_blocks[(c, nb)] for nb in range(BPC)], axis=1)
        # reorder half-tiles to match the two-pass consumption order:
        # tile nh = half*BPC + nb  (all first halves, then all second halves)
        Ac = np.ascontiguousarray(
            Ac.reshape(P, BPC, 2, HC * P).transpose(0, 2, 1, 3)
        ).reshape(P, BPC * NCHUNK * P)

        degc = deg[lo:hi].reshape(BPC, P).T
        maskc = (degc > 0).astype(np.float32)
        degc = np.maximum(degc, 1.0).astype(np.float32)

        pselc = psel_full[lo:hi].reshape(BPC, P, G)
        pselc = np.ascontiguousarray(pselc.transpose(1, 0, 2)).reshape(P, BPC * G)

        in_maps.append(dict(
            A=Ac, xT=np.ascontiguousarray(x_np[lo:hi].T),
            deg=degc, mask=maskc, psel=pselc.astype(bfnp),
            W=W_lhsT.astype(bfnp), gb=gb, emb0=emb0, emb1=emb1,
            w1=w1, w2=w2, w3=w3, b1=b1, b2=b2, b3=b3,
        ))
    return in_maps


# --------------------------------------------------------------------------
# device program
# --------------------------------------------------------------------------

def _build():
    nc = bacc.Bacc(None, target_bir_lowering=False)

    d_A = nc.dram_tensor("A", [P, BPC * NCHUNK * P], bf16, kind="ExternalInput")
    d_xT = nc.dram_tensor("xT", [NF, NPC], f32, kind="ExternalInput")
    d_deg = nc.dram_tensor("deg", [P, BPC], f32, kind="ExternalInput")
    d_mask = nc.dram_tensor("mask", [P, BPC], f32, kind="ExternalInput")
    d_psel = nc.dram_tensor("psel", [P, BPC * G], bf16, kind="ExternalInput")
    d_W = nc.dram_tensor("W", [P, L * 2 * H], bf16, kind="ExternalInput")
    d_gb = nc.dram_tensor("gb", [1, 2 * L * H], f32, kind="ExternalInput")
    d_emb0 = nc.dram_tensor("emb0", [NF, H], f32, kind="ExternalInput")
    d_emb1 = nc.dram_tensor("emb1", [NF, H], f32, kind="ExternalInput")
    d_w1 = nc.dram_tensor("w1", [P, 2 * P], f32, kind="ExternalInput")
    d_w2 = nc.dram_tensor("w2", [P, 64], f32, kind="ExternalInput")
    d_w3 = nc.dram_tensor("w3", [64, 1], f32, kind="ExternalInput")
    d_b1 = nc.dram_tensor("b1", [P, 1], f32, kind="ExternalInput")
    d_b2 = nc.dram_tensor("b2", [64, 1], f32, kind="ExternalInput")
    d_b3 = nc.dram_tensor("b3", [1, 1], f32, kind="ExternalInput")
    d_out = nc.dram_tensor("out", [1, G], f32, kind="ExternalOutput")

    rg = [list(range(NCORE))]
    HW = HB * H          # half payload width per partition (1280 cols)

    with tile.TileContext(nc) as tc, ExitStack() as ctx:
        pers = ctx.enter_context(tc.tile_pool(name="pers", bufs=1))
        psA = ctx.enter_context(tc.tile_pool(name="psA", bufs=2, space="PSUM"))
        psB = ctx.enter_context(tc.tile_pool(name="psB", bufs=2, space="PSUM"))
        psC = ctx.enter_context(tc.tile_pool(name="psC", bufs=1, space="PSUM"))
        apool = ctx.enter_context(tc.tile_pool(name="apool", bufs=3))
        work = ctx.enter_context(tc.tile_pool(name="work", bufs=2))
        stream = ctx.enter_context(tc.tile_pool(name="stream", bufs=2))
        dram = ctx.enter_context(tc.tile_pool(name="dram", bufs=2, space="DRAM"))

        # ---- persistent SBUF state -------------------------------------
        deg_sb = pers.tile([P, BPC], f32, tag="deg")
        mask_sb = pers.tile([P, BPC], f32, tag="mask")
        psel_sb = pers.tile([P, BPC * G], bf16, tag="psel")
        W_sb = pers.tile([P, L * 2 * H], bf16, tag="W")
        gb_sb = pers.tile([1, 2 * L * H], f32, tag="gb")
        emb0_sb = pers.tile([NF, H], f32, tag="emb0")
        emb1_sb = pers.tile([NF, H], f32, tag="emb1")
        w1_sb = pers.tile([P, 2 * P], f32, tag="w1")
        w2_sb = pers.tile([P, 64], f32, tag="w2")
        w3_sb = pers.tile([64, 1], f32, tag="w3")
        b1_sb = pers.tile([P, 1], f32, tag="b1")
        b2_sb = pers.tile([64, 1], f32, tag="b2")
        b3_sb = pers.tile([1, 1], f32, tag="b3")

        tab_sb = pers.tile([P, NCHUNK * H], bf16, tag="tab")
        h_sb = pers.tile([P, BPC * H], f32, tag="h")
        hsT_sb = pers.tile([P, BPC * 2 * P], bf16, tag="hsT")
        hws_sb = pers.tile([P, BPC * H], bf16, tag="hws")
        t_all = pers.tile([P, BPC * H], f32, tag="t_all")
        nrm_sb = pers.tile([P, BPC], f32, tag="nrm")
        acc_s = pers.tile([P, H], f32, tag="acc_s")
        acc_q = pers.tile([P, H], f32, tag="acc_q")
        D_sb = pers.tile([NF, H], f32, tag="D")
        base_rep = pers.tile([P, H], f32, tag="base_rep")
        a_rep = pers.tile([P, H], f32, tag="a_rep")
        c_rep = pers.tile([P, H], f32, tag="c_rep")
        ident_bf = pers.tile([P, P], bf16, tag="ident")
        ones9 = pers.tile([NF, 1], f32, tag="ones9")
        ones1 = pers.tile([1, P], f32, tag="ones1")
        ones128 = pers.tile([P, 1], f32, tag="ones128")
        ones128b = pers.tile([P, 1], bf16, tag="ones128b")
        stv = pers.tile([1, 2 * H], f32, tag="stv")
        scal = pers.tile([1, 8 * H], f32, tag="scal")

        # ---- DRAM bounce buffers ---------------------------------------
        # AllGather halves: ag_inX[p, :] = hws rows for blocks of that half
        # (5 blocks x 256 = 1280 cols, a contiguous 2.5KB run per partition;
        # ag_outX row c*128+p holds core c's half-run for partition p).
        ag_inA = dram.tile([P, HW], bf16, tag="ag_inA")
        ag_inB = dram.tile([P, HW], bf16, tag="ag_inB")
        ag_outA = dram.tile([NCORE * P, HW], bf16, tag="ag_outA")
        ag_outB = dram.tile([NCORE * P, HW], bf16, tag="ag_outB")
        ar_in = dram.tile([P, 2 * H], f32, tag="ar_in")
        ar_out = dram.tile([P, 2 * H], f32, tag="ar_out")
        pr_in = dram.tile([2 * P + 1, G], f32, tag="pr_in")
        pr_out = dram.tile([2 * P + 1, G], f32, tag="pr_out")

        # ---- input loads ------------------------------------------------
        for t, d in [(deg_sb, d_deg), (mask_sb, d_mask), (psel_sb, d_psel),
                     (W_sb, d_W), (gb_sb, d_gb), (emb0_sb, d_emb0),
                     (emb1_sb, d_emb1), (w1_sb, d_w1), (w2_sb, d_w2),
                     (w3_sb, d_w3), (b1_sb, d_b1), (b2_sb, d_b2),
                     (b3_sb, d_b3)]:
            nc.sync.dma_start(out=t[:], in_=d[:])

        make_identity(nc, ident_bf[:])
        nc.vector.memset(ones9[:], 1.0)
        nc.vector.memset(ones1[:], 1.0)
        nc.vector.memset(ones128[:], 1.0)
        nc.vector.memset(ones128b[:], 1.0)

        # nrm = rsqrt(deg) * mask
        rdeg = work.tile([P, BPC], f32, tag="rdeg")
        nc.vector.reciprocal(out=rdeg[:], in_=deg_sb[:])
        nc.scalar.activation(out=rdeg[:], in_=rdeg[:], func=FT.Sqrt)
        nc.vector.tensor_tensor(out=nrm_sb[:], in0=rdeg[:], in1=mask_sb[:], op=OP.mult)

        # encoder prep: D = emb1 - emb0 ; base = ones9^T @ emb0, broadcast
        nc.vector.tensor_tensor(out=D_sb[:], in0=emb1_sb[:], in1=emb0_sb[:], op=OP.subtract)
        ps_b = psB.tile([1, H], f32, tag="vec")
        nc.tensor.matmul(out=ps_b[:], lhsT=ones9[:], rhs=emb0_sb[:], start=True, stop=True)
        bvec = scal[:, 0:H]
        nc.vector.tensor_copy(out=bvec, in_=ps_b[:])
        ps_br = psB.tile([P, H], f32, tag="vec")
        nc.tensor.matmul(out=ps_br[:], lhsT=ones1[:], rhs=bvec, start=True, stop=True)
        nc.vector.tensor_copy(out=base_rep[:], in_=ps_br[:])

        def hslice(nb):
            return h_sb[:, nb * H:(nb + 1) * H]

        def emit_hs_transpose(nb):
            """hs = h*nrm (bf16), transpose both 128-halves into hsT_sb."""
            hs_bf = work.tile([P, H], bf16, tag="hs_bf")
            nc.vector.tensor_scalar_mul(hs_bf[:], hslice(nb), nrm_sb[:, nb:nb + 1])
            for k in range(2):
                pst = psB.tile([P, P], bf16, tag="pst")
                nc.tensor.transpose(out=pst[:], in_=hs_bf[:, k * P:(k + 1) * P],
                                    identity=ident_bf[:])
                nc.vector.tensor_copy(out=hsT_sb[:, (nb * 2 + k) * P:(nb * 2 + k + 1) * P],
                                      in_=pst[:])

        # encoder: h0 = base + xT^T @ D  (per block)
        for nb in range(BPC):
            xT_t = stream.tile([NF, P], f32, tag="xT_t")
            nc.sync.dma_start(out=xT_t[:], in_=d_xT[:, nb * P:(nb + 1) * P])
            ps_h = psA.tile([P, H], f32, tag="mm")
            nc.tensor.matmul(out=ps_h[:], lhsT=xT_t[:],
                             rhs=D_sb[:], start=True, stop=True)
            nc.vector.tensor_tensor(out=hslice(nb), in0=ps_h[:], in1=base_rep[:], op=OP.add)
            emit_hs_transpose(nb)

        HC = NCHUNK // 2     # chunks per A half-tile

        # ---- layers -----------------------------------------------------
        for l in range(L):
            a_fifo = []

            def a_prefetch(nh):
                a_t = apool.tile([P, HC * P], bf16, tag="A")
                nc.sync.dma_start(out=a_t[:], in_=d_A[:, nh * HC * P:(nh + 1) * HC * P])
                a_fifo.append(a_t)

            def emit_gemm(nb):
                ps_g = psA.tile([P, H], f32, tag="mm")
                for k in range(2):
                    nc.tensor.matmul(
                        out=ps_g[:],
                        lhsT=hsT_sb[:, (nb * 2 + k) * P:(nb * 2 + k + 1) * P],
                        rhs=W_sb[:, (l * 2 + k) * H:(l * 2 + k + 1) * H],
                        start=(k == 0), stop=(k == 1))
                nc.vector.tensor_copy(out=hws_sb[:, nb * H:(nb + 1) * H], in_=ps_g[:])

            # GEMM half A -> AllGather A posts while half B GEMMs run
            for nb in range(0, HB):
                emit_gemm(nb)
            nc.sync.dma_start(out=ag_inA[:], in_=hws_sb[:, 0:HW])
            a_prefetch(0)
            a_prefetch(1)
            nc.gpsimd.collective_compute(
                "AllGather", OP.bypass, replica_groups=rg,
                ins=[ag_inA[:]], outs=[ag_outA[:]])
            for nb in range(HB, BPC):
                emit_gemm(nb)
            nc.sync.dma_start(out=ag_inB[:], in_=hws_sb[:, HW:2 * HW])
            nc.gpsimd.collective_compute(
                "AllGather", OP.bypass, replica_groups=rg,
                ins=[ag_inB[:]], outs=[ag_outB[:]])

            # bulk-load both table halves into SBUF (first-half chunks of
            # every core occupy tab columns [0, 40*H), matching CHUNK_ORDER)
            nc.sync.dma_start(
                out=tab_sb[:, 0:NCHUNK * H // 2].rearrange("p (c w) -> p c w", c=NCORE),
                in_=ag_outA[:].rearrange("(c p) w -> p c w", p=P))
            nc.sync.dma_start(
                out=tab_sb[:, NCHUNK * H // 2:].rearrange("p (c w) -> p c w", c=NCORE),
                in_=ag_outB[:].rearrange("(c p) w -> p c w", p=P))

            nc.vector.memset(acc_s[:], 0.0)
            nc.vector.memset(acc_q[:], 0.0)

            # tab column of CHUNK_ORDER[k]: half = k // 40, core = chunk//10,
            # block-in-half = chunk%10 (mod 5) -> with the half-major tab
            # layout this is simply column k*H. (CHUNK_ORDER groups half-A
            # chunks first; within a half, chunks are ordered (c, nb) which
            # is exactly the ag_outX row-major order.)

            # Two passes over dst blocks: pass 0 consumes only first-half
            # chunks (available right after AllGather A), so all 400 pass-0
            # matmuls run while AllGather B is still in flight; pass 1 adds
            # the second-half chunks via t_all.
            def emit_half_chain(nb, half):
                nh = half * BPC + nb
                if nh + 2 < 2 * BPC:
                    a_prefetch(nh + 2)
                a_t = a_fifo.pop(0)
                ps_t = psA.tile([P, H], f32, tag="mm")
                for cc in range(HC):
                    nc.tensor.matmul(
                        out=ps_t[:],
                        lhsT=a_t[:, cc * P:(cc + 1) * P],
                        rhs=tab_sb[:, (half * HC + cc) * H:(half * HC + cc + 1) * H],
                        start=(cc == 0), stop=(cc == HC - 1))
                tsl = t_all[:, nb * H:(nb + 1) * H]
                if half == 0:
                    nc.vector.tensor_copy(out=tsl, in_=ps_t[:])
                else:
                    # t = nrm*(partA+partB); accumulate BN stats
                    nc.vector.tensor_tensor(out=tsl, in0=tsl, in1=ps_t[:], op=OP.add)
                    nc.vector.tensor_scalar_mul(tsl, tsl, nrm_sb[:, nb:nb + 1])
                    sq = work.tile([P, H], f32, tag="tmp")
                    nc.vector.tensor_tensor(out=sq[:], in0=tsl, in1=tsl, op=OP.mult)
                    nc.vector.tensor_tensor(out=acc_s[:], in0=acc_s[:], in1=tsl, op=OP.add)
                    nc.vector.tensor_tensor(out=acc_q[:], in0=acc_q[:], in1=sq[:], op=OP.add)

            for half in range(2):
                for nb in range(BPC):
                    emit_half_chain(nb, half)

            # stats: cross-partition reduce, replicate to 128 rows, AllReduce
            ps_s = psB.tile([1, 2 * H], f32, tag="vec")
            nc.tensor.matmul(out=ps_s[:, 0:H], lhsT=ones128[:], rhs=acc_s[:],
                             start=True, stop=True)
            nc.tensor.matmul(out=ps_s[:, H:2 * H], lhsT=ones128[:], rhs=acc_q[:],
                             start=True, stop=True)
            st_sb = scal[:, 6 * H:8 * H]
            nc.vector.tensor_copy(out=st_sb, in_=ps_s[:])
            st_rep = work.tile([P, 2 * H], f32, tag="strep")
            for half in range(2):
                ps_r2 = psB.tile([P, H], f32, tag="vec")
                nc.tensor.matmul(out=ps_r2[:], lhsT=ones1[:],
                                 rhs=st_sb[:, half * H:(half + 1) * H],
                                 start=True, stop=True)
                nc.vector.tensor_copy(out=st_rep[:, half * H:(half + 1) * H],
                                      in_=ps_r2[:])
            nc.sync.dma_start(out=ar_in[:], in_=st_rep[:])
            nc.gpsimd.collective_compute(
                "AllReduce", OP.add, replica_groups=rg,
                ins=[ar_in[:]], outs=[ar_out[:]])
            nc.sync.dma_start(out=stv[:], in_=ar_out[0:1, :])

            # a = gamma*istd ; c = beta - mu*a
            mu = scal[:, H:2 * H]
            var = scal[:, 2 * H:3 * H]
            av = scal[:, 3 * H:4 * H]
            cv = scal[:, 4 * H:5 * H]
            msq = scal[:, 5 * H:6 * H]
            nc.vector.tensor_scalar_mul(mu, stv[:, 0:H], 1.0 / N)
            nc.vector.tensor_scalar_mul(var, stv[:, H:2 * H], 1.0 / N)
            nc.vector.tensor_tensor(out=msq, in0=mu, in1=mu, op=OP.mult)
            nc.vector.tensor_tensor(out=var, in0=var, in1=msq, op=OP.subtract)
            nc.vector.tensor_scalar_add(var, var, BN_EPS)
            nc.vector.reciprocal(out=var, in_=var)
            nc.scalar.activation(out=var, in_=var, func=FT.Sqrt)  # istd
            nc.vector.tensor_tensor(out=av, in0=var,
                                    in1=gb_sb[:, l * H:(l + 1) * H], op=OP.mult)
            nc.vector.tensor_tensor(out=msq, in0=mu, in1=av, op=OP.mult)
            nc.vector.tensor_tensor(out=cv, in0=gb_sb[:, (L + l) * H:(L + l + 1) * H],
                                    in1=msq, op=OP.subtract)
            ps_a = psB.tile([P, H], f32, tag="vec")
            nc.tensor.matmul(out=ps_a[:], lhsT=ones1[:], rhs=av, start=True, stop=True)
            nc.vector.tensor_copy(out=a_rep[:], in_=ps_a[:])
            ps_c = psB.tile([P, H], f32, tag="vec")
            nc.tensor.matmul(out=ps_c[:], lhsT=ones1[:], rhs=cv, start=True, stop=True)
            nc.vector.tensor_copy(out=c_rep[:], in_=ps_c[:])

            # h = relu(t*a + c) + h ; prepare hsT for next layer
            for nb in range(BPC):
                tsl = t_all[:, nb * H:(nb + 1) * H]
                u = work.tile([P, H], f32, tag="tmp")
                nc.vector.tensor_tensor(out=u[:], in0=tsl, in1=a_rep[:], op=OP.mult)
                nc.vector.tensor_tensor(out=u[:], in0=u[:], in1=c_rep[:], op=OP.add)
                r = work.tile([P, H], f32, tag="tmp2")
                nc.scalar.activation(out=r[:], in_=u[:], func=FT.Relu)
                nc.vector.tensor_tensor(out=hslice(nb), in0=hslice(nb), in1=r[:], op=OP.add)
                if l < L - 1:
                    emit_hs_transpose(nb)

        # ---- pooling (bf16 matmuls; psel preloaded) ---------------------
        ps_p0 = psC.tile([P, G], f32, tag="p0")
        ps_p1 = psC.tile([P, G], f32, tag="p1")
        ps_pc = psB.tile([1, G], f32, tag="vec")
        for nb in range(BPC):
            hb_t = work.tile([P, H], bf16, tag="hb")
            nc.vector.tensor_copy(out=hb_t[:], in_=hslice(nb))
            pssl = psel_sb[:, nb * G:(nb + 1) * G]
            nc.tensor.matmul(out=ps_p0[:], lhsT=hb_t[:, 0:P],
                             rhs=pssl, start=(nb == 0), stop=(nb == BPC - 1))
            nc.tensor.matmul(out=ps_p1[:], lhsT=hb_t[:, P:2 * P],
                             rhs=pssl, start=(nb == 0), stop=(nb == BPC - 1))
            nc.tensor.matmul(out=ps_pc[:], lhsT=ones128b[:],
                             rhs=pssl, start=(nb == 0), stop=(nb == BPC - 1))
        g0 = work.tile([P, G], f32, tag="g0")
        g1 = work.tile([P, G], f32, tag="g1")
        cnt = scal[:, 0:G]
        nc.vector.tensor_copy(out=g0[:], in_=ps_p0[:])
        nc.vector.tensor_copy(out=g1[:], in_=ps_p1[:])
        nc.vector.tensor_copy(out=cnt, in_=ps_pc[:])
        nc.sync.dma_start(out=pr_in[0:P, :], in_=g0[:])
        nc.sync.dma_start(out=pr_in[P:2 * P, :], in_=g1[:])
        nc.sync.dma_start(out=pr_in[2 * P:2 * P + 1, :], in_=cnt)
        nc.gpsimd.collective_compute(
            "AllReduce", OP.add, replica_groups=rg,
            ins=[pr_in[:]], outs=[pr_out[:]])
        nc.sync.dma_start(out=g0[:], in_=pr_out[0:P, :])
        nc.sync.dma_start(out=g1[:], in_=pr_out[P:2 * P, :])
        nc.sync.dma_start(out=cnt, in_=pr_out[2 * P:2 * P + 1, :])
        nc.vector.tensor_scalar_max(cnt, cnt, 1.0)
        nc.vector.reciprocal(out=cnt, in_=cnt)
        ps_r = psB.tile([P, G], f32, tag="vec")
        nc.tensor.matmul(out=ps_r[:], lhsT=ones1[:], rhs=cnt, start=True, stop=True)
        rc_rep = work.tile([P, G], f32, tag="rc_rep")
        nc.vector.tensor_copy(out=rc_rep[:], in_=ps_r[:])
        nc.vector.tensor_tensor(out=g0[:], in0=g0[:], in1=rc_rep[:], op=OP.mult)
        nc.vector.tensor_tensor(out=g1[:], in0=g1[:], in1=rc_rep[:], op=OP.mult)

        # MLP head (transposed: weights are lhsT, graphs along free dim)
        ps1 = psB.tile([P, G], f32, tag="vec")
        nc.tensor.matmul(out=ps1[:], lhsT=w1_sb[:, 0:P], rhs=g0[:], start=True, stop=False)
        nc.tensor.matmul(out=ps1[:], lhsT=w1_sb[:, P:2 * P], rhs=g1[:], start=False, stop=True)
        y1 = work.tile([P, G], f32, tag="y1")
        nc.scalar.activation(out=y1[:], in_=ps1[:], func=FT.Relu, bias=b1_sb[:, 0:1])
        ps2 = psB.tile([64, G], f32, tag="vec")
        nc.tensor.matmul(out=ps2[:], lhsT=w2_sb[:], rhs=y1[:], start=True, stop=True)
        y2 = work.tile([64, G], f32, tag="y2")
        nc.scalar.activation(out=y2[:], in_=ps2[:], func=FT.Relu, bias=b2_sb[:, 0:1])
        ps3 = psB.tile([1, G], f32, tag="vec")
        nc.tensor.matmul(out=ps3[:], lhsT=w3_sb[:], rhs=y2[:], start=True, stop=True)
        y3 = work.tile([1, G], f32, tag="y3")
        nc.vector.tensor_scalar_add(y3[:], ps3[:], b3_sb[0:1, 0:1])
        nc.sync.dma_start(out=d_out[:], in_=y3[:])

    nc.compile()
    return nc


# --------------------------------------------------------------------------
# entry point
# --------------------------------------------------------------------------

def kernel(x, edge_index, batch_ids, emb, W, b, gamma, beta,
           mlp_W1, mlp_b1, mlp_W2, mlp_b2, mlp_W3, mlp_b3,
           _trace=False, _trace_kwargs=None):
    # NB: reference BN subtracts the per-channel mean, so the additive bias b
    # cancels exactly and is not needed by the device program.
    in_maps = _preprocess(x, edge_index, batch_ids, emb, W, gamma, beta,
                          mlp_W1, mlp_b1, mlp_W2, mlp_b2, mlp_W3, mlp_b3)
    if "nc" not in _compiled:
        _compiled["nc"] = _build()
    nc = _compiled["nc"]
    kw = {}
    if _trace:
        kw = dict(trace=True, **(_trace_kwargs or {}))
    res = run_bass_kernel_spmd(nc, in_maps, core_ids=list(range(NCORE)), **kw)
    out = np.asarray(res.results[0]["out"], np.float32).reshape(G, 1)
    kernel._last_results = res
    return out


# revision 25
# speedup vs baseline: 2.4212x; 1.0107x over previous
"""Trainium2 Bass kernel for HIVNet GCN message passing (8-core SPMD).

v6 strategy (baseline 2.29ms -> v5 hybrid 1.48ms -> v6 pure-dense):
  - Pad N=10000 nodes to 10240 = 80 chunks x 128; core c owns 10 dst-blocks
    (global chunks c*10..c*10+9).
  - Per layer: hws = (h*nrm) @ W[l] on the owned shard (bf16), AllGather the
    partition-major table (row c*128+p holds core c's 10 chunks for
    partition p as one contiguous 5KB run), bulk-load into SBUF.
  - Aggregation is PURE dense one-hot adjacency on TensorE: per dst-block an
    80-chunk PSUM-accumulated matmul chain (lhsT = A chunk [128 src x 128
    dst] with edge-multiplicity counts, rhs = table chunk [128 src x 256]).
    A (26MB/core) is streamed from HBM in half-block tiles, software
    pipelined 2 halves ahead; zero GPSIMD, zero scattered DMA (SWDGE
    gathers at 8ns/row descriptor cost were the v1 bottleneck, and their
    512B scattered packets degraded the whole DMA subsystem).
  - The AllGather is split in two half-payload collectives: the second half
    transfers while TensorE chews the first half's chunks (chain order is
    free under PSUM accumulation).
  - BatchNorm: partial sums/sumsq -> stats replicated to 128 rows so the
    2KB AllReduce becomes a 256KB RDH AllReduce (Mesh at 2KB costs 79us);
    scale/shift broadcast via rank-1 TensorE matmul; relu+residual on DVE.
  - Readout: graph mean-pool via preloaded bf16 one-hot pool matrices,
    257-row AllReduce, 3-layer MLP computed redundantly on every core.
"""

import os
import sys

sys.path.insert(0, "/opt/trn_rl_repo")

from contextlib import ExitStack

import numpy as np
import ml_dtypes

from concourse import bass, mybir, bacc, tile, library_config
from concourse.bass_utils import run_bass_kernel_spmd
from concourse.masks import make_identity

NCORE = 8
P = 128
H = 256
L = 4
NF = 9
G = 256
N = 10000
BPC = 10                # dst blocks per core
NPC = BPC * P           # 1280 nodes per core
NPAD = NCORE * NPC      # 10240
NCHUNK = NPAD // P      # 80 src chunks
HB = BPC // 2           # blocks per AllGather half
BN_EPS = 1e-5

f32 = mybir.dt.float32
bf16 = mybir.dt.bfloat16
bfnp = ml_dtypes.bfloat16

FT = mybir.ActivationFunctionType
OP = mybir.AluOpType

_compiled = {}

# chunk consumption order: first-half chunks (block%10 < 5) first, so the
# dense chains can start right after AllGather half A lands
CHUNK_ORDER = [g for g in range(NCHUNK) if g % BPC < HB] + \
              [g for g in range(NCHUNK) if g % BPC >= HB]


# --------------------------------------------------------------------------
# host-side structural preprocessing
# --------------------------------------------------------------------------

def _preprocess(x, edge_index, batch_ids, emb, W, gamma, beta,
                mlp_W1, mlp_b1, mlp_W2, mlp_b2, mlp_W3, mlp_b3):
    src = np.asarray(edge_index[0], np.int64)
    dst = np.asarray(edge_index[1], np.int64)
    # self loops for every real node (weight nrm[d]^2 folds in)
    src_all = np.concatenate([src, np.arange(N, dtype=np.int64)])
    dst_all = np.concatenate([dst, np.arange(N, dtype=np.int64)])
    order = np.argsort(dst_all, kind="stable")
    s_sorted = src_all[order]
    d_sorted = dst_all[order]

    deg = np.bincount(dst_all, minlength=NPAD).astype(np.float64)  # incl self

    nblk = NCORE * BPC
    starts = np.searchsorted(d_sorted, np.arange(nblk) * P)
    ends = np.searchsorted(d_sorted, (np.arange(nblk) + 1) * P)

    # dense adjacency per dst block, chunk-major in CHUNK_ORDER
    A_blocks = {}
    for g in range(nblk):
        c, nb = divmod(g, BPC)
        e_s = s_sorted[starts[g]:ends[g]]
        e_d = d_sorted[starts[g]:ends[g]] - g * P
        A = np.zeros((NPAD, P), np.float32)
        np.add.at(A, (e_s, e_d), 1.0)
        A = A.reshape(NCHUNK, P, P)[CHUNK_ORDER]          # reorder chunks
        A_blocks[(c, nb)] = np.ascontiguousarray(
            A.transpose(1, 0, 2).reshape(P, NCHUNK * P)).astype(bfnp)

    # graph pool one-hot [node, graph] (bf16: values 0/1 exact)
    bids = np.asarray(batch_ids, np.int64)
    psel_full = np.zeros((NPAD, G), np.float32)
    psel_full[np.arange(N), bids] = 1.0

    x_np = np.zeros((NPAD, NF), np.float32)
    x_np[:N] = np.asarray(x, np.float64)

    Wf = np.asarray(W, np.float32)
    W_lhsT = Wf.reshape(L, 2, P, H).transpose(2, 0, 1, 3).reshape(P, L * 2 * H)
    gb = np.concatenate([np.asarray(gamma, np.float32).reshape(-1),
                         np.asarray(beta, np.float32).reshape(-1)])[None, :]
    embf = np.asarray(emb, np.float32)
    emb0 = np.ascontiguousarray(embf[:, 0, :])
    emb1 = np.ascontiguousarray(embf[:, 1, :])
    w1 = np.asarray(mlp_W1, np.float32).reshape(2, P, P).transpose(1, 0, 2).reshape(P, 2 * P)
    w2 = np.asarray(mlp_W2, np.float32)
    w3 = np.asarray(mlp_W3, np.float32)
    b1 = np.asarray(mlp_b1, np.float32).reshape(P, 1)
    b2 = np.asarray(mlp_b2, np.float32).reshape(64, 1)
    b3 = np.asarray(mlp_b3, np.float32).reshape(1, 1)

    in_maps = []
    HC = NCHUNK // 2
    for c in range(NCORE):
        lo, hi = c * NPC, (c + 1) * NPC
        Ac = np.concatenate([A_blocks[(c, nb)] for nb in range(BPC)], axis=1)
        # reorder half-tiles to match the two-pass consumption order:
        # tile nh = half*BPC + nb  (all first halves, then all second halves)
        Ac = np.ascontiguousarray(
            Ac.reshape(P, BPC, 2, HC * P).transpose(0, 2, 1, 3)
        ).reshape(P, BPC * NCHUNK * P)

        degc = deg[lo:hi].reshape(BPC, P).T
        maskc = (degc > 0).astype(np.float32)
        degc = np.maximum(degc, 1.0).astype(np.float32)

        pselc = psel_full[lo:hi].reshape(BPC, P, G)
        pselc = np.ascontiguousarray(pselc.transpose(1, 0, 2)).reshape(P, BPC * G)

        in_maps.append(dict(
            A=Ac, xT=np.ascontiguousarray(x_np[lo:hi].T),
            deg=degc, mask=maskc, psel=pselc.astype(bfnp),
            W=W_lhsT.astype(bfnp), gb=gb, emb0=emb0, emb1=emb1,
            w1=w1, w2=w2, w3=w3, b1=b1, b2=b2, b3=b3,
        ))
    return in_maps


# --------------------------------------------------------------------------
# device program
# --------------------------------------------------------------------------

def _build():
    nc = bacc.Bacc(None, target_bir_lowering=False)

    d_A = nc.dram_tensor("A", [P, BPC * NCHUNK * P], bf16, kind="ExternalInput")
    d_xT = nc.dram_tensor("xT", [NF, NPC], f32, kind="ExternalInput")
    d_deg = nc.dram_tensor("deg", [P, BPC], f32, kind="ExternalInput")
    d_mask = nc.dram_tensor("mask", [P, BPC], f32, kind="ExternalInput")
    d_psel = nc.dram_tensor("psel", [P, BPC * G], bf16, kind="ExternalInput")
    d_W = nc.dram_tensor("W", [P, L * 2 * H], bf16, kind="ExternalInput")
    d_gb = nc.dram_tensor("gb", [1, 2 * L * H], f32, kind="ExternalInput")
    d_emb0 = nc.dram_tensor("emb0", [NF, H], f32, kind="ExternalInput")
    d_emb1 = nc.dram_tensor("emb1", [NF, H], f32, kind="ExternalInput")
    d_w1 = nc.dram_tensor("w1", [P, 2 * P], f32, kind="ExternalInput")
    d_w2 = nc.dram_tensor("w2", [P, 64], f32, kind="ExternalInput")
    d_w3 = nc.dram_tensor("w3", [64, 1], f32, kind="ExternalInput")
    d_b1 = nc.dram_tensor("b1", [P, 1], f32, kind="ExternalInput")
    d_b2 = nc.dram_tensor("b2", [64, 1], f32, kind="ExternalInput")
    d_b3 = nc.dram_tensor("b3", [1, 1], f32, kind="ExternalInput")
    d_out = nc.dram_tensor("out", [1, G], f32, kind="ExternalOutput")

    rg = [list(range(NCORE))]
    HW = HB * H          # half payload width per partition (1280 cols)

    with tile.TileContext(nc) as tc, ExitStack() as ctx:
        pers = ctx.enter_context(tc.tile_pool(name="pers", bufs=1))
        psA = ctx.enter_context(tc.tile_pool(name="psA", bufs=2, space="PSUM"))
        psB = ctx.enter_context(tc.tile_pool(name="psB", bufs=2, space="PSUM"))
        psC = ctx.enter_context(tc.tile_pool(name="psC", bufs=1, space="PSUM"))
        apool = ctx.enter_context(tc.tile_pool(name="apool", bufs=3))
        work = ctx.enter_context(tc.tile_pool(name="work", bufs=2))
        stream = ctx.enter_context(tc.tile_pool(name="stream", bufs=2))
        dram = ctx.enter_context(tc.tile_pool(name="dram", bufs=2, space="DRAM"))

        # ---- persistent SBUF state -------------------------------------
        deg_sb = pers.tile([P, BPC], f32, tag="deg")
        mask_sb = pers.tile([P, BPC], f32, tag="mask")
        psel_sb = pers.tile([P, BPC * G], bf16, tag="psel")
        W_sb = pers.tile([P, L * 2 * H], bf16, tag="W")
        gb_sb = pers.tile([1, 2 * L * H], f32, tag="gb")
        emb0_sb = pers.tile([NF, H], f32, tag="emb0")
        emb1_sb = pers.tile([NF, H], f32, tag="emb1")
        w1_sb = pers.tile([P, 2 * P], f32, tag="w1")
        w2_sb = pers.tile([P, 64], f32, tag="w2")
        w3_sb = pers.tile([64, 1], f32, tag="w3")
        b1_sb = pers.tile([P, 1], f32, tag="b1")
        b2_sb = pers.tile([64, 1], f32, tag="b2")
        b3_sb = pers.tile([1, 1], f32, tag="b3")

        tab_sb = pers.tile([P, NCHUNK * H], bf16, tag="tab")
        h_sb = pers.tile([P, BPC * H], f32, tag="h")
        hsT_sb = pers.tile([P, BPC * 2 * P], bf16, tag="hsT")
        hws_sb = pers.tile([P, BPC * H], bf16, tag="hws")
        t_all = pers.tile([P, BPC * H], f32, tag="t_all")
        nrm_sb = pers.tile([P, BPC], f32, tag="nrm")
        acc_s = pers.tile([P, H], f32, tag="acc_s")
        acc_q = pers.tile([P, H], f32, tag="acc_q")
        D_sb = pers.tile([NF, H], f32, tag="D")
        base_rep = pers.tile([P, H], f32, tag="base_rep")
        a_rep = pers.tile([P, H], f32, tag="a_rep")
        c_rep = pers.tile([P, H], f32, tag="c_rep")
        ident_bf = pers.tile([P, P], bf16, tag="ident")
        ones9 = pers.tile([NF, 1], f32, tag="ones9")
        ones1 = pers.tile([1, P], f32, tag="ones1")
        ones128 = pers.tile([P, 1], f32, tag="ones128")
        ones128b = pers.tile([P, 1], bf16, tag="ones128b")
        stv = pers.tile([1, 2 * H], f32, tag="stv")
        scal = pers.tile([1, 8 * H], f32, tag="scal")

        # ---- DRAM bounce buffers ---------------------------------------
        # AllGather halves: ag_inX[p, :] = hws rows for blocks of that half
        # (5 blocks x 256 = 1280 cols, a contiguous 2.5KB run per partition;
        # ag_outX row c*128+p holds core c's half-run for partition p).
        ag_inA = dram.tile([P, HW], bf16, tag="ag_inA")
        ag_inB = dram.tile([P, HW], bf16, tag="ag_inB")
        ag_outA = dram.tile([NCORE * P, HW], bf16, tag="ag_outA")
        ag_outB = dram.tile([NCORE * P, HW], bf16, tag="ag_outB")
        ar_in = dram.tile([P, 2 * H], f32, tag="ar_in")
        ar_out = dram.tile([P, 2 * H], f32, tag="ar_out")
        pr_in = dram.tile([2 * P + 1, G], f32, tag="pr_in")
        pr_out = dram.tile([2 * P + 1, G], f32, tag="pr_out")

        # ---- input loads ------------------------------------------------
        for t, d in [(deg_sb, d_deg), (mask_sb, d_mask), (psel_sb, d_psel),
                     (W_sb, d_W), (gb_sb, d_gb), (emb0_sb, d_emb0),
                     (emb1_sb, d_emb1), (w1_sb, d_w1), (w2_sb, d_w2),
                     (w3_sb, d_w3), (b1_sb, d_b1), (b2_sb, d_b2),
                     (b3_sb, d_b3)]:
            nc.sync.dma_start(out=t[:], in_=d[:])

        make_identity(nc, ident_bf[:])
        nc.vector.memset(ones9[:], 1.0)
        nc.vector.memset(ones1[:], 1.0)
        nc.vector.memset(ones128[:], 1.0)
        nc.vector.memset(ones128b[:], 1.0)

        # nrm = rsqrt(deg) * mask
        rdeg = work.tile([P, BPC], f32, tag="rdeg")
        nc.vector.reciprocal(out=rdeg[:], in_=deg_sb[:])
        nc.scalar.activation(out=rdeg[:], in_=rdeg[:], func=FT.Sqrt)
        nc.vector.tensor_tensor(out=nrm_sb[:], in0=rdeg[:], in1=mask_sb[:], op=OP.mult)

        # encoder prep: D = emb1 - emb0 ; base = ones9^T @ emb0, broadcast
        nc.vector.tensor_tensor(out=D_sb[:], in0=emb1_sb[:], in1=emb0_sb[:], op=OP.subtract)
        ps_b = psB.tile([1, H], f32, tag="vec")
        nc.tensor.matmul(out=ps_b[:], lhsT=ones9[:], rhs=emb0_sb[:], start=True, stop=True)
        bvec = scal[:, 0:H]
        nc.vector.tensor_copy(out=bvec, in_=ps_b[:])
        ps_br = psB.tile([P, H], f32, tag="vec")
        nc.tensor.matmul(out=ps_br[:], lhsT=ones1[:], rhs=bvec, start=True, stop=True)
        nc.vector.tensor_copy(out=base_rep[:], in_=ps_br[:])

        def hslice(nb):
            return h_sb[:, nb * H:(nb + 1) * H]

        def emit_hs_transpose(nb):
            """hs = h*nrm (bf16), transpose both 128-halves into hsT_sb."""
            hs_bf = work.tile([P, H], bf16, tag="hs_bf")
            nc.vector.tensor_scalar_mul(hs_bf[:], hslice(nb), nrm_sb[:, nb:nb + 1])
            for k in range(2):
                pst = psB.tile([P, P], bf16, tag="pst")
                nc.tensor.transpose(out=pst[:], in_=hs_bf[:, k * P:(k + 1) * P],
                                    identity=ident_bf[:])
                nc.vector.tensor_copy(out=hsT_sb[:, (nb * 2 + k) * P:(nb * 2 + k + 1) * P],
                                      in_=pst[:])

        # encoder: h0 = base + xT^T @ D  (per block)
        for nb in range(BPC):
            xT_t = stream.tile([NF, P], f32, tag="xT_t")
            nc.sync.dma_start(out=xT_t[:], in_=d_xT[:, nb * P:(nb + 1) * P])
            ps_h = psA.tile([P, H], f32, tag="mm")
            nc.tensor.matmul(out=ps_h[:], lhsT=xT_t[:],
                             rhs=D_sb[:], start=True, stop=True)
            nc.vector.tensor_tensor(out=hslice(nb), in0=ps_h[:], in1=base_rep[:], op=OP.add)
            emit_hs_transpose(nb)

        HC = NCHUNK // 2     # chunks per A half-tile
        a_fifo = []

        def a_prefetch(nh):
            a_t = apool.tile([P, HC * P], bf16, tag="A")
            nc.sync.dma_start(out=a_t[:], in_=d_A[:, nh * HC * P:(nh + 1) * HC * P])
            a_fifo.append(a_t)

        def emit_gemm(l, nb):
            ps_g = psA.tile([P, H], f32, tag="mm")
            for k in range(2):
                nc.tensor.matmul(
                    out=ps_g[:],
                    lhsT=hsT_sb[:, (nb * 2 + k) * P:(nb * 2 + k + 1) * P],
                    rhs=W_sb[:, (l * 2 + k) * H:(l * 2 + k + 1) * H],
                    start=(k == 0), stop=(k == 1))
            nc.vector.tensor_copy(out=hws_sb[:, nb * H:(nb + 1) * H], in_=ps_g[:])

        def emit_ag_half(half):
            if half == 0:
                nc.sync.dma_start(out=ag_inA[:], in_=hws_sb[:, 0:HW])
                nc.gpsimd.collective_compute(
                    "AllGather", OP.bypass, replica_groups=rg,
                    ins=[ag_inA[:]], outs=[ag_outA[:]])
            else:
                nc.sync.dma_start(out=ag_inB[:], in_=hws_sb[:, HW:2 * HW])
                nc.gpsimd.collective_compute(
                    "AllGather", OP.bypass, replica_groups=rg,
                    ins=[ag_inB[:]], outs=[ag_outB[:]])

        # Two passes over dst blocks: pass 0 consumes only first-half chunks
        # (available right after AllGather A), so all 400 pass-0 matmuls run
        # while AllGather B is in flight; pass 1 adds second-half chunks.
        def emit_half_chain(nb, half):
            nh = half * BPC + nb
            if nh + 2 < 2 * BPC:
                a_prefetch(nh + 2)
            a_t = a_fifo.pop(0)
            ps_t = psA.tile([P, H], f32, tag="mm")
            for cc in range(HC):
                nc.tensor.matmul(
                    out=ps_t[:],
                    lhsT=a_t[:, cc * P:(cc + 1) * P],
                    rhs=tab_sb[:, (half * HC + cc) * H:(half * HC + cc + 1) * H],
                    start=(cc == 0), stop=(cc == HC - 1))
            tsl = t_all[:, nb * H:(nb + 1) * H]
            if half == 0:
                nc.vector.tensor_copy(out=tsl, in_=ps_t[:])
            else:
                # t = nrm*(partA+partB); accumulate BN stats
                nc.vector.tensor_tensor(out=tsl, in0=tsl, in1=ps_t[:], op=OP.add)
                nc.vector.tensor_scalar_mul(tsl, tsl, nrm_sb[:, nb:nb + 1])
                sq = work.tile([P, H], f32, tag="tmp")
                nc.vector.tensor_tensor(out=sq[:], in0=tsl, in1=tsl, op=OP.mult)
                nc.vector.tensor_tensor(out=acc_s[:], in0=acc_s[:], in1=tsl, op=OP.add)
                nc.vector.tensor_tensor(out=acc_q[:], in0=acc_q[:], in1=sq[:], op=OP.add)

        # bootstrap layer 0's GEMM + AllGather halves (from encoder hsT)
        for nb in range(0, HB):
            emit_gemm(0, nb)
        emit_ag_half(0)
        a_prefetch(0)
        a_prefetch(1)
        for nb in range(HB, BPC):
            emit_gemm(0, nb)
        emit_ag_half(1)

        # ---- layers -----------------------------------------------------
        for l in range(L):
            # table half A -> pass-0 chains (tab half B load is emitted
            # AFTER the pass-0 prefetches so it doesn't block the Sync FIFO)
            nc.sync.dma_start(
                out=tab_sb[:, 0:NCHUNK * H // 2].rearrange("p (c w) -> p c w", c=NCORE),
                in_=ag_outA[:].rearrange("(c p) w -> p c w", p=P))
            nc.vector.memset(acc_s[:], 0.0)
            nc.vector.memset(acc_q[:], 0.0)
            for nb in range(BPC):
                emit_half_chain(nb, 0)
            nc.sync.dma_start(
                out=tab_sb[:, NCHUNK * H // 2:].rearrange("p (c w) -> p c w", c=NCORE),
                in_=ag_outB[:].rearrange("(c p) w -> p c w", p=P))
            for nb in range(BPC):
                emit_half_chain(nb, 1)

            # stats: cross-partition reduce, replicate to 128 rows, AllReduce
            ps_s = psB.tile([1, 2 * H], f32, tag="vec")
            nc.tensor.matmul(out=ps_s[:, 0:H], lhsT=ones128[:], rhs=acc_s[:],
                             start=True, stop=True)
            nc.tensor.matmul(out=ps_s[:, H:2 * H], lhsT=ones128[:], rhs=acc_q[:],
                             start=True, stop=True)
            st_sb = scal[:, 6 * H:8 * H]
            nc.vector.tensor_copy(out=st_sb, in_=ps_s[:])
            st_rep = work.tile([P, 2 * H], f32, tag="strep")
            for half in range(2):
                ps_r2 = psB.tile([P, H], f32, tag="vec")
                nc.tensor.matmul(out=ps_r2[:], lhsT=ones1[:],
                                 rhs=st_sb[:, half * H:(half + 1) * H],
                                 start=True, stop=True)
                nc.vector.tensor_copy(out=st_rep[:, half * H:(half + 1) * H],
                                      in_=ps_r2[:])
            nc.sync.dma_start(out=ar_in[:], in_=st_rep[:])
            nc.gpsimd.collective_compute(
                "AllReduce", OP.add, replica_groups=rg,
                ins=[ar_in[:]], outs=[ar_out[:]])
            nc.sync.dma_start(out=stv[:], in_=ar_out[0:1, :])

            # a = gamma*istd ; c = beta - mu*a
            mu = scal[:, H:2 * H]
            var = scal[:, 2 * H:3 * H]
            av = scal[:, 3 * H:4 * H]
            cv = scal[:, 4 * H:5 * H]
            msq = scal[:, 5 * H:6 * H]
            nc.vector.tensor_scalar_mul(mu, stv[:, 0:H], 1.0 / N)
            nc.vector.tensor_scalar_mul(var, stv[:, H:2 * H], 1.0 / N)
            nc.vector.tensor_tensor(out=msq, in0=mu, in1=mu, op=OP.mult)
            nc.vector.tensor_tensor(out=var, in0=var, in1=msq, op=OP.subtract)
            nc.vector.tensor_scalar_add(var, var, BN_EPS)
            nc.vector.reciprocal(out=var, in_=var)
            nc.scalar.activation(out=var, in_=var, func=FT.Sqrt)  # istd
            nc.vector.tensor_tensor(out=av, in0=var,
                                    in1=gb_sb[:, l * H:(l + 1) * H], op=OP.mult)
            nc.vector.tensor_tensor(out=msq, in0=mu, in1=av, op=OP.mult)
            nc.vector.tensor_tensor(out=cv, in0=gb_sb[:, (L + l) * H:(L + l + 1) * H],
                                    in1=msq, op=OP.subtract)
            ps_a = psB.tile([P, H], f32, tag="vec")
            nc.tensor.matmul(out=ps_a[:], lhsT=ones1[:], rhs=av, start=True, stop=True)
            nc.vector.tensor_copy(out=a_rep[:], in_=ps_a[:])
            ps_c = psB.tile([P, H], f32, tag="vec")
            nc.tensor.matmul(out=ps_c[:], lhsT=ones1[:], rhs=cv, start=True, stop=True)
            nc.vector.tensor_copy(out=c_rep[:], in_=ps_c[:])

            # h = relu(t*a + c) + h ; immediately GEMM the updated block for
            # the next layer and post the AllGather halves as soon as each
            # half's blocks are done.
            for nb in range(BPC):
                tsl = t_all[:, nb * H:(nb + 1) * H]
                u = work.tile([P, H], f32, tag="tmp")
                nc.vector.tensor_tensor(out=u[:], in0=tsl, in1=a_rep[:], op=OP.mult)
                nc.vector.tensor_tensor(out=u[:], in0=u[:], in1=c_rep[:], op=OP.add)
                r = work.tile([P, H], f32, tag="tmp2")
                nc.scalar.activation(out=r[:], in_=u[:], func=FT.Relu)
                nc.vector.tensor_tensor(out=hslice(nb), in0=hslice(nb), in1=r[:], op=OP.add)
                if l < L - 1:
                    emit_hs_transpose(nb)
                    emit_gemm(l + 1, nb)
                    if nb == HB - 1:
                        emit_ag_half(0)
                        a_prefetch(0)
                        a_prefetch(1)
                    elif nb == BPC - 1:
                        emit_ag_half(1)

        # ---- pooling (bf16 matmuls; psel preloaded) ---------------------
        ps_p0 = psC.tile([P, G], f32, tag="p0")
        ps_p1 = psC.tile([P, G], f32, tag="p1")
        ps_pc = psB.tile([1, G], f32, tag="vec")
        for nb in range(BPC):
            hb_t = work.tile([P, H], bf16, tag="hb")
            nc.vector.tensor_copy(out=hb_t[:], in_=hslice(nb))
            pssl = psel_sb[:, nb * G:(nb + 1) * G]
            nc.tensor.matmul(out=ps_p0[:], lhsT=hb_t[:, 0:P],
                             rhs=pssl, start=(nb == 0), stop=(nb == BPC - 1))
            nc.tensor.matmul(out=ps_p1[:], lhsT=hb_t[:, P:2 * P],
                             rhs=pssl, start=(nb == 0), stop=(nb == BPC - 1))
            nc.tensor.matmul(out=ps_pc[:], lhsT=ones128b[:],
                             rhs=pssl, start=(nb == 0), stop=(nb == BPC - 1))
        g0 = work.tile([P, G], f32, tag="g0")
        g1 = work.tile([P, G], f32, tag="g1")
        cnt = scal[:, 0:G]
        nc.vector.tensor_copy(out=g0[:], in_=ps_p0[:])
        nc.vector.tensor_copy(out=g1[:], in_=ps_p1[:])
        nc.vector.tensor_copy(out=cnt, in_=ps_pc[:])
        nc.sync.dma_start(out=pr_in[0:P, :], in_=g0[:])
        nc.sync.dma_start(out=pr_in[P:2 * P, :], in_=g1[:])
        nc.sync.dma_start(out=pr_in[2 * P:2 * P + 1, :], in_=cnt)
        nc.gpsimd.collective_compute(
            "AllReduce", OP.add, replica_groups=rg,
            ins=[pr_in[:]], outs=[pr_out[:]])
        nc.sync.dma_start(out=g0[:], in_=pr_out[0:P, :])
        nc.sync.dma_start(out=g1[:], in_=pr_out[P:2 * P, :])
        nc.sync.dma_start(out=cnt, in_=pr_out[2 * P:2 * P + 1, :])
        nc.vector.tensor_scalar_max(cnt, cnt, 1.0)
        nc.vector.reciprocal(out=cnt, in_=cnt)
        ps_r = psB.tile([P, G], f32, tag="vec")
        nc.tensor.matmul(out=ps_r[:], lhsT=ones1[:], rhs=cnt, start=True, stop=True)
        rc_rep = work.tile([P, G], f32, tag="rc_rep")
        nc.vector.tensor_copy(out=rc_rep[:], in_=ps_r[:])
        nc.vector.tensor_tensor(out=g0[:], in0=g0[:], in1=rc_rep[:], op=OP.mult)
        nc.vector.tensor_tensor(out=g1[:], in0=g1[:], in1=rc_rep[:], op=OP.mult)

        # MLP head (transposed: weights are lhsT, graphs along free dim)
        ps1 = psB.tile([P, G], f32, tag="vec")
        nc.tensor.matmul(out=ps1[:], lhsT=w1_sb[:, 0:P], rhs=g0[:], start=True, stop=False)
        nc.tensor.matmul(out=ps1[:], lhsT=w1_sb[:, P:2 * P], rhs=g1[:], start=False, stop=True)
        y1 = work.tile([P, G], f32, tag="y1")
        nc.scalar.activation(out=y1[:], in_=ps1[:], func=FT.Relu, bias=b1_sb[:, 0:1])
        ps2 = psB.tile([64, G], f32, tag="vec")
        nc.tensor.matmul(out=ps2[:], lhsT=w2_sb[:], rhs=y1[:], start=True, stop=True)
        y2 = work.tile([64, G], f32, tag="y2")
        nc.scalar.activation(out=y2[:], in_=ps2[:], func=FT.Relu, bias=b2_sb[:, 0:1])
        ps3 = psB.tile([1, G], f32, tag="vec")
        nc.tensor.matmul(out=ps3[:], lhsT=w3_sb[:], rhs=y2[:], start=True, stop=True)
        y3 = work.tile([1, G], f32, tag="y3")
        nc.vector.tensor_scalar_add(y3[:], ps3[:], b3_sb[0:1, 0:1])
        nc.sync.dma_start(out=d_out[:], in_=y3[:])

    nc.compile()
    return nc


# --------------------------------------------------------------------------
# entry point
# --------------------------------------------------------------------------

def kernel(x, edge_index, batch_ids, emb, W, b, gamma, beta,
           mlp_W1, mlp_b1, mlp_W2, mlp_b2, mlp_W3, mlp_b3,
           _trace=False, _trace_kwargs=None):
    # NB: reference BN subtracts the per-channel mean, so the additive bias b
    # cancels exactly and is not needed by the device program.
    in_maps = _preprocess(x, edge_index, batch_ids, emb, W, gamma, beta,
                          mlp_W1, mlp_b1, mlp_W2, mlp_b2, mlp_W3, mlp_b3)
    if "nc" not in _compiled:
        _compiled["nc"] = _build()
    nc = _compiled["nc"]
    kw = {}
    if _trace:
        kw = dict(trace=True, **(_trace_kwargs or {}))
    res = run_bass_kernel_spmd(nc, in_maps, core_ids=list(range(NCORE)), **kw)
    out = np.asarray(res.results[0]["out"], np.float32).reshape(G, 1)
    kernel._last_results = res
    return out


# revision 27
# speedup vs baseline: 2.5430x; 1.0503x over previous
"""Trainium2 Bass kernel for HIVNet GCN message passing (8-core SPMD).

v6 strategy (baseline 2.29ms -> v5 hybrid 1.48ms -> v6 pure-dense):
  - Pad N=10000 nodes to 10240 = 80 chunks x 128; core c owns 10 dst-blocks
    (global chunks c*10..c*10+9).
  - Per layer: hws = (h*nrm) @ W[l] on the owned shard (bf16), AllGather the
    partition-major table (row c*128+p holds core c's 10 chunks for
    partition p as one contiguous 5KB run), bulk-load into SBUF.
  - Aggregation is PURE dense one-hot adjacency on TensorE: per dst-block an
    80-chunk PSUM-accumulated matmul chain (lhsT = A chunk [128 src x 128
    dst] with edge-multiplicity counts, rhs = table chunk [128 src x 256]).
    A (26MB/core) is streamed from HBM in half-block tiles, software
    pipelined 2 halves ahead; zero GPSIMD, zero scattered DMA (SWDGE
    gathers at 8ns/row descriptor cost were the v1 bottleneck, and their
    512B scattered packets degraded the whole DMA subsystem).
  - The AllGather is split in two half-payload collectives: the second half
    transfers while TensorE chews the first half's chunks (chain order is
    free under PSUM accumulation).
  - BatchNorm: partial sums/sumsq -> stats replicated to 128 rows so the
    2KB AllReduce becomes a 256KB RDH AllReduce (Mesh at 2KB costs 79us);
    scale/shift broadcast via rank-1 TensorE matmul; relu+residual on DVE.
  - Readout: graph mean-pool via preloaded bf16 one-hot pool matrices,
    257-row AllReduce, 3-layer MLP computed redundantly on every core.
"""

import os
import sys

sys.path.insert(0, "/opt/trn_rl_repo")

from contextlib import ExitStack

import numpy as np
import ml_dtypes

from concourse import bass, mybir, bacc, tile, library_config
from concourse.bass_utils import run_bass_kernel_spmd
from concourse.masks import make_identity

NCORE = 8
P = 128
H = 256
L = 4
NF = 9
G = 256
N = 10000
BPC = 10                # dst blocks per core
NPC = BPC * P           # 1280 nodes per core
NPAD = NCORE * NPC      # 10240
NCHUNK = NPAD // P      # 80 src chunks
HB = BPC // 2           # blocks per AllGather half
BN_EPS = 1e-5

f32 = mybir.dt.float32
bf16 = mybir.dt.bfloat16
bfnp = ml_dtypes.bfloat16

FT = mybir.ActivationFunctionType
OP = mybir.AluOpType

_compiled = {}

# chunk consumption order: first-half chunks (block%10 < 5) first, so the
# dense chains can start right after AllGather half A lands
CHUNK_ORDER = [g for g in range(NCHUNK) if g % BPC < HB] + \
              [g for g in range(NCHUNK) if g % BPC >= HB]


# --------------------------------------------------------------------------
# host-side structural preprocessing
# --------------------------------------------------------------------------

def _preprocess(x, edge_index, batch_ids, emb, W, gamma, beta,
                mlp_W1, mlp_b1, mlp_W2, mlp_b2, mlp_W3, mlp_b3):
    src = np.asarray(edge_index[0], np.int64)
    dst = np.asarray(edge_index[1], np.int64)
    # self loops for every real node (weight nrm[d]^2 folds in)
    src_all = np.concatenate([src, np.arange(N, dtype=np.int64)])
    dst_all = np.concatenate([dst, np.arange(N, dtype=np.int64)])
    order = np.argsort(dst_all, kind="stable")
    s_sorted = src_all[order]
    d_sorted = dst_all[order]

    deg = np.bincount(dst_all, minlength=NPAD).astype(np.float64)  # incl self

    nblk = NCORE * BPC
    starts = np.searchsorted(d_sorted, np.arange(nblk) * P)
    ends = np.searchsorted(d_sorted, (np.arange(nblk) + 1) * P)

    # dense adjacency per dst block, chunk-major in CHUNK_ORDER
    A_blocks = {}
    for g in range(nblk):
        c, nb = divmod(g, BPC)
        e_s = s_sorted[starts[g]:ends[g]]
        e_d = d_sorted[starts[g]:ends[g]] - g * P
        A = np.zeros((NPAD, P), np.float32)
        np.add.at(A, (e_s, e_d), 1.0)
        A = A.reshape(NCHUNK, P, P)[CHUNK_ORDER]          # reorder chunks
        A_blocks[(c, nb)] = np.ascontiguousarray(
            A.transpose(1, 0, 2).reshape(P, NCHUNK * P)).astype(bfnp)

    # graph pool one-hot [node, graph] (bf16: values 0/1 exact)
    bids = np.asarray(batch_ids, np.int64)
    psel_full = np.zeros((NPAD, G), np.float32)
    psel_full[np.arange(N), bids] = 1.0

    x_np = np.zeros((NPAD, NF), np.float32)
    x_np[:N] = np.asarray(x, np.float64)

    Wf = np.asarray(W, np.float32)
    W_lhsT = Wf.reshape(L, 2, P, H).transpose(2, 0, 1, 3).reshape(P, L * 2 * H)
    gb = np.concatenate([np.asarray(gamma, np.float32).reshape(-1),
                         np.asarray(beta, np.float32).reshape(-1)])[None, :]
    embf = np.asarray(emb, np.float32)
    emb0 = np.ascontiguousarray(embf[:, 0, :])
    emb1 = np.ascontiguousarray(embf[:, 1, :])
    w1 = np.asarray(mlp_W1, np.float32).reshape(2, P, P).transpose(1, 0, 2).reshape(P, 2 * P)
    w2 = np.asarray(mlp_W2, np.float32)
    w3 = np.asarray(mlp_W3, np.float32)
    b1 = np.asarray(mlp_b1, np.float32).reshape(P, 1)
    b2 = np.asarray(mlp_b2, np.float32).reshape(64, 1)
    b3 = np.asarray(mlp_b3, np.float32).reshape(1, 1)

    in_maps = []
    HC = NCHUNK // 2
    for c in range(NCORE):
        lo, hi = c * NPC, (c + 1) * NPC
        Ac = np.concatenate([A_blocks[(c, nb)] for nb in range(BPC)], axis=1)
        # reorder half-tiles to match the two-pass consumption order:
        # tile nh = half*BPC + nb  (all first halves, then all second halves)
        Ac = np.ascontiguousarray(
            Ac.reshape(P, BPC, 2, HC * P).transpose(0, 2, 1, 3)
        ).reshape(P, BPC * NCHUNK * P)

        degc = deg[lo:hi].reshape(BPC, P).T
        maskc = (degc > 0).astype(np.float32)
        degc = np.maximum(degc, 1.0).astype(np.float32)

        pselc = psel_full[lo:hi].reshape(BPC, P, G)
        pselc = np.ascontiguousarray(pselc.transpose(1, 0, 2)).reshape(P, BPC * G)

        in_maps.append(dict(
            A=Ac, xT=np.ascontiguousarray(x_np[lo:hi].T),
            deg=degc, mask=maskc, psel=pselc.astype(bfnp),
            W=W_lhsT.astype(bfnp), gb=gb, emb0=emb0, emb1=emb1,
            w1=w1, w2=w2, w3=w3, b1=b1, b2=b2, b3=b3,
        ))
    return in_maps


# --------------------------------------------------------------------------
# device program
# --------------------------------------------------------------------------

def _build():
    nc = bacc.Bacc(None, target_bir_lowering=False)

    d_A = nc.dram_tensor("A", [P, BPC * NCHUNK * P], bf16, kind="ExternalInput")
    d_xT = nc.dram_tensor("xT", [NF, NPC], f32, kind="ExternalInput")
    d_deg = nc.dram_tensor("deg", [P, BPC], f32, kind="ExternalInput")
    d_mask = nc.dram_tensor("mask", [P, BPC], f32, kind="ExternalInput")
    d_psel = nc.dram_tensor("psel", [P, BPC * G], bf16, kind="ExternalInput")
    d_W = nc.dram_tensor("W", [P, L * 2 * H], bf16, kind="ExternalInput")
    d_gb = nc.dram_tensor("gb", [1, 2 * L * H], f32, kind="ExternalInput")
    d_emb0 = nc.dram_tensor("emb0", [NF, H], f32, kind="ExternalInput")
    d_emb1 = nc.dram_tensor("emb1", [NF, H], f32, kind="ExternalInput")
    d_w1 = nc.dram_tensor("w1", [P, 2 * P], f32, kind="ExternalInput")
    d_w2 = nc.dram_tensor("w2", [P, 64], f32, kind="ExternalInput")
    d_w3 = nc.dram_tensor("w3", [64, 1], f32, kind="ExternalInput")
    d_b1 = nc.dram_tensor("b1", [P, 1], f32, kind="ExternalInput")
    d_b2 = nc.dram_tensor("b2", [64, 1], f32, kind="ExternalInput")
    d_b3 = nc.dram_tensor("b3", [1, 1], f32, kind="ExternalInput")
    d_out = nc.dram_tensor("out", [1, G], f32, kind="ExternalOutput")

    rg = [list(range(NCORE))]
    HW = HB * H          # half payload width per partition (1280 cols)

    with tile.TileContext(nc) as tc, ExitStack() as ctx:
        pers = ctx.enter_context(tc.tile_pool(name="pers", bufs=1))
        psA = ctx.enter_context(tc.tile_pool(name="psA", bufs=2, space="PSUM"))
        psB = ctx.enter_context(tc.tile_pool(name="psB", bufs=2, space="PSUM"))
        psC = ctx.enter_context(tc.tile_pool(name="psC", bufs=1, space="PSUM"))
        apool = ctx.enter_context(tc.tile_pool(name="apool", bufs=4))
        work = ctx.enter_context(tc.tile_pool(name="work", bufs=2))
        stream = ctx.enter_context(tc.tile_pool(name="stream", bufs=2))
        dram = ctx.enter_context(tc.tile_pool(name="dram", bufs=2, space="DRAM"))

        # ---- persistent SBUF state -------------------------------------
        deg_sb = pers.tile([P, BPC], f32, tag="deg")
        mask_sb = pers.tile([P, BPC], f32, tag="mask")
        psel_sb = pers.tile([P, BPC * G], bf16, tag="psel")
        W_sb = pers.tile([P, L * 2 * H], bf16, tag="W")
        gb_sb = pers.tile([1, 2 * L * H], f32, tag="gb")
        emb0_sb = pers.tile([NF, H], f32, tag="emb0")
        emb1_sb = pers.tile([NF, H], f32, tag="emb1")
        w1_sb = pers.tile([P, 2 * P], f32, tag="w1")
        w2_sb = pers.tile([P, 64], f32, tag="w2")
        w3_sb = pers.tile([64, 1], f32, tag="w3")
        b1_sb = pers.tile([P, 1], f32, tag="b1")
        b2_sb = pers.tile([64, 1], f32, tag="b2")
        b3_sb = pers.tile([1, 1], f32, tag="b3")

        tab_sb = pers.tile([P, NCHUNK * H], bf16, tag="tab")
        h_sb = pers.tile([P, BPC * H], f32, tag="h")
        hsT_sb = pers.tile([P, BPC * 2 * P], bf16, tag="hsT")
        hws_sb = pers.tile([P, BPC * H], bf16, tag="hws")
        t_all = pers.tile([P, BPC * H], f32, tag="t_all")
        nrm_sb = pers.tile([P, BPC], f32, tag="nrm")
        acc_s = pers.tile([P, H], f32, tag="acc_s")
        acc_q = pers.tile([P, H], f32, tag="acc_q")
        D_sb = pers.tile([NF, H], f32, tag="D")
        base_rep = pers.tile([P, H], f32, tag="base_rep")
        a_rep = pers.tile([P, H], f32, tag="a_rep")
        c_rep = pers.tile([P, H], f32, tag="c_rep")
        ident_bf = pers.tile([P, P], bf16, tag="ident")
        ones9 = pers.tile([NF, 1], f32, tag="ones9")
        ones1 = pers.tile([1, P], f32, tag="ones1")
        ones128 = pers.tile([P, 1], f32, tag="ones128")
        ones128b = pers.tile([P, 1], bf16, tag="ones128b")
        stv = pers.tile([1, 2 * H], f32, tag="stv")
        scal = pers.tile([1, 8 * H], f32, tag="scal")

        # ---- DRAM bounce buffers ---------------------------------------
        # AllGather halves: ag_inX[p, :] = hws rows for blocks of that half
        # (5 blocks x 256 = 1280 cols, a contiguous 2.5KB run per partition;
        # ag_outX row c*128+p holds core c's half-run for partition p).
        ag_inA = dram.tile([P, HW], bf16, tag="ag_inA")
        ag_inB = dram.tile([P, HW], bf16, tag="ag_inB")
        ag_outA = dram.tile([NCORE * P, HW], bf16, tag="ag_outA")
        ag_outB = dram.tile([NCORE * P, HW], bf16, tag="ag_outB")
        ar_in = dram.tile([P, 2 * H], f32, tag="ar_in")
        ar_out = dram.tile([P, 2 * H], f32, tag="ar_out")
        pr_in = dram.tile([2 * P + 1, G], f32, tag="pr_in")
        pr_out = dram.tile([2 * P + 1, G], f32, tag="pr_out")

        # ---- input loads ------------------------------------------------
        for t, d in [(deg_sb, d_deg), (mask_sb, d_mask), (psel_sb, d_psel),
                     (W_sb, d_W), (gb_sb, d_gb), (emb0_sb, d_emb0),
                     (emb1_sb, d_emb1), (w1_sb, d_w1), (w2_sb, d_w2),
                     (w3_sb, d_w3), (b1_sb, d_b1), (b2_sb, d_b2),
                     (b3_sb, d_b3)]:
            nc.sync.dma_start(out=t[:], in_=d[:])

        make_identity(nc, ident_bf[:])
        nc.vector.memset(ones9[:], 1.0)
        nc.vector.memset(ones1[:], 1.0)
        nc.vector.memset(ones128[:], 1.0)
        nc.vector.memset(ones128b[:], 1.0)

        # nrm = rsqrt(deg) * mask
        rdeg = work.tile([P, BPC], f32, tag="rdeg")
        nc.vector.reciprocal(out=rdeg[:], in_=deg_sb[:])
        nc.scalar.activation(out=rdeg[:], in_=rdeg[:], func=FT.Sqrt)
        nc.vector.tensor_tensor(out=nrm_sb[:], in0=rdeg[:], in1=mask_sb[:], op=OP.mult)

        # encoder prep: D = emb1 - emb0 ; base = ones9^T @ emb0, broadcast
        nc.vector.tensor_tensor(out=D_sb[:], in0=emb1_sb[:], in1=emb0_sb[:], op=OP.subtract)
        ps_b = psB.tile([1, H], f32, tag="vec")
        nc.tensor.matmul(out=ps_b[:], lhsT=ones9[:], rhs=emb0_sb[:], start=True, stop=True)
        bvec = scal[:, 0:H]
        nc.vector.tensor_copy(out=bvec, in_=ps_b[:])
        ps_br = psB.tile([P, H], f32, tag="vec")
        nc.tensor.matmul(out=ps_br[:], lhsT=ones1[:], rhs=bvec, start=True, stop=True)
        nc.vector.tensor_copy(out=base_rep[:], in_=ps_br[:])

        def hslice(nb):
            return h_sb[:, nb * H:(nb + 1) * H]

        def emit_hs_transpose(nb):
            """hs = h*nrm (bf16), transpose both 128-halves into hsT_sb."""
            hs_bf = work.tile([P, H], bf16, tag="hs_bf")
            nc.vector.tensor_scalar_mul(hs_bf[:], hslice(nb), nrm_sb[:, nb:nb + 1])
            for k in range(2):
                pst = psB.tile([P, P], bf16, tag="pst")
                nc.tensor.transpose(out=pst[:], in_=hs_bf[:, k * P:(k + 1) * P],
                                    identity=ident_bf[:])
                nc.vector.tensor_copy(out=hsT_sb[:, (nb * 2 + k) * P:(nb * 2 + k + 1) * P],
                                      in_=pst[:])

        # encoder: h0 = base + xT^T @ D  (per block)
        for nb in range(BPC):
            xT_t = stream.tile([NF, P], f32, tag="xT_t")
            nc.sync.dma_start(out=xT_t[:], in_=d_xT[:, nb * P:(nb + 1) * P])
            ps_h = psA.tile([P, H], f32, tag="mm")
            nc.tensor.matmul(out=ps_h[:], lhsT=xT_t[:],
                             rhs=D_sb[:], start=True, stop=True)
            nc.vector.tensor_tensor(out=hslice(nb), in0=ps_h[:], in1=base_rep[:], op=OP.add)
            emit_hs_transpose(nb)

        HC = NCHUNK // 2     # chunks per A half-tile
        a_fifo = []

        def a_prefetch(nh):
            a_t = apool.tile([P, HC * P], bf16, tag="A")
            nc.sync.dma_start(out=a_t[:], in_=d_A[:, nh * HC * P:(nh + 1) * HC * P])
            a_fifo.append(a_t)

        def emit_gemm(l, nb):
            ps_g = psA.tile([P, H], f32, tag="mm")
            for k in range(2):
                nc.tensor.matmul(
                    out=ps_g[:],
                    lhsT=hsT_sb[:, (nb * 2 + k) * P:(nb * 2 + k + 1) * P],
                    rhs=W_sb[:, (l * 2 + k) * H:(l * 2 + k + 1) * H],
                    start=(k == 0), stop=(k == 1))
            nc.vector.tensor_copy(out=hws_sb[:, nb * H:(nb + 1) * H], in_=ps_g[:])

        def emit_ag_half(half):
            if half == 0:
                nc.sync.dma_start(out=ag_inA[:], in_=hws_sb[:, 0:HW])
                nc.gpsimd.collective_compute(
                    "AllGather", OP.bypass, replica_groups=rg,
                    ins=[ag_inA[:]], outs=[ag_outA[:]])
            else:
                nc.sync.dma_start(out=ag_inB[:], in_=hws_sb[:, HW:2 * HW])
                nc.gpsimd.collective_compute(
                    "AllGather", OP.bypass, replica_groups=rg,
                    ins=[ag_inB[:]], outs=[ag_outB[:]])

        # Two passes over dst blocks: pass 0 consumes only first-half chunks
        # (available right after AllGather A), so all 400 pass-0 matmuls run
        # while AllGather B is in flight; pass 1 adds second-half chunks.
        def emit_half_chain(nb, half):
            nh = half * BPC + nb
            if nh + 2 < 2 * BPC:
                a_prefetch(nh + 2)
            a_t = a_fifo.pop(0)
            ps_t = psA.tile([P, H], f32, tag="mm")
            for cc in range(HC):
                nc.tensor.matmul(
                    out=ps_t[:],
                    lhsT=a_t[:, cc * P:(cc + 1) * P],
                    rhs=tab_sb[:, (half * HC + cc) * H:(half * HC + cc + 1) * H],
                    start=(cc == 0), stop=(cc == HC - 1))
            tsl = t_all[:, nb * H:(nb + 1) * H]
            if half == 0:
                nc.vector.tensor_copy(out=tsl, in_=ps_t[:])
            else:
                # t = nrm*(partA+partB); accumulate BN stats
                nc.vector.tensor_tensor(out=tsl, in0=tsl, in1=ps_t[:], op=OP.add)
                nc.vector.tensor_scalar_mul(tsl, tsl, nrm_sb[:, nb:nb + 1])
                sq = work.tile([P, H], f32, tag="tmp")
                nc.vector.tensor_tensor(out=sq[:], in0=tsl, in1=tsl, op=OP.mult)
                nc.vector.tensor_tensor(out=acc_s[:], in0=acc_s[:], in1=tsl, op=OP.add)
                nc.vector.tensor_tensor(out=acc_q[:], in0=acc_q[:], in1=sq[:], op=OP.add)

        # bootstrap layer 0's GEMM + AllGather halves (from encoder hsT)
        for nb in range(0, HB):
            emit_gemm(0, nb)
        emit_ag_half(0)
        a_prefetch(0)
        a_prefetch(1)
        for nb in range(HB, BPC):
            emit_gemm(0, nb)
        emit_ag_half(1)

        # ---- layers -----------------------------------------------------
        for l in range(L):
            # Table loads go on the Scalar engine's DMA queue: on the Sync
            # queue the tab-B load (which waits for AllGather B) gets
            # scheduled ahead of the A prefetches and stalls them ~28us.
            nc.scalar.dma_start(
                out=tab_sb[:, 0:NCHUNK * H // 2].rearrange("p (c w) -> p c w", c=NCORE),
                in_=ag_outA[:].rearrange("(c p) w -> p c w", p=P))
            nc.scalar.dma_start(
                out=tab_sb[:, NCHUNK * H // 2:].rearrange("p (c w) -> p c w", c=NCORE),
                in_=ag_outB[:].rearrange("(c p) w -> p c w", p=P))
            nc.vector.memset(acc_s[:], 0.0)
            nc.vector.memset(acc_q[:], 0.0)
            for nb in range(BPC):
                emit_half_chain(nb, 0)
            for nb in range(BPC):
                emit_half_chain(nb, 1)

            # stats: cross-partition reduce, replicate to 128 rows, AllReduce
            ps_s = psB.tile([1, 2 * H], f32, tag="vec")
            nc.tensor.matmul(out=ps_s[:, 0:H], lhsT=ones128[:], rhs=acc_s[:],
                             start=True, stop=True)
            nc.tensor.matmul(out=ps_s[:, H:2 * H], lhsT=ones128[:], rhs=acc_q[:],
                             start=True, stop=True)
            st_sb = scal[:, 6 * H:8 * H]
            nc.vector.tensor_copy(out=st_sb, in_=ps_s[:])
            st_rep = work.tile([P, 2 * H], f32, tag="strep")
            for half in range(2):
                ps_r2 = psB.tile([P, H], f32, tag="vec")
                nc.tensor.matmul(out=ps_r2[:], lhsT=ones1[:],
                                 rhs=st_sb[:, half * H:(half + 1) * H],
                                 start=True, stop=True)
                nc.vector.tensor_copy(out=st_rep[:, half * H:(half + 1) * H],
                                      in_=ps_r2[:])
            nc.sync.dma_start(out=ar_in[:], in_=st_rep[:])
            nc.gpsimd.collective_compute(
                "AllReduce", OP.add, replica_groups=rg,
                ins=[ar_in[:]], outs=[ar_out[:]])
            nc.sync.dma_start(out=stv[:], in_=ar_out[0:1, :])

            # a = gamma*istd ; c = beta - mu*a
            mu = scal[:, H:2 * H]
            var = scal[:, 2 * H:3 * H]
            av = scal[:, 3 * H:4 * H]
            cv = scal[:, 4 * H:5 * H]
            msq = scal[:, 5 * H:6 * H]
            nc.vector.tensor_scalar_mul(mu, stv[:, 0:H], 1.0 / N)
            nc.vector.tensor_scalar_mul(var, stv[:, H:2 * H], 1.0 / N)
            nc.vector.tensor_tensor(out=msq, in0=mu, in1=mu, op=OP.mult)
            nc.vector.tensor_tensor(out=var, in0=var, in1=msq, op=OP.subtract)
            nc.vector.tensor_scalar_add(var, var, BN_EPS)
            nc.vector.reciprocal(out=var, in_=var)
            nc.scalar.activation(out=var, in_=var, func=FT.Sqrt)  # istd
            nc.vector.tensor_tensor(out=av, in0=var,
                                    in1=gb_sb[:, l * H:(l + 1) * H], op=OP.mult)
            nc.vector.tensor_tensor(out=msq, in0=mu, in1=av, op=OP.mult)
            nc.vector.tensor_tensor(out=cv, in0=gb_sb[:, (L + l) * H:(L + l + 1) * H],
                                    in1=msq, op=OP.subtract)
            ps_a = psB.tile([P, H], f32, tag="vec")
            nc.tensor.matmul(out=ps_a[:], lhsT=ones1[:], rhs=av, start=True, stop=True)
            nc.vector.tensor_copy(out=a_rep[:], in_=ps_a[:])
            ps_c = psB.tile([P, H], f32, tag="vec")
            nc.tensor.matmul(out=ps_c[:], lhsT=ones1[:], rhs=cv, start=True, stop=True)
            nc.vector.tensor_copy(out=c_rep[:], in_=ps_c[:])

            # h = relu(t*a + c) + h ; immediately GEMM the updated block for
            # the next layer and post the AllGather halves as soon as each
            # half's blocks are done.
            for nb in range(BPC):
                tsl = t_all[:, nb * H:(nb + 1) * H]
                u = work.tile([P, H], f32, tag="tmp")
                nc.vector.tensor_tensor(out=u[:], in0=tsl, in1=a_rep[:], op=OP.mult)
                nc.vector.tensor_tensor(out=u[:], in0=u[:], in1=c_rep[:], op=OP.add)
                r = work.tile([P, H], f32, tag="tmp2")
                nc.scalar.activation(out=r[:], in_=u[:], func=FT.Relu)
                nc.vector.tensor_tensor(out=hslice(nb), in0=hslice(nb), in1=r[:], op=OP.add)
                if l < L - 1:
                    emit_hs_transpose(nb)
                    emit_gemm(l + 1, nb)
                    if nb == HB - 1:
                        emit_ag_half(0)
                        a_prefetch(0)
                        a_prefetch(1)
                    elif nb == BPC - 1:
                        emit_ag_half(1)

        # ---- pooling (bf16 matmuls; psel preloaded) ---------------------
        ps_p0 = psC.tile([P, G], f32, tag="p0")
        ps_p1 = psC.tile([P, G], f32, tag="p1")
        ps_pc = psB.tile([1, G], f32, tag="vec")
        for nb in range(BPC):
            hb_t = work.tile([P, H], bf16, tag="hb")
            nc.vector.tensor_copy(out=hb_t[:], in_=hslice(nb))
            pssl = psel_sb[:, nb * G:(nb + 1) * G]
            nc.tensor.matmul(out=ps_p0[:], lhsT=hb_t[:, 0:P],
                             rhs=pssl, start=(nb == 0), stop=(nb == BPC - 1))
            nc.tensor.matmul(out=ps_p1[:], lhsT=hb_t[:, P:2 * P],
                             rhs=pssl, start=(nb == 0), stop=(nb == BPC - 1))
            nc.tensor.matmul(out=ps_pc[:], lhsT=ones128b[:],
                             rhs=pssl, start=(nb == 0), stop=(nb == BPC - 1))
        g0 = work.tile([P, G], f32, tag="g0")
        g1 = work.tile([P, G], f32, tag="g1")
        cnt = scal[:, 0:G]
        nc.vector.tensor_copy(out=g0[:], in_=ps_p0[:])
        nc.vector.tensor_copy(out=g1[:], in_=ps_p1[:])
        nc.vector.tensor_copy(out=cnt, in_=ps_pc[:])
        nc.sync.dma_start(out=pr_in[0:P, :], in_=g0[:])
        nc.sync.dma_start(out=pr_in[P:2 * P, :], in_=g1[:])
        nc.sync.dma_start(out=pr_in[2 * P:2 * P + 1, :], in_=cnt)
        nc.gpsimd.collective_compute(
            "AllReduce", OP.add, replica_groups=rg,
            ins=[pr_in[:]], outs=[pr_out[:]])
        nc.sync.dma_start(out=g0[:], in_=pr_out[0:P, :])
        nc.sync.dma_start(out=g1[:], in_=pr_out[P:2 * P, :])
        nc.sync.dma_start(out=cnt, in_=pr_out[2 * P:2 * P + 1, :])
        nc.vector.tensor_scalar_max(cnt, cnt, 1.0)
        nc.vector.reciprocal(out=cnt, in_=cnt)
        ps_r = psB.tile([P, G], f32, tag="vec")
        nc.tensor.matmul(out=ps_r[:], lhsT=ones1[:], rhs=cnt, start=True, stop=True)
        rc_rep = work.tile([P, G], f32, tag="rc_rep")
        nc.vector.tensor_copy(out=rc_rep[:], in_=ps_r[:])
        nc.vector.tensor_tensor(out=g0[:], in0=g0[:], in1=rc_rep[:], op=OP.mult)
        nc.vector.tensor_tensor(out=g1[:], in0=g1[:], in1=rc_rep[:], op=OP.mult)

        # MLP head (transposed: weights are lhsT, graphs along free dim)
        ps1 = psB.tile([P, G], f32, tag="vec")
        nc.tensor.matmul(out=ps1[:], lhsT=w1_sb[:, 0:P], rhs=g0[:], start=True, stop=False)
        nc.tensor.matmul(out=ps1[:], lhsT=w1_sb[:, P:2 * P], rhs=g1[:], start=False, stop=True)
        y1 = work.tile([P, G], f32, tag="y1")
        nc.scalar.activation(out=y1[:], in_=ps1[:], func=FT.Relu, bias=b1_sb[:, 0:1])
        ps2 = psB.tile([64, G], f32, tag="vec")
        nc.tensor.matmul(out=ps2[:], lhsT=w2_sb[:], rhs=y1[:], start=True, stop=True)
        y2 = work.tile([64, G], f32, tag="y2")
        nc.scalar.activation(out=y2[:], in_=ps2[:], func=FT.Relu, bias=b2_sb[:, 0:1])
        ps3 = psB.tile([1, G], f32, tag="vec")
        nc.tensor.matmul(out=ps3[:], lhsT=w3_sb[:], rhs=y2[:], start=True, stop=True)
        y3 = work.tile([1, G], f32, tag="y3")
        nc.vector.tensor_scalar_add(y3[:], ps3[:], b3_sb[0:1, 0:1])
        nc.sync.dma_start(out=d_out[:], in_=y3[:])

    nc.compile()
    return nc


# --------------------------------------------------------------------------
# entry point
# --------------------------------------------------------------------------

def kernel(x, edge_index, batch_ids, emb, W, b, gamma, beta,
           mlp_W1, mlp_b1, mlp_W2, mlp_b2, mlp_W3, mlp_b3,
           _trace=False, _trace_kwargs=None):
    # NB: reference BN subtracts the per-channel mean, so the additive bias b
    # cancels exactly and is not needed by the device program.
    in_maps = _preprocess(x, edge_index, batch_ids, emb, W, gamma, beta,
                          mlp_W1, mlp_b1, mlp_W2, mlp_b2, mlp_W3, mlp_b3)
    if "nc" not in _compiled:
        _compiled["nc"] = _build()
    nc = _compiled["nc"]
    kw = {}
    if _trace:
        kw = dict(trace=True, **(_trace_kwargs or {}))
    res = run_bass_kernel_spmd(nc, in_maps, core_ids=list(range(NCORE)), **kw)
    out = np.asarray(res.results[0]["out"], np.float32).reshape(G, 1)
    kernel._last_results = res
    return out


# revision 39
# speedup vs baseline: 2.7758x; 1.0916x over previous
"""Trainium2 Bass kernel for HIVNet GCN message passing (8-core SPMD).

v6 strategy (baseline 2.29ms -> v5 hybrid 1.48ms -> v6 pure-dense):
  - Pad N=10000 nodes to 10240 = 80 chunks x 128; core c owns 10 dst-blocks
    (global chunks c*10..c*10+9).
  - Per layer: hws = (h*nrm) @ W[l] on the owned shard (bf16), AllGather the
    partition-major table (row c*128+p holds core c's 10 chunks for
    partition p as one contiguous 5KB run), bulk-load into SBUF.
  - Aggregation is PURE dense one-hot adjacency on TensorE: per dst-block an
    80-chunk PSUM-accumulated matmul chain (lhsT = A chunk [128 src x 128
    dst] with edge-multiplicity counts, rhs = table chunk [128 src x 256]).
    A (26MB/core) is streamed from HBM in half-block tiles, software
    pipelined 2 halves ahead; zero GPSIMD, zero scattered DMA (SWDGE
    gathers at 8ns/row descriptor cost were the v1 bottleneck, and their
    512B scattered packets degraded the whole DMA subsystem).
  - The AllGather is split in two half-payload collectives: the second half
    transfers while TensorE chews the first half's chunks (chain order is
    free under PSUM accumulation).
  - BatchNorm: partial sums/sumsq -> stats replicated to 128 rows so the
    2KB AllReduce becomes a 256KB RDH AllReduce (Mesh at 2KB costs 79us);
    scale/shift broadcast via rank-1 TensorE matmul; relu+residual on DVE.
  - Readout: graph mean-pool via preloaded bf16 one-hot pool matrices,
    257-row AllReduce, 3-layer MLP computed redundantly on every core.
"""

import os
import sys

sys.path.insert(0, "/opt/trn_rl_repo")

from contextlib import ExitStack

import numpy as np
import ml_dtypes

from concourse import bass, mybir, bacc, tile, library_config
from concourse.bass_utils import run_bass_kernel_spmd
from concourse.masks import make_identity

NCORE = 8
P = 128
H = 256
L = 4
NF = 9
G = 256
N = 10000
BPC = 10                # dst blocks per core
NPC = BPC * P           # 1280 nodes per core
NPAD = NCORE * NPC      # 10240
NCHUNK = NPAD // P      # 80 src chunks
HB = BPC // 2           # blocks per AllGather half
BN_EPS = 1e-5

f32 = mybir.dt.float32
bf16 = mybir.dt.bfloat16
f8 = mybir.dt.float8e4
bfnp = ml_dtypes.bfloat16

FT = mybir.ActivationFunctionType
OP = mybir.AluOpType

_compiled = {}

# chunk consumption order: first-half chunks (block%10 < 5) first, so the
# dense chains can start right after AllGather half A lands
CHUNK_ORDER = [g for g in range(NCHUNK) if g % BPC < HB] + \
              [g for g in range(NCHUNK) if g % BPC >= HB]


# --------------------------------------------------------------------------
# host-side structural preprocessing
# --------------------------------------------------------------------------

def _preprocess(x, edge_index, batch_ids, emb, W, gamma, beta,
                mlp_W1, mlp_b1, mlp_W2, mlp_b2, mlp_W3, mlp_b3):
    src = np.asarray(edge_index[0], np.int64)
    dst = np.asarray(edge_index[1], np.int64)
    # self loops for every real node (weight nrm[d]^2 folds in)
    src_all = np.concatenate([src, np.arange(N, dtype=np.int64)])
    dst_all = np.concatenate([dst, np.arange(N, dtype=np.int64)])
    order = np.argsort(dst_all, kind="stable")
    s_sorted = src_all[order]
    d_sorted = dst_all[order]

    deg = np.bincount(dst_all, minlength=NPAD).astype(np.float64)  # incl self

    nblk = NCORE * BPC
    starts = np.searchsorted(d_sorted, np.arange(nblk) * P)
    ends = np.searchsorted(d_sorted, (np.arange(nblk) + 1) * P)

    # dense adjacency per dst block, chunk-major in CHUNK_ORDER
    A_blocks = {}
    for g in range(nblk):
        c, nb = divmod(g, BPC)
        e_s = s_sorted[starts[g]:ends[g]]
        e_d = d_sorted[starts[g]:ends[g]] - g * P
        A = np.zeros((NPAD, P), np.float32)
        np.add.at(A, (e_s, e_d), 1.0)
        A = A.reshape(NCHUNK, P, P)[CHUNK_ORDER]          # reorder chunks
        # fp8 e4m3: edge multiplicities (<= 3 incl. self loop) are exact,
        # and mixed fp8 lhsT x bf16 rhs matmul is supported -- halves the
        # 26MB/core/layer A-stream, the agg-phase DMA floor.
        A_blocks[(c, nb)] = np.ascontiguousarray(
            A.transpose(1, 0, 2).reshape(P, NCHUNK * P)
        ).astype(ml_dtypes.float8_e4m3)

    # graph pool one-hot [node, graph] (bf16: values 0/1 exact)
    bids = np.asarray(batch_ids, np.int64)
    psel_full = np.zeros((NPAD, G), np.float32)
    psel_full[np.arange(N), bids] = 1.0
    cnt = np.bincount(bids, minlength=G).astype(np.float64)
    rcnt = (1.0 / np.maximum(cnt, 1.0)).astype(np.float32)[None, :]

    x_np = np.zeros((NPAD, NF), np.float32)
    x_np[:N] = np.asarray(x, np.float64)

    Wf = np.asarray(W, np.float32)
    W_lhsT = Wf.reshape(L, 2, P, H).transpose(2, 0, 1, 3).reshape(P, L * 2 * H)
    gb = np.concatenate([np.asarray(gamma, np.float32).reshape(-1),
                         np.asarray(beta, np.float32).reshape(-1)])[None, :]
    embf = np.asarray(emb, np.float32)
    emb0 = np.ascontiguousarray(embf[:, 0, :])
    emb1 = np.ascontiguousarray(embf[:, 1, :])
    w1 = np.asarray(mlp_W1, np.float32).reshape(2, P, P).transpose(1, 0, 2).reshape(P, 2 * P)
    w2 = np.asarray(mlp_W2, np.float32)
    w3 = np.asarray(mlp_W3, np.float32)
    b1 = np.asarray(mlp_b1, np.float32).reshape(P, 1)
    b2 = np.asarray(mlp_b2, np.float32).reshape(64, 1)
    b3 = np.asarray(mlp_b3, np.float32).reshape(1, 1)

    in_maps = []
    HC = NCHUNK // 2
    for c in range(NCORE):
        lo, hi = c * NPC, (c + 1) * NPC
        Ac = np.concatenate([A_blocks[(c, nb)] for nb in range(BPC)], axis=1)
        # reorder half-tiles to match the two-pass consumption order:
        # tile nh = half*BPC + nb  (all first halves, then all second halves)
        Ac = np.ascontiguousarray(
            Ac.reshape(P, BPC, 2, HC * P).transpose(0, 2, 1, 3)
        ).reshape(P, BPC * NCHUNK * P)

        degc = deg[lo:hi].reshape(BPC, P).T
        maskc = (degc > 0).astype(np.float32)
        degc = np.maximum(degc, 1.0).astype(np.float32)

        pselc = psel_full[lo:hi].reshape(BPC, P, G)
        pselc = np.ascontiguousarray(pselc.transpose(1, 0, 2)).reshape(P, BPC * G)

        in_maps.append(dict(
            A=Ac, xT=np.ascontiguousarray(x_np[lo:hi].T),
            deg=degc, mask=maskc, psel=pselc.astype(bfnp),
            W=W_lhsT.astype(bfnp), gb=gb, emb0=emb0, emb1=emb1,
            w1=w1, w2=w2, w3=w3, b1=b1, b2=b2, b3=b3, rcnt=rcnt,
        ))
    return in_maps


# --------------------------------------------------------------------------
# device program
# --------------------------------------------------------------------------

def _build():
    nc = bacc.Bacc(None, target_bir_lowering=False)

    d_A = nc.dram_tensor("A", [P, BPC * NCHUNK * P], f8, kind="ExternalInput")
    d_xT = nc.dram_tensor("xT", [NF, NPC], f32, kind="ExternalInput")
    d_deg = nc.dram_tensor("deg", [P, BPC], f32, kind="ExternalInput")
    d_mask = nc.dram_tensor("mask", [P, BPC], f32, kind="ExternalInput")
    d_psel = nc.dram_tensor("psel", [P, BPC * G], bf16, kind="ExternalInput")
    d_W = nc.dram_tensor("W", [P, L * 2 * H], bf16, kind="ExternalInput")
    d_gb = nc.dram_tensor("gb", [1, 2 * L * H], f32, kind="ExternalInput")
    d_emb0 = nc.dram_tensor("emb0", [NF, H], f32, kind="ExternalInput")
    d_emb1 = nc.dram_tensor("emb1", [NF, H], f32, kind="ExternalInput")
    d_w1 = nc.dram_tensor("w1", [P, 2 * P], f32, kind="ExternalInput")
    d_w2 = nc.dram_tensor("w2", [P, 64], f32, kind="ExternalInput")
    d_w3 = nc.dram_tensor("w3", [64, 1], f32, kind="ExternalInput")
    d_b1 = nc.dram_tensor("b1", [P, 1], f32, kind="ExternalInput")
    d_b2 = nc.dram_tensor("b2", [64, 1], f32, kind="ExternalInput")
    d_b3 = nc.dram_tensor("b3", [1, 1], f32, kind="ExternalInput")
    d_rcnt = nc.dram_tensor("rcnt", [1, G], f32, kind="ExternalInput")
    d_out = nc.dram_tensor("out", [1, G], f32, kind="ExternalOutput")

    rg = [list(range(NCORE))]
    HW = HB * H          # half payload width per partition (1280 cols)

    with tile.TileContext(nc) as tc, ExitStack() as ctx:
        pers = ctx.enter_context(tc.tile_pool(name="pers", bufs=1))
        psA = ctx.enter_context(tc.tile_pool(name="psA", bufs=2, space="PSUM"))
        psB = ctx.enter_context(tc.tile_pool(name="psB", bufs=2, space="PSUM"))
        psC = ctx.enter_context(tc.tile_pool(name="psC", bufs=1, space="PSUM"))
        apool = ctx.enter_context(tc.tile_pool(name="apool", bufs=6))
        work = ctx.enter_context(tc.tile_pool(name="work", bufs=2))
        stream = ctx.enter_context(tc.tile_pool(name="stream", bufs=2))
        dram = ctx.enter_context(tc.tile_pool(name="dram", bufs=2, space="DRAM"))

        # ---- persistent SBUF state -------------------------------------
        deg_sb = pers.tile([P, BPC], f32, tag="deg")
        mask_sb = pers.tile([P, BPC], f32, tag="mask")
        psel_sb = pers.tile([P, BPC * G], bf16, tag="psel")
        W_sb = pers.tile([P, L * 2 * H], bf16, tag="W")
        gb_sb = pers.tile([1, 2 * L * H], f32, tag="gb")
        emb0_sb = pers.tile([NF, H], f32, tag="emb0")
        emb1_sb = pers.tile([NF, H], f32, tag="emb1")
        w1_sb = pers.tile([P, 2 * P], f32, tag="w1")
        w2_sb = pers.tile([P, 64], f32, tag="w2")
        w3_sb = pers.tile([64, 1], f32, tag="w3")
        b1_sb = pers.tile([P, 1], f32, tag="b1")
        b2_sb = pers.tile([64, 1], f32, tag="b2")
        b3_sb = pers.tile([1, 1], f32, tag="b3")

        tab_sb = pers.tile([P, NCHUNK * H], bf16, tag="tab")
        h_sb = pers.tile([P, BPC * H], f32, tag="h")
        hsT_sb = pers.tile([P, BPC * 2 * P], bf16, tag="hsT")
        hws_sb = pers.tile([P, BPC * H], bf16, tag="hws")
        t_all = pers.tile([P, BPC * H], f32, tag="t_all")
        nrm_sb = pers.tile([P, BPC], f32, tag="nrm")
        acc_s = pers.tile([P, H], f32, tag="acc_s")
        acc_q = pers.tile([P, H], f32, tag="acc_q")
        D_sb = pers.tile([NF, H], f32, tag="D")
        base_rep = pers.tile([P, H], f32, tag="base_rep")
        a_rep = pers.tile([P, H], f32, tag="a_rep")
        c_rep = pers.tile([P, H], f32, tag="c_rep")
        ident_bf = pers.tile([P, P], bf16, tag="ident")
        ones9 = pers.tile([NF, 1], f32, tag="ones9")
        ones1 = pers.tile([1, P], f32, tag="ones1")
        ones128 = pers.tile([P, 1], f32, tag="ones128")
        ones128b = pers.tile([P, 1], bf16, tag="ones128b")
        stv = pers.tile([1, 2 * H], f32, tag="stv")
        rcnt_sb = pers.tile([1, G], f32, tag="rcnt")
        scal = pers.tile([1, 8 * H], f32, tag="scal")

        # ---- DRAM bounce buffers ---------------------------------------
        # AllGather halves: ag_inX[p, :] = hws rows for blocks of that half
        # (5 blocks x 256 = 1280 cols, a contiguous 2.5KB run per partition;
        # ag_outX row c*128+p holds core c's half-run for partition p).
        ag_inA = dram.tile([P, HW], bf16, tag="ag_inA")
        ag_inB = dram.tile([P, HW], bf16, tag="ag_inB")
        ag_outA = dram.tile([NCORE * P, HW], bf16, tag="ag_outA")
        ag_outB = dram.tile([NCORE * P, HW], bf16, tag="ag_outB")
        ar_in = dram.tile([P, 2 * H], f32, tag="ar_in")
        ar_out = dram.tile([P, 2 * H], f32, tag="ar_out")
        pr_in = dram.tile([2 * P, G], f32, tag="pr_in")
        pr_out = dram.tile([2 * P, G], f32, tag="pr_out")

        # ---- input loads ------------------------------------------------
        for t, d in [(deg_sb, d_deg), (mask_sb, d_mask), (psel_sb, d_psel),
                     (W_sb, d_W), (gb_sb, d_gb), (emb0_sb, d_emb0),
                     (emb1_sb, d_emb1), (w1_sb, d_w1), (w2_sb, d_w2),
                     (w3_sb, d_w3), (b1_sb, d_b1), (b2_sb, d_b2),
                     (b3_sb, d_b3), (rcnt_sb, d_rcnt)]:
            nc.sync.dma_start(out=t[:], in_=d[:])

        make_identity(nc, ident_bf[:])
        nc.vector.memset(ones9[:], 1.0)
        nc.vector.memset(ones1[:], 1.0)
        nc.vector.memset(ones128[:], 1.0)
        nc.vector.memset(ones128b[:], 1.0)

        # nrm = rsqrt(deg) * mask
        rdeg = work.tile([P, BPC], f32, tag="rdeg")
        nc.vector.reciprocal(out=rdeg[:], in_=deg_sb[:])
        nc.scalar.activation(out=rdeg[:], in_=rdeg[:], func=FT.Sqrt)
        nc.vector.tensor_tensor(out=nrm_sb[:], in0=rdeg[:], in1=mask_sb[:], op=OP.mult)

        # encoder prep: D = emb1 - emb0 ; base = ones9^T @ emb0, broadcast
        nc.vector.tensor_tensor(out=D_sb[:], in0=emb1_sb[:], in1=emb0_sb[:], op=OP.subtract)
        ps_b = psB.tile([1, H], f32, tag="vec")
        nc.tensor.matmul(out=ps_b[:], lhsT=ones9[:], rhs=emb0_sb[:], start=True, stop=True)
        bvec = scal[:, 0:H]
        nc.vector.tensor_copy(out=bvec, in_=ps_b[:])
        ps_br = psB.tile([P, H], f32, tag="vec")
        nc.tensor.matmul(out=ps_br[:], lhsT=ones1[:], rhs=bvec, start=True, stop=True)
        nc.vector.tensor_copy(out=base_rep[:], in_=ps_br[:])

        def hslice(nb):
            return h_sb[:, nb * H:(nb + 1) * H]

        def emit_hs_transpose(nb):
            """hs = h*nrm (bf16), transpose both 128-halves into hsT_sb."""
            hs_bf = work.tile([P, H], bf16, tag="hs_bf")
            nc.vector.tensor_scalar_mul(hs_bf[:], hslice(nb), nrm_sb[:, nb:nb + 1])
            for k in range(2):
                pst = psB.tile([P, P], bf16, tag="pst")
                nc.tensor.transpose(out=pst[:], in_=hs_bf[:, k * P:(k + 1) * P],
                                    identity=ident_bf[:])
                nc.vector.tensor_copy(out=hsT_sb[:, (nb * 2 + k) * P:(nb * 2 + k + 1) * P],
                                      in_=pst[:])

        # encoder: h0 = base + xT^T @ D  (per block)
        for nb in range(BPC):
            xT_t = stream.tile([NF, P], f32, tag="xT_t")
            nc.sync.dma_start(out=xT_t[:], in_=d_xT[:, nb * P:(nb + 1) * P])
            ps_h = psA.tile([P, H], f32, tag="mm")
            nc.tensor.matmul(out=ps_h[:], lhsT=xT_t[:],
                             rhs=D_sb[:], start=True, stop=True)
            nc.vector.tensor_tensor(out=hslice(nb), in0=ps_h[:], in1=base_rep[:], op=OP.add)
            emit_hs_transpose(nb)

        HC = NCHUNK // 2     # chunks per A half-tile
        a_fifo = []

        def a_prefetch(nh):
            a_t = apool.tile([P, HC * P], f8, tag="A")
            nc.sync.dma_start(out=a_t[:], in_=d_A[:, nh * HC * P:(nh + 1) * HC * P])
            a_fifo.append(a_t)

        def emit_gemm(l, nb):
            ps_g = psA.tile([P, H], f32, tag="mm")
            for k in range(2):
                nc.tensor.matmul(
                    out=ps_g[:],
                    lhsT=hsT_sb[:, (nb * 2 + k) * P:(nb * 2 + k + 1) * P],
                    rhs=W_sb[:, (l * 2 + k) * H:(l * 2 + k + 1) * H],
                    start=(k == 0), stop=(k == 1))
            nc.vector.tensor_copy(out=hws_sb[:, nb * H:(nb + 1) * H], in_=ps_g[:])

        def emit_ag_half(half):
            if half == 0:
                nc.sync.dma_start(out=ag_inA[:], in_=hws_sb[:, 0:HW])
                nc.gpsimd.collective_compute(
                    "AllGather", OP.bypass, replica_groups=rg,
                    ins=[ag_inA[:]], outs=[ag_outA[:]])
            else:
                nc.sync.dma_start(out=ag_inB[:], in_=hws_sb[:, HW:2 * HW])
                nc.gpsimd.collective_compute(
                    "AllGather", OP.bypass, replica_groups=rg,
                    ins=[ag_inB[:]], outs=[ag_outB[:]])

        # Two passes over dst blocks: pass 0 consumes only first-half chunks
        # (available right after AllGather A), so all 400 pass-0 matmuls run
        # while AllGather B is in flight; pass 1 adds second-half chunks.
        def emit_half_chain(nb, half):
            nh = half * BPC + nb
            if nh + 3 < 2 * BPC:
                a_prefetch(nh + 3)
            a_t = a_fifo.pop(0)
            ps_t = psA.tile([P, H], f32, tag="mm")
            for cc in range(HC):
                nc.tensor.matmul(
                    out=ps_t[:],
                    lhsT=a_t[:, cc * P:(cc + 1) * P],
                    rhs=tab_sb[:, (half * HC + cc) * H:(half * HC + cc + 1) * H],
                    start=(cc == 0), stop=(cc == HC - 1))
            tsl = t_all[:, nb * H:(nb + 1) * H]
            if half == 0:
                nc.vector.tensor_copy(out=tsl, in_=ps_t[:])
            else:
                # t = nrm*(partA+partB); accumulate BN stats
                nc.vector.tensor_tensor(out=tsl, in0=tsl, in1=ps_t[:], op=OP.add)
                nc.vector.tensor_scalar_mul(tsl, tsl, nrm_sb[:, nb:nb + 1])
                sq = work.tile([P, H], f32, tag="tmp")
                nc.vector.tensor_tensor(out=sq[:], in0=tsl, in1=tsl, op=OP.mult)
                nc.vector.tensor_tensor(out=acc_s[:], in0=acc_s[:], in1=tsl, op=OP.add)
                nc.vector.tensor_tensor(out=acc_q[:], in0=acc_q[:], in1=sq[:], op=OP.add)

        # pooling PSUM accumulators (filled inside the last layer's update
        # loop as each block's h finalizes)
        ps_p0 = psC.tile([P, G], f32, tag="p0")
        ps_p1 = psC.tile([P, G], f32, tag="p1")

        # bootstrap layer 0's GEMM + AllGather halves (from encoder hsT)
        for nb in range(0, HB):
            emit_gemm(0, nb)
        emit_ag_half(0)
        a_prefetch(0)
        a_prefetch(1)
        a_prefetch(2)
        for nb in range(HB, BPC):
            emit_gemm(0, nb)
        emit_ag_half(1)

        # ---- layers -----------------------------------------------------
        for l in range(L):
            # Table loads go on the Scalar engine's DMA queue: on the Sync
            # queue the tab-B load (which waits for AllGather B) gets
            # scheduled ahead of the A prefetches and stalls them ~28us.
            nc.scalar.dma_start(
                out=tab_sb[:, 0:NCHUNK * H // 2].rearrange("p (c w) -> p c w", c=NCORE),
                in_=ag_outA[:].rearrange("(c p) w -> p c w", p=P))
            nc.scalar.dma_start(
                out=tab_sb[:, NCHUNK * H // 2:].rearrange("p (c w) -> p c w", c=NCORE),
                in_=ag_outB[:].rearrange("(c p) w -> p c w", p=P))
            nc.vector.memset(acc_s[:], 0.0)
            nc.vector.memset(acc_q[:], 0.0)
            for nb in range(BPC):
                emit_half_chain(nb, 0)
            for nb in range(BPC):
                emit_half_chain(nb, 1)

            # stats: cross-partition reduce, replicate to 128 rows, AllReduce
            ps_s = psB.tile([1, 2 * H], f32, tag="vec")
            nc.tensor.matmul(out=ps_s[:, 0:H], lhsT=ones128[:], rhs=acc_s[:],
                             start=True, stop=True)
            nc.tensor.matmul(out=ps_s[:, H:2 * H], lhsT=ones128[:], rhs=acc_q[:],
                             start=True, stop=True)
            st_sb = scal[:, 6 * H:8 * H]
            nc.vector.tensor_copy(out=st_sb, in_=ps_s[:])
            st_rep = work.tile([P, 2 * H], f32, tag="strep")
            for half in range(2):
                ps_r2 = psB.tile([P, H], f32, tag="vec")
                nc.tensor.matmul(out=ps_r2[:], lhsT=ones1[:],
                                 rhs=st_sb[:, half * H:(half + 1) * H],
                                 start=True, stop=True)
                nc.vector.tensor_copy(out=st_rep[:, half * H:(half + 1) * H],
                                      in_=ps_r2[:])
            nc.sync.dma_start(out=ar_in[:], in_=st_rep[:])
            nc.gpsimd.collective_compute(
                "AllReduce", OP.add, replica_groups=rg,
                ins=[ar_in[:]], outs=[ar_out[:]])
            nc.sync.dma_start(out=stv[:], in_=ar_out[0:1, :])

            # a = gamma*istd ; c = beta - mu*a
            mu = scal[:, H:2 * H]
            var = scal[:, 2 * H:3 * H]
            av = scal[:, 3 * H:4 * H]
            cv = scal[:, 4 * H:5 * H]
            msq = scal[:, 5 * H:6 * H]
            nc.vector.tensor_scalar_mul(mu, stv[:, 0:H], 1.0 / N)
            nc.vector.tensor_scalar_mul(var, stv[:, H:2 * H], 1.0 / N)
            nc.vector.tensor_tensor(out=msq, in0=mu, in1=mu, op=OP.mult)
            nc.vector.tensor_tensor(out=var, in0=var, in1=msq, op=OP.subtract)
            nc.vector.tensor_scalar_add(var, var, BN_EPS)
            nc.vector.reciprocal(out=var, in_=var)
            nc.scalar.activation(out=var, in_=var, func=FT.Sqrt)  # istd
            nc.vector.tensor_tensor(out=av, in0=var,
                                    in1=gb_sb[:, l * H:(l + 1) * H], op=OP.mult)
            nc.vector.tensor_tensor(out=msq, in0=mu, in1=av, op=OP.mult)
            nc.vector.tensor_tensor(out=cv, in0=gb_sb[:, (L + l) * H:(L + l + 1) * H],
                                    in1=msq, op=OP.subtract)
            ps_a = psB.tile([P, H], f32, tag="vec")
            nc.tensor.matmul(out=ps_a[:], lhsT=ones1[:], rhs=av, start=True, stop=True)
            nc.vector.tensor_copy(out=a_rep[:], in_=ps_a[:])
            ps_c = psB.tile([P, H], f32, tag="vec")
            nc.tensor.matmul(out=ps_c[:], lhsT=ones1[:], rhs=cv, start=True, stop=True)
            nc.vector.tensor_copy(out=c_rep[:], in_=ps_c[:])

            # h = relu(t*a + c) + h ; immediately GEMM the updated block for
            # the next layer and post the AllGather halves as soon as each
            # half's blocks are done.
            for nb in range(BPC):
                tsl = t_all[:, nb * H:(nb + 1) * H]
                u = work.tile([P, H], f32, tag="tmp")
                nc.vector.tensor_tensor(out=u[:], in0=tsl, in1=a_rep[:], op=OP.mult)
                nc.vector.tensor_tensor(out=u[:], in0=u[:], in1=c_rep[:], op=OP.add)
                r = work.tile([P, H], f32, tag="tmp2")
                nc.scalar.activation(out=r[:], in_=u[:], func=FT.Relu)
                nc.vector.tensor_tensor(out=hslice(nb), in0=hslice(nb), in1=r[:], op=OP.add)
                if l < L - 1:
                    emit_hs_transpose(nb)
                    emit_gemm(l + 1, nb)
                    if nb == HB - 1:
                        emit_ag_half(0)
                        a_prefetch(0)
                        a_prefetch(1)
                        a_prefetch(2)
                    elif nb == BPC - 1:
                        emit_ag_half(1)
                else:
                    # last layer: pool matmuls per block as h finalizes
                    hb_t = work.tile([P, H], bf16, tag="hb")
                    nc.vector.tensor_copy(out=hb_t[:], in_=hslice(nb))
                    pssl = psel_sb[:, nb * G:(nb + 1) * G]
                    nc.tensor.matmul(out=ps_p0[:], lhsT=hb_t[:, 0:P], rhs=pssl,
                                     start=(nb == 0), stop=(nb == BPC - 1))
                    nc.tensor.matmul(out=ps_p1[:], lhsT=hb_t[:, P:2 * P], rhs=pssl,
                                     start=(nb == 0), stop=(nb == BPC - 1))

        # ---- pooling readout (matmuls already accumulated in-layer) -----
        g0 = work.tile([P, G], f32, tag="g0")
        g1 = work.tile([P, G], f32, tag="g1")
        nc.vector.tensor_copy(out=g0[:], in_=ps_p0[:])
        nc.vector.tensor_copy(out=g1[:], in_=ps_p1[:])
        nc.sync.dma_start(out=pr_in[0:P, :], in_=g0[:])
        nc.sync.dma_start(out=pr_in[P:2 * P, :], in_=g1[:])
        nc.gpsimd.collective_compute(
            "AllReduce", OP.add, replica_groups=rg,
            ins=[pr_in[:]], outs=[pr_out[:]])
        nc.sync.dma_start(out=g0[:], in_=pr_out[0:P, :])
        nc.sync.dma_start(out=g1[:], in_=pr_out[P:2 * P, :])
        ps_r = psB.tile([P, G], f32, tag="vec")
        nc.tensor.matmul(out=ps_r[:], lhsT=ones1[:], rhs=rcnt_sb[:], start=True, stop=True)
        rc_rep = work.tile([P, G], f32, tag="rc_rep")
        nc.vector.tensor_copy(out=rc_rep[:], in_=ps_r[:])
        nc.vector.tensor_tensor(out=g0[:], in0=g0[:], in1=rc_rep[:], op=OP.mult)
        nc.vector.tensor_tensor(out=g1[:], in0=g1[:], in1=rc_rep[:], op=OP.mult)

        # MLP head (transposed: weights are lhsT, graphs along free dim)
        ps1 = psB.tile([P, G], f32, tag="vec")
        nc.tensor.matmul(out=ps1[:], lhsT=w1_sb[:, 0:P], rhs=g0[:], start=True, stop=False)
        nc.tensor.matmul(out=ps1[:], lhsT=w1_sb[:, P:2 * P], rhs=g1[:], start=False, stop=True)
        y1 = work.tile([P, G], f32, tag="y1")
        nc.scalar.activation(out=y1[:], in_=ps1[:], func=FT.Relu, bias=b1_sb[:, 0:1])
        ps2 = psB.tile([64, G], f32, tag="vec")
        nc.tensor.matmul(out=ps2[:], lhsT=w2_sb[:], rhs=y1[:], start=True, stop=True)
        y2 = work.tile([64, G], f32, tag="y2")
        nc.scalar.activation(out=y2[:], in_=ps2[:], func=FT.Relu, bias=b2_sb[:, 0:1])
        ps3 = psB.tile([1, G], f32, tag="vec")
        nc.tensor.matmul(out=ps3[:], lhsT=w3_sb[:], rhs=y2[:], start=True, stop=True)
        y3 = work.tile([1, G], f32, tag="y3")
        nc.vector.tensor_scalar_add(y3[:], ps3[:], b3_sb[0:1, 0:1])
        nc.sync.dma_start(out=d_out[:], in_=y3[:])

    nc.compile()
    return nc


# --------------------------------------------------------------------------
# entry point
# --------------------------------------------------------------------------

def kernel(x, edge_index, batch_ids, emb, W, b, gamma, beta,
           mlp_W1, mlp_b1, mlp_W2, mlp_b2, mlp_W3, mlp_b3,
           _trace=False, _trace_kwargs=None):
    # NB: reference BN subtracts the per-channel mean, so the additive bias b
    # cancels exactly and is not needed by the device program.
    in_maps = _preprocess(x, edge_index, batch_ids, emb, W, gamma, beta,
                          mlp_W1, mlp_b1, mlp_W2, mlp_b2, mlp_W3, mlp_b3)
    if "nc" not in _compiled:
        _compiled["nc"] = _build()
    nc = _compiled["nc"]
    kw = {}
    if _trace:
        kw = dict(trace=True, **(_trace_kwargs or {}))
    res = run_bass_kernel_spmd(nc, in_maps, core_ids=list(range(NCORE)), **kw)
    out = np.asarray(res.results[0]["out"], np.float32).reshape(G, 1)
    kernel._last_results = res
    return out
